# revision 1
# baseline (speedup 1.0000x reference)
"""Trainium2 Bass kernel for 3D deformable attention (8 NeuronCores).

Sharding: core i handles batch b = i // 4 and head-pair j = i % 4
(heads 2j, 2j+1, i.e. value/out channels [64j, 64j+64)).

Per-core device pipeline:
  1. value projection  v = W_val[64j:64j+64] @ value[b]  (PE, voxel-stationary
     so the output lands voxel-major), + b_val, stored to DRAM as two
     head-blocks of [NVOX+8, 32] (VDT dtype).
  2. coords = qs^T @ Wcat^T + [48*ref | ones*bias]  (PE) giving, per query,
     30 pixel coords (2 heads x 5 pts x 3 axes, biased +64) and 10 logits.
  3. DVE/ACT: softmax over 5 points, clamped trilinear corner weights for
     x/y, z handled as a W-slot "hat" over a 4-voxel-aligned window, int16
     gather row indices, combined mask m[r, k] = aw*wx*wy*wz[k].
  4. idx shuffle via DRAM round trip into dma_gather's 16-partition-wrapped
     layout; per (query-subtile, head) one dma_gather of 2560 rows
     (W voxels x 32ch each).
  5. DVE: P = G * mask (broadcast over channel), strided-AP reduce over
     (p,dx,dy,k) -> S[q, 64].
  6. PE transpose of S, then out^T = Wout_cols^T @ S^T, DMA out.
Host combines: out[b] = sum_j outp_j^T + b_out.
"""
import numpy as np

import concourse.bass as bass
import concourse.mybir as mybir
from concourse import bacc, tile
from concourse.masks import make_identity
from contextlib import ExitStack

F32 = mybir.dt.float32
I16 = mybir.dt.int16
AX = mybir.AxisListType
OP = mybir.AluOpType
ACT = mybir.ActivationFunctionType

H, P = 8, 5
NQ, C, GRID = 4096, 256, 48
NVOX = GRID ** 3            # 110592
NSUB = 4                    # query subtiles (of 128) per supertile
TQ = 128 * NSUB             # 512
NSUP = NQ // TQ             # 8
VSUP = 1024                 # voxels per value-proj supertile
NVSUP = NVOX // VSUP        # 108
NR = 4                      # value supertiles per DRAM flush

# gather configuration
GATHER_FP32 = False
VDT = F32 if GATHER_FP32 else mybir.dt.bfloat16
W = 6 if GATHER_FP32 else 8         # voxels per gathered row
SROW = 4                            # voxels per row step (4*32*VDT bytes % 256 == 0)
NROWH = NVOX // SROW                # 27648 rows per head (< 32768 for int16)
BSV = NVOX + 8                      # voxels per head block incl pad
NIDX = 20 * 128                     # rows per (subtile, head) gather

DEBUG = False

_NC_CACHE = None


def build_nc():
    nc = bacc.Bacc("TRN2", target_bir_lowering=False, debug=False, num_devices=8)

    value_in = nc.dram_tensor("value_in", [C, NVOX], F32, kind="ExternalInput")
    qT = nc.dram_tensor("qT", [C, NQ], F32, kind="ExternalInput")
    pT = nc.dram_tensor("pT", [C, NQ], F32, kind="ExternalInput")
    refT = nc.dram_tensor("refT", [4, NQ], F32, kind="ExternalInput")
    wcat = nc.dram_tensor("wcat", [C, 40], F32, kind="ExternalInput")
    ref_rhs = nc.dram_tensor("ref_rhs", [4, 40], F32, kind="ExternalInput")
    wval = nc.dram_tensor("wval", [C, 64], F32, kind="ExternalInput")
    bval = nc.dram_tensor("bval", [128, 64], F32, kind="ExternalInput")
    wout = nc.dram_tensor("wout", [64, C], F32, kind="ExternalInput")
    zoff = nc.dram_tensor("zoff", [128, W], F32, kind="ExternalInput")
    outp = nc.dram_tensor("outp", [C, NQ], F32, kind="ExternalOutput")
    vflat = nc.dram_tensor("vflat", [2 * BSV * 32], VDT)
    idxscr = nc.dram_tensor("idxscr", [NSUP * 128, 160], I16)

    dbg = {}
    if DEBUG:
        for nm, sh in [("d_coords", [128, 160]), ("d_mask", [128, NSUB * 40 * W]),
                       ("d_idxf", [128, 160]), ("d_S", [128, 64])]:
            dbg[nm] = nc.dram_tensor(nm, sh, F32, kind="ExternalOutput")
        dbg["d_G"] = nc.dram_tensor("d_G", [128, 20 * W * 32], VDT,
                                    kind="ExternalOutput")

    vec = nc.vector
    act = nc.scalar

    with tile.TileContext(nc) as tc, ExitStack() as ctx:
        const = ctx.enter_context(tc.tile_pool(name="const", bufs=1))
        vpool = ctx.enter_context(tc.tile_pool(name="vpool", bufs=2))
        qpool = ctx.enter_context(tc.tile_pool(name="qpool", bufs=2))
        gpool = ctx.enter_context(tc.tile_pool(name="gpool", bufs=2))
        opool = ctx.enter_context(tc.tile_pool(name="opool", bufs=2))
        ps_v = ctx.enter_context(tc.tile_pool(name="ps_v", bufs=2, space="PSUM"))
        ps_c = ctx.enter_context(tc.tile_pool(name="ps_c", bufs=2, space="PSUM"))
        ps_t = ctx.enter_context(tc.tile_pool(name="ps_t", bufs=2, space="PSUM"))

        # ---- constants into SBUF ----
        wcat_sb = [const.tile([128, 40], F32, tag=f"wcat{k}", name=f"wcat{k}")
                   for k in range(2)]
        for k in range(2):
            nc.sync.dma_start(out=wcat_sb[k][:], in_=wcat[k * 128:(k + 1) * 128, :])
        refrhs_sb = const.tile([4, 40], F32, tag="refrhs", name="refrhs")
        nc.sync.dma_start(out=refrhs_sb[:], in_=ref_rhs[:])
        wval_sb = [const.tile([128, 64], F32, tag=f"wval{k}", name=f"wval{k}")
                   for k in range(2)]
        for k in range(2):
            nc.sync.dma_start(out=wval_sb[k][:], in_=wval[k * 128:(k + 1) * 128, :])
        bval_sb = const.tile([128, 64], F32, tag="bval", name="bval")
        nc.sync.dma_start(out=bval_sb[:], in_=bval[:])
        wout_sb = const.tile([64, C], F32, tag="wout", name="wout")
        nc.sync.dma_start(out=wout_sb[:], in_=wout[:])
        zoff_sb = const.tile([128, W], F32, tag="zoff", name="zoff")
        nc.sync.dma_start(out=zoff_sb[:], in_=zoff[:])
        ident = const.tile([128, 128], F32, tag="ident", name="ident")
        make_identity(nc, ident[:])

        # persistent big buffers
        qs_sb = [const.tile([128, NQ], F32, tag=f"qs{k}", name=f"qs{k}")
                 for k in range(2)]
        ref_sb = const.tile([4, NQ], F32, tag="refq", name="refq")
        st_sb = const.tile([64, NQ], F32, tag="st", name="st")

        # ---- stage Q0: load q, pos, ref; qs = q + p ----
        for k in range(2):
            for half in range(2):
                sl = slice(half * (NQ // 2), (half + 1) * (NQ // 2))
                ptmp = qpool.tile([128, NQ // 2], F32, tag="ptmp", name="ptmp")
                nc.sync.dma_start(out=qs_sb[k][:, sl],
                                  in_=qT[k * 128:(k + 1) * 128, sl])
                nc.sync.dma_start(out=ptmp[:], in_=pT[k * 128:(k + 1) * 128, sl])
                vec.tensor_tensor(out=qs_sb[k][:, sl], in0=qs_sb[k][:, sl],
                                  in1=ptmp[:], op=OP.add)
        nc.sync.dma_start(out=ref_sb[:], in_=refT[:])

        # ---- stage V: value projection ----
        # zero the pad voxels at the end of each head block
        zpad = const.tile([8, 32], VDT, tag="zpad", name="zpad")
        vec.memset(zpad[:], 0.0)
        vflat_r = vflat[:].rearrange("(v c) -> v c", c=32)
        for hl in range(2):
            nc.sync.dma_start(
                out=vflat_r[hl * BSV + NVOX:hl * BSV + NVOX + 8, :], in_=zpad[:])
        for vg in range(NVSUP // NR):          # flush groups of NR supertiles
            vb = [vpool.tile([128, NR * 256], VDT, tag=f"vb{hl}", name=f"vb{hl}")
                  for hl in range(2)]
            for i in range(NR):
                vt = vg * NR + i
                vin = [vpool.tile([128, VSUP], F32, tag=f"vin{k}", name=f"vin{k}")
                       for k in range(2)]
                for k in range(2):
                    nc.sync.dma_start(
                        out=vin[k][:],
                        in_=value_in[k * 128:(k + 1) * 128,
                                     vt * VSUP:(vt + 1) * VSUP])
                psv = ps_v.tile([128, 512], F32, tag="psv", name="psv")
                for s in range(8):
                    lhs0 = vin[0][:].rearrange("p (v e) -> p e v", e=8)[:, s, :]
                    lhs1 = vin[1][:].rearrange("p (v e) -> p e v", e=8)[:, s, :]
                    nc.tensor.matmul(psv[:, s * 64:(s + 1) * 64], lhs0,
                                     wval_sb[0][:], start=True, stop=False)
                    nc.tensor.matmul(psv[:, s * 64:(s + 1) * 64], lhs1,
                                     wval_sb[1][:], start=False, stop=True)
                # split heads, add bias, pack [128, (s,c)=256] per head
                psr = psv[:].rearrange("p (s hc) -> p s hc", s=8)
                for hl in range(2):
                    bv = bval_sb[:, hl * 32:(hl + 1) * 32] \
                        .unsqueeze(1).to_broadcast([128, 8, 32])
                    vec.tensor_tensor(
                        out=vb[hl][:, i * 256:(i + 1) * 256]
                            .rearrange("p (s c) -> p s c", s=8),
                        in0=psr[:, :, hl * 32:(hl + 1) * 32],
                        in1=bv, op=OP.add)
            # flush NR supertiles (NR*1024 voxels) per head
            for hl in range(2):
                base = hl * BSV + vg * NR * VSUP
                dst = vflat_r[base:base + NR * VSUP, :] \
                    .rearrange("(i p s) c -> p i (s c)", i=NR, p=128)
                nc.sync.dma_start(out=dst, in_=vb[hl][:]
                                  .rearrange("p (i sc) -> p i sc", i=NR))

        # ---- stage Q: per supertile of 512 queries ----
        for g in range(NSUP):
            q0 = g * TQ
            psc = ps_c.tile([128, 160], F32, tag="psc", name="psc")
            for s in range(NSUB):
                qsl = slice(q0 + s * 128, q0 + (s + 1) * 128)
                nc.tensor.matmul(psc[:, s * 40:(s + 1) * 40],
                                 qs_sb[0][:, qsl], wcat_sb[0][:],
                                 start=True, stop=False)
                nc.tensor.matmul(psc[:, s * 40:(s + 1) * 40],
                                 qs_sb[1][:, qsl], wcat_sb[1][:],
                                 start=False, stop=False)
                nc.tensor.matmul(psc[:, s * 40:(s + 1) * 40],
                                 ref_sb[:, qsl], refrhs_sb[:],
                                 start=False, stop=True)
            coords = qpool.tile([128, 160], F32, tag="coords", name="coords")
            act.activation(out=coords[:], in_=psc[:], func=ACT.Copy)
            if DEBUG and g == 0:
                nc.sync.dma_start(out=dbg["d_coords"][:], in_=coords[:])

            co = coords[:].rearrange("p (s r) -> p s r", s=NSUB)
            pix = co[:, :, 0:30]                        # (s, hp*ax)
            logit = co[:, :, 30:40]                     # (s, hp)

            # softmax over P
            exlog = qpool.tile([128, NSUB * 10], F32, tag="exlog", name="exlog")
            act.activation(out=exlog[:], in_=logit, func=ACT.Exp)
            ex4 = exlog[:].rearrange("p (s h q) -> p s h q", s=NSUB, h=2)
            sums = qpool.tile([128, NSUB * 2], F32, tag="sums", name="sums")
            vec.tensor_reduce(out=sums[:].rearrange("p (s h) -> p s h", s=NSUB),
                              in_=ex4, axis=AX.X, op=OP.add)
            rsum = qpool.tile([128, NSUB * 2], F32, tag="rsum", name="rsum")
            vec.reciprocal(out=rsum[:], in_=sums[:])
            aw = qpool.tile([128, NSUB * 10], F32, tag="aw", name="aw")
            vec.tensor_tensor(
                out=aw[:].rearrange("p (sh q) -> p sh q", q=5),
                in0=exlog[:].rearrange("p (sh q) -> p sh q", q=5),
                in1=rsum[:].unsqueeze(2).to_broadcast([128, NSUB * 2, 5]),
                op=OP.mult)

            # corner math on the 30 pixel rows
            NPX = NSUB * 30
            # flo = round(pix - 0.5) via the 2^23 magic add (== floor except
            # exactly-integer pix, where the phantom corner gets zero weight)
            flo = qpool.tile([128, NPX], F32, tag="flo", name="flo")
            vec.tensor_scalar(out=flo[:].rearrange("p (s r) -> p s r", s=NSUB),
                              in0=pix, scalar1=8388607.5, scalar2=8388608.0,
                              op0=OP.add, op1=OP.subtract)
            fl3 = flo[:].rearrange("p (s r) -> p s r", s=NSUB)
            frac = qpool.tile([128, NPX], F32, tag="frac", name="frac")
            vec.tensor_tensor(out=frac[:].rearrange("p (s r) -> p s r", s=NSUB),
                              in0=pix, in1=fl3, op=OP.subtract)
            fr3 = frac[:].rearrange("p (s r) -> p s r", s=NSUB)
            # gcorn: (s, hp, ax, dx) — clamped corner coords (biased +64)
            gcorn = qpool.tile([128, NPX * 2], F32, tag="gcorn", name="gcorn")
            gc4 = gcorn[:].rearrange("p (s r d) -> p s r d", s=NSUB, d=2)
            vec.tensor_scalar(out=gc4[:, :, :, 0], in0=fl3,
                              scalar1=64.0, scalar2=111.0, op0=OP.max, op1=OP.min)
            g1m = qpool.tile([128, NPX], F32, tag="g1m", name="g1m")
            vec.tensor_scalar(out=g1m[:], in0=flo[:],
                              scalar1=63.0, scalar2=110.0, op0=OP.max, op1=OP.min)
            vec.tensor_scalar(out=gc4[:, :, :, 1], in0=g1m[:]
                              .rearrange("p (s r) -> p s r", s=NSUB),
                              scalar1=1.0, scalar2=None, op0=OP.add)
            # validity via clip-equality
            v0 = qpool.tile([128, NPX], F32, tag="v0", name="v0")
            vec.tensor_tensor(out=v0[:].rearrange("p (s r) -> p s r", s=NSUB),
                              in0=gc4[:, :, :, 0], in1=fl3, op=OP.is_equal)
            v1 = qpool.tile([128, NPX], F32, tag="v1", name="v1")
            vec.tensor_tensor(out=v1[:], in0=g1m[:], in1=flo[:], op=OP.is_equal)
            # corner weights (x/y rows used; z rows ignored later)
            om = qpool.tile([128, NPX], F32, tag="om", name="om")
            vec.tensor_scalar(out=om[:], in0=frac[:], scalar1=-1.0, scalar2=1.0,
                              op0=OP.mult, op1=OP.add)
            wcorn = qpool.tile([128, NPX * 2], F32, tag="wcorn", name="wcorn")
            wc4 = wcorn[:].rearrange("p (s r d) -> p s r d", s=NSUB, d=2)
            vec.tensor_tensor(out=wc4[:, :, :, 0],
                              in0=om[:].rearrange("p (s r) -> p s r", s=NSUB),
                              in1=v0[:].rearrange("p (s r) -> p s r", s=NSUB),
                              op=OP.mult)
            vec.tensor_tensor(out=wc4[:, :, :, 1],
                              in0=fr3,
                              in1=v1[:].rearrange("p (s r) -> p s r", s=NSUB),
                              op=OP.mult)

            # z window: rzq = floor((gz-64)/4) in [0,11]; W-slot hat weights
            gc6 = gcorn[:].rearrange("p (s hp a d) -> p s hp a d",
                                     s=NSUB, hp=10, a=3)
            gz = gc6[:, :, :, 2, 0]
            pz = co[:, :, 0:30].rearrange("p s (hp a) -> p s hp a", a=3)[:, :, :, 2]
            rzq = qpool.tile([128, NSUB * 10], F32, tag="rzq", name="rzq")
            tq = qpool.tile([128, NSUB * 10], F32, tag="tq", name="tq")
            vec.tensor_scalar(out=tq[:].rearrange("p (s h) -> p s h", s=NSUB),
                              in0=gz, scalar1=0.25, scalar2=16.375,
                              op0=OP.mult, op1=OP.subtract)
            vec.tensor_scalar(out=rzq[:], in0=tq[:],
                              scalar1=8388624.0, scalar2=8388624.0,
                              op0=OP.add, op1=OP.subtract)
            # d0 = (4*rzq + 64) - pz ; dk = d0 + k
            zb4 = qpool.tile([128, NSUB * 10], F32, tag="zb4", name="zb4")
            vec.tensor_scalar(out=zb4[:], in0=rzq[:], scalar1=4.0, scalar2=64.0,
                              op0=OP.mult, op1=OP.add)
            d0 = qpool.tile([128, NSUB * 10], F32, tag="d0", name="d0")
            vec.tensor_tensor(out=d0[:].rearrange("p (s h) -> p s h", s=NSUB),
                              in0=zb4[:].rearrange("p (s h) -> p s h", s=NSUB),
                              in1=pz, op=OP.subtract)
            dk = qpool.tile([128, NSUB * 10 * W], F32, tag="dk", name="dk")
            vec.tensor_tensor(
                out=dk[:].rearrange("p (sh k) -> p sh k", k=W),
                in0=d0[:].unsqueeze(2).to_broadcast([128, NSUB * 10, W]),
                in1=zoff_sb[:].unsqueeze(1).to_broadcast([128, NSUB * 10, W]),
                op=OP.add)
            adk = qpool.tile([128, NSUB * 10 * W], F32, tag="adk", name="adk")
            act.activation(out=adk[:], in_=dk[:], func=ACT.Abs)
            hat = qpool.tile([128, NSUB * 10 * W], F32, tag="hat", name="hat")
            act.activation(out=hat[:], in_=adk[:], func=ACT.Relu,
                           scale=-1.0, bias=1.0)
            # upper bound: slot z 4*rzq+64+k <= 111  <=>  dk <= 111 - pz
            ub = qpool.tile([128, NSUB * 10], F32, tag="ub", name="ub")
            vec.tensor_scalar(out=ub[:].rearrange("p (s h) -> p s h", s=NSUB),
                              in0=pz, scalar1=-1.0, scalar2=111.0,
                              op0=OP.mult, op1=OP.add)
            vub = qpool.tile([128, NSUB * 10 * W], F32, tag="vub", name="vub")
            vec.tensor_tensor(
                out=vub[:].rearrange("p (sh k) -> p sh k", k=W),
                in0=dk[:].rearrange("p (sh k) -> p sh k", k=W),
                in1=ub[:].unsqueeze(2).to_broadcast([128, NSUB * 10, W]),
                op=OP.is_le)
            wz = qpool.tile([128, NSUB * 10 * W], F32, tag="wz", name="wz")
            vec.tensor_tensor(out=wz[:], in0=hat[:], in1=vub[:], op=OP.mult)

            # mask assembly: m[s, hp, dx, dy, k]
            wc6 = wcorn[:].rearrange("p (s hp a d) -> p s hp a d",
                                     s=NSUB, hp=10, a=3)
            wx = wc6[:, :, :, 0, :]                     # (s, hp, dx)
            wy = wc6[:, :, :, 1, :]                     # (s, hp, dy)
            t1 = qpool.tile([128, NSUB * 40], F32, tag="t1", name="t1")
            vec.tensor_tensor(
                out=t1[:].rearrange("p (s hp x y) -> p s hp x y",
                                    s=NSUB, hp=10, x=2),
                in0=wx.unsqueeze(4).to_broadcast([128, NSUB, 10, 2, 2]),
                in1=wy.unsqueeze(3).to_broadcast([128, NSUB, 10, 2, 2]),
                op=OP.mult)
            t2 = qpool.tile([128, NSUB * 40], F32, tag="t2", name="t2")
            aw4 = aw[:].rearrange("p (s hp) -> p s hp", s=NSUB) \
                .unsqueeze(3).to_broadcast([128, NSUB, 10, 4])
            vec.tensor_tensor(
                out=t2[:].rearrange("p (s hp xy) -> p s hp xy", s=NSUB, hp=10),
                in0=t1[:].rearrange("p (s hp xy) -> p s hp xy", s=NSUB, hp=10),
                in1=aw4, op=OP.mult)
            mask = qpool.tile([128, NSUB * 40 * W], F32, tag="mask", name="mask")
            vec.tensor_tensor(
                out=mask[:].rearrange("p (shp xy k) -> p shp xy k", xy=4, k=W),
                in0=t2[:].rearrange("p (shp xy) -> p shp xy", xy=4)
                    .unsqueeze(3).to_broadcast([128, NSUB * 10, 4, W]),
                in1=wz[:].rearrange("p (shp k) -> p shp k", k=W)
                    .unsqueeze(2).to_broadcast([128, NSUB * 10, 4, W]),
                op=OP.mult)
            maskb = qpool.tile([128, NSUB * 40 * W], VDT, tag="maskb", name="maskb")
            vec.tensor_copy(out=maskb[:], in_=mask[:])
            if DEBUG and g == 0:
                nc.sync.dma_start(out=dbg["d_mask"][:], in_=mask[:])

            # gather row indices: 576*(gx-64) + 12*(gy-64) + rzq
            gx = gc6[:, :, :, 0, :]                     # (s, hp, dx)
            gy = gc6[:, :, :, 1, :]                     # (s, hp, dy)
            ga = qpool.tile([128, NSUB * 20], F32, tag="ga", name="ga")
            vec.tensor_scalar(out=ga[:].rearrange("p (s hp d) -> p s hp d",
                                                  s=NSUB, hp=10),
                              in0=gx, scalar1=576.0, scalar2=37632.0,
                              op0=OP.mult, op1=OP.subtract)
            gb = qpool.tile([128, NSUB * 20], F32, tag="gb", name="gb")
            vec.tensor_scalar(out=gb[:].rearrange("p (s hp d) -> p s hp d",
                                                  s=NSUB, hp=10),
                              in0=gy, scalar1=12.0, scalar2=None, op0=OP.mult)
            t4 = qpool.tile([128, NSUB * 40], F32, tag="t4", name="t4")
            vec.tensor_tensor(
                out=t4[:].rearrange("p (s hp x y) -> p s hp x y",
                                    s=NSUB, hp=10, x=2),
                in0=ga[:].rearrange("p (s hp x) -> p s hp x", s=NSUB, hp=10)
                    .unsqueeze(4).to_broadcast([128, NSUB, 10, 2, 2]),
                in1=gb[:].rearrange("p (s hp y) -> p s hp y", s=NSUB, hp=10)
                    .unsqueeze(3).to_broadcast([128, NSUB, 10, 2, 2]),
                op=OP.add)
            idxf = qpool.tile([128, NSUB * 40], F32, tag="idxf", name="idxf")
            vec.tensor_tensor(
                out=idxf[:].rearrange("p (s hp xy) -> p s hp xy", s=NSUB, hp=10),
                in0=t4[:].rearrange("p (s hp xy) -> p s hp xy", s=NSUB, hp=10),
                in1=rzq[:].rearrange("p (s hp) -> p s hp", s=NSUB)
                    .unsqueeze(3).to_broadcast([128, NSUB, 10, 4]),
                op=OP.add)
            idx16 = qpool.tile([128, NSUB * 40], I16, tag="idx16", name="idx16")
            vec.tensor_copy(out=idx16[:], in_=idxf[:])
            if DEBUG and g == 0:
                nc.sync.dma_start(out=dbg["d_idxf"][:], in_=idxf[:])

            # idx shuffle via DRAM: store [128, 160], reload 16-partition-wrapped
            nc.sync.dma_start(out=idxscr[g * 128:(g + 1) * 128, :], in_=idx16[:])
            idxw = gpool.tile([128, NSUB * 2 * 160], I16, tag="idxw", name="idxw")
            scr = idxscr[g * 128:(g + 1) * 128, :]
            for grp in range(8):
                # dest[p, sub, hl, r*8+s2] = scr[s2*16+p, sub*40+hl*20+r]
                src = bass.AP(scr.tensor, scr.offset,
                              [[160, 16], [40, NSUB], [20, 2], [1, 20],
                               [16 * 160, 8]])
                dst = idxw[grp * 16:(grp + 1) * 16, :] \
                    .rearrange("p (sub hl r s2) -> p sub hl r s2",
                               sub=NSUB, hl=2, r=20)
                nc.sync.dma_start(out=dst, in_=src)

            # gather + weighted reduce per (subtile, head)
            S = qpool.tile([128, NSUB * 64], F32, tag="S", name="S")
            for s in range(NSUB):
                for hl in range(2):
                    G = gpool.tile([128, 20 * W * 32], VDT, tag="G", name="G")
                    in_g = bass.AP(vflat[:].tensor, hl * BSV * 32,
                                   [[SROW * 32, NROWH], [1, W * 32]])
                    nc.gpsimd.dma_gather(
                        out_ap=G[:].rearrange("p (i e) -> p i e", i=20),
                        in_ap=in_g,
                        idxs_ap=idxw[:, (s * 2 + hl) * 160:(s * 2 + hl + 1) * 160],
                        num_idxs=NIDX, num_idxs_reg=NIDX,
                        elem_size=W * 32, elem_step=SROW * 32,
                        single_packet=False)
                    if DEBUG and g == 0 and s == 0 and hl == 0:
                        nc.sync.dma_start(out=dbg["d_G"][:], in_=G[:])
                    Pt = gpool.tile([128, 20 * W * 32], VDT, tag="Pt", name="Pt")
                    moff = s * (40 * W) + hl * (5 * 4 * W)
                    mg = bass.AP(maskb[:].tensor, maskb[:].offset + moff,
                                 [[NSUB * 40 * W, 128], [1, 20 * W], [0, 32]])
                    vec.tensor_tensor(
                        out=Pt[:].rearrange("p (m c) -> p m c", c=32),
                        in0=G[:].rearrange("p (m c) -> p m c", c=32),
                        in1=mg, op=OP.mult)
                    vec.tensor_reduce(
                        out=S[:, s * 64 + hl * 32:s * 64 + hl * 32 + 32],
                        in_=Pt[:].rearrange("p (m c) -> p c m", c=32),
                        axis=AX.X, op=OP.add)
            if DEBUG and g == 0:
                nc.sync.dma_start(out=dbg["d_S"][:], in_=S[:, 0:64])

            # transpose S [128, 64] -> [64, 128] per subtile
            for s in range(NSUB):
                pst = ps_t.tile([64, 128], F32, tag="pst", name="pst")
                nc.tensor.transpose(pst[:], S[:, s * 64:(s + 1) * 64], ident[:])
                act.activation(out=st_sb[:, q0 + s * 128:q0 + (s + 1) * 128],
                               in_=pst[:], func=ACT.Copy)

        # ---- GEMM2: outT = wout^T @ ST ----
        for mc in range(2):
            for ntile in range(NQ // 512):
                ps2 = ps_c.tile([128, 512], F32, tag="ps2", name="ps2")
                nc.tensor.matmul(ps2[:],
                                 wout_sb[:, mc * 128:(mc + 1) * 128],
                                 st_sb[:, ntile * 512:(ntile + 1) * 512],
                                 start=True, stop=True)
                ob = opool.tile([128, 512], F32, tag="ob", name="ob")
                vec.tensor_copy(out=ob[:], in_=ps2[:])
                nc.sync.dma_start(
                    out=outp[mc * 128:(mc + 1) * 128,
                             ntile * 512:(ntile + 1) * 512],
                    in_=ob[:])

    nc.compile()
    return nc


def _prep_core_inputs(inputs, b, j):
    q = np.ascontiguousarray(inputs["query"][b].T, np.float32)
    p = np.ascontiguousarray(inputs["pos"][b].T, np.float32)
    r = np.concatenate([inputs["reference_points"][b].T,
                        np.ones((1, NQ), np.float32)]).astype(np.float32)
    r = np.ascontiguousarray(r)
    value = np.ascontiguousarray(inputs["value"][b].reshape(C, NVOX), np.float32)

    W_off, b_off = inputs["W_off"], inputs["b_off"]
    W_attn, b_attn = inputs["W_attn"], inputs["b_attn"]
    heads = [2 * j, 2 * j + 1]
    rows, biases, refr = [], [], []
    for h in heads:
        for pp in range(P):
            for ax in range(3):
                rows.append(W_off[(h * P + pp) * 3 + ax])
                biases.append(b_off[(h * P + pp) * 3 + ax] - 0.5 + 64.0)
                e = np.zeros(3, np.float32)
                e[ax] = GRID
                refr.append(e)
    for h in heads:
        for pp in range(P):
            rows.append(W_attn[h * P + pp])
            biases.append(b_attn[h * P + pp])
            refr.append(np.zeros(3, np.float32))
    wcat = np.ascontiguousarray(np.stack(rows).T, np.float32)       # (256, 40)
    ref_rhs = np.concatenate(
        [np.stack(refr).T, np.asarray(biases, np.float32)[None, :]])
    ref_rhs = np.ascontiguousarray(ref_rhs, np.float32)             # (4, 40)

    wval = np.ascontiguousarray(inputs["W_val"][64 * j:64 * j + 64].T, np.float32)
    bval = np.ascontiguousarray(
        np.repeat(inputs["b_val"][64 * j:64 * j + 64][None, :], 128, axis=0),
        np.float32)
    wout = np.ascontiguousarray(inputs["W_out"][:, 64 * j:64 * j + 64].T,
                                np.float32)
    zoffs = np.repeat(np.arange(W, dtype=np.float32)[None, :], 128, axis=0)
    return {
        "value_in": value, "qT": q, "pT": p, "refT": r,
        "wcat": wcat, "ref_rhs": ref_rhs,
        "wval": wval, "bval": bval, "wout": wout, "zoff": zoffs,
    }


def get_nc():
    global _NC_CACHE
    if _NC_CACHE is None:
        _NC_CACHE = build_nc()
    return _NC_CACHE


def kernel(**inputs):
    from concourse.bass_utils import run_bass_kernel_spmd

    inputs = {k: np.asarray(v) for k, v in inputs.items()}
    nc = get_nc()
    in_maps = [_prep_core_inputs(inputs, core // 4, core % 4) for core in range(8)]
    res = run_bass_kernel_spmd(nc, in_maps, list(range(8)))
    bs = inputs["query"].shape[0]
    out = np.zeros((bs, NQ, C), np.float32)
    for core in range(8):
        out[core // 4] += res.results[core]["outp"].T
    out += inputs["b_out"][None, None, :].astype(np.float32)
    return out



# revision 5
# speedup vs baseline: 2.4428x; 2.4428x over previous
"""Trainium2 Bass kernel for 3D deformable attention (8 NeuronCores).

Sharding: core i handles batch b = i // 4 and head-pair j = i % 4
(heads 2j, 2j+1, i.e. value/out channels [64j, 64j+64)).

Per-core device pipeline (emission order = coords first so DVE mask work
overlaps the value-projection DMA/PE stage):
  C. per query-supertile (512 q): coords = qs^T @ Wcat^T + [48*ref | b] (PE);
     softmax over 5 points, clamped trilinear corner weights, z-window "hat"
     weights, combined mask m = aw*wx*wy*wz (DVE, bf16); int16 gather row
     indices; idx fold into dma_gather's 16-partition-wrapped layout via a
     contiguous DRAM bounce (320B packets) + DVE column permute.
  V. value projection v = W_val[64j:64j+64] @ value[b] in bf16 (PE,
     voxel-stationary), + b_val; packed per 4-voxel block as (c, v4) and
     flushed to DRAM as two head blocks of [NVOX+8 vox, 32ch] rows.
  G. per (query-subtile, head): one dma_gather of 2560 rows (8 vox x 32ch
     bf16 = 512B each); P = G * mask (DVE bf16, contiguous); two-step
     reduce (over v4 contiguous, then over (pt,xy,blk)) -> S[q, 64].
  O. PE transpose of S, then out^T = Wout^T @ S^T (float32r), DMA out.
Host combines: out[b] = sum_j outp_j^T + b_out.
"""
import numpy as np

import concourse.bass as bass
import concourse.mybir as mybir
from concourse import bacc, tile
from concourse.masks import make_identity
from contextlib import ExitStack

F32 = mybir.dt.float32
F32R = mybir.dt.float32r
I16 = mybir.dt.int16
AX = mybir.AxisListType
OP = mybir.AluOpType
ACT = mybir.ActivationFunctionType

H, P = 8, 5
NQ, C, GRID = 4096, 256, 48
NVOX = GRID ** 3            # 110592
NSUB = 4                    # query subtiles (of 128) per supertile
TQ = 128 * NSUB             # 512
NSUP = NQ // TQ             # 8
VSUP = 1024                 # voxels per value-proj supertile
NVSUP = NVOX // VSUP        # 108
NR = 4                      # value supertiles per DRAM flush

VDT = mybir.dt.bfloat16
W = 8                       # voxels per gathered row
SROW = 4                    # voxels per row step (4*32*2B = 256B)
NROWH = NVOX // SROW        # 27648 rows per head (< 32768 for int16)
BSV = NVOX + 8              # voxels per head block incl pad
NIDX = 20 * 128             # rows per (subtile, head) gather
MCOL = NSUB * 40 * W        # mask columns per supertile (1280)
ICOL = NSUB * 2 * 160       # idx columns per supertile (1280)

_NC_CACHE = None


def build_nc():
    nc = bacc.Bacc("TRN2", target_bir_lowering=False, debug=False, num_devices=8)

    value_in = nc.dram_tensor("value_in", [C, NVOX], VDT, kind="ExternalInput")
    qT = nc.dram_tensor("qT", [C, NQ], F32, kind="ExternalInput")
    pT = nc.dram_tensor("pT", [C, NQ], F32, kind="ExternalInput")
    refT = nc.dram_tensor("refT", [4, NQ], F32, kind="ExternalInput")
    wcat = nc.dram_tensor("wcat", [C, 40], F32, kind="ExternalInput")
    ref_rhs = nc.dram_tensor("ref_rhs", [4, 40], F32, kind="ExternalInput")
    wval = nc.dram_tensor("wval", [C, 64], VDT, kind="ExternalInput")
    bval = nc.dram_tensor("bval", [128, 64], F32, kind="ExternalInput")
    wout = nc.dram_tensor("wout", [64, C], F32, kind="ExternalInput")
    zoff = nc.dram_tensor("zoff", [128, W], F32, kind="ExternalInput")
    outp = nc.dram_tensor("outp", [C, NQ], F32, kind="ExternalOutput")
    vflat = nc.dram_tensor("vflat", [2 * BSV * 32], VDT)
    idxscr = nc.dram_tensor("idxscr", [NSUP * 128, 160], I16)

    vec = nc.vector
    act = nc.scalar

    with tile.TileContext(nc) as tc, ExitStack() as ctx:
        const = ctx.enter_context(tc.tile_pool(name="const", bufs=1))
        vpool = ctx.enter_context(tc.tile_pool(name="vpool", bufs=2))
        qpool = ctx.enter_context(tc.tile_pool(name="qpool", bufs=2))
        gpool = ctx.enter_context(tc.tile_pool(name="gpool", bufs=2))
        opool = ctx.enter_context(tc.tile_pool(name="opool", bufs=2))
        ps_v = ctx.enter_context(tc.tile_pool(name="ps_v", bufs=2, space="PSUM"))
        ps_c = ctx.enter_context(tc.tile_pool(name="ps_c", bufs=2, space="PSUM"))
        ps_t = ctx.enter_context(tc.tile_pool(name="ps_t", bufs=2, space="PSUM"))

        # ---- constants into SBUF ----
        wcat_sb = [const.tile([128, 40], F32, tag=f"wcat{k}", name=f"wcat{k}")
                   for k in range(2)]
        for k in range(2):
            nc.sync.dma_start(out=wcat_sb[k][:], in_=wcat[k * 128:(k + 1) * 128, :])
        refrhs_sb = const.tile([4, 40], F32, tag="refrhs", name="refrhs")
        nc.sync.dma_start(out=refrhs_sb[:], in_=ref_rhs[:])
        wval_sb = [const.tile([128, 64], VDT, tag=f"wval{k}", name=f"wval{k}")
                   for k in range(2)]
        for k in range(2):
            nc.sync.dma_start(out=wval_sb[k][:], in_=wval[k * 128:(k + 1) * 128, :])
        bval_sb = const.tile([128, 64], F32, tag="bval", name="bval")
        nc.sync.dma_start(out=bval_sb[:], in_=bval[:])
        wout_sb = const.tile([64, C], F32, tag="wout", name="wout")
        nc.sync.dma_start(out=wout_sb[:], in_=wout[:])
        zoff_sb = const.tile([128, W], F32, tag="zoff", name="zoff")
        nc.sync.dma_start(out=zoff_sb[:], in_=zoff[:])
        ident = const.tile([128, 128], F32, tag="ident", name="ident")
        make_identity(nc, ident[:])

        # persistent big buffers
        qs_sb = [const.tile([128, NQ], F32, tag=f"qs{k}", name=f"qs{k}")
                 for k in range(2)]
        ref_sb = const.tile([4, NQ], F32, tag="refq", name="refq")
        st_sb = const.tile([64, NQ], F32, tag="st", name="st")
        maskb_all = const.tile([128, NSUP * MCOL], VDT, tag="maskb", name="maskb")
        idxw_all = const.tile([128, NSUP * ICOL], I16, tag="idxw", name="idxw")

        # ---- stage Q0: load q, pos, ref; qs = q + p ----
        for k in range(2):
            for half in range(4):
                sl = slice(half * (NQ // 4), (half + 1) * (NQ // 4))
                ptmp = qpool.tile([128, NQ // 4], F32, tag="ptmp", name="ptmp")
                nc.sync.dma_start(out=qs_sb[k][:, sl],
                                  in_=qT[k * 128:(k + 1) * 128, sl])
                nc.sync.dma_start(out=ptmp[:], in_=pT[k * 128:(k + 1) * 128, sl])
                vec.tensor_tensor(out=qs_sb[k][:, sl], in0=qs_sb[k][:, sl],
                                  in1=ptmp[:], op=OP.add)
        nc.sync.dma_start(out=ref_sb[:], in_=refT[:])

        # ---- stage C: coords / masks / gather indices, all supertiles ----
        for g in range(NSUP):
            q0 = g * TQ
            psc = ps_c.tile([128, 160], F32, tag="psc", name="psc")
            for s in range(NSUB):
                qsl = slice(q0 + s * 128, q0 + (s + 1) * 128)
                nc.tensor.matmul(psc[:, s * 40:(s + 1) * 40],
                                 qs_sb[0][:, qsl], wcat_sb[0][:],
                                 start=True, stop=False)
                nc.tensor.matmul(psc[:, s * 40:(s + 1) * 40],
                                 qs_sb[1][:, qsl], wcat_sb[1][:],
                                 start=False, stop=False)
                nc.tensor.matmul(psc[:, s * 40:(s + 1) * 40],
                                 ref_sb[:, qsl], refrhs_sb[:],
                                 start=False, stop=True)
            coords = qpool.tile([128, 160], F32, tag="coords", name="coords")
            act.activation(out=coords[:], in_=psc[:], func=ACT.Copy)

            co = coords[:].rearrange("p (s r) -> p s r", s=NSUB)
            pix = co[:, :, 0:30]                        # (s, hp*ax)
            logit = co[:, :, 30:40]                     # (s, hp)

            # softmax over P
            exlog = qpool.tile([128, NSUB * 10], F32, tag="exlog", name="exlog")
            act.activation(out=exlog[:], in_=logit, func=ACT.Exp)
            ex4 = exlog[:].rearrange("p (s h q) -> p s h q", s=NSUB, h=2)
            sums = qpool.tile([128, NSUB * 2], F32, tag="sums", name="sums")
            vec.tensor_reduce(out=sums[:].rearrange("p (s h) -> p s h", s=NSUB),
                              in_=ex4, axis=AX.X, op=OP.add)
            rsum = qpool.tile([128, NSUB * 2], F32, tag="rsum", name="rsum")
            vec.reciprocal(out=rsum[:], in_=sums[:])
            aw = qpool.tile([128, NSUB * 10], F32, tag="aw", name="aw")
            vec.tensor_tensor(
                out=aw[:].rearrange("p (sh q) -> p sh q", q=5),
                in0=exlog[:].rearrange("p (sh q) -> p sh q", q=5),
                in1=rsum[:].unsqueeze(2).to_broadcast([128, NSUB * 2, 5]),
                op=OP.mult)

            # corner math on the 30 pixel rows
            NPX = NSUB * 30
            # flo = round(pix - 0.5) via the 2^23 magic add (== floor except
            # exactly-integer pix, where the phantom corner gets zero weight)
            flo = qpool.tile([128, NPX], F32, tag="flo", name="flo")
            vec.tensor_scalar(out=flo[:].rearrange("p (s r) -> p s r", s=NSUB),
                              in0=pix, scalar1=8388607.5, scalar2=8388608.0,
                              op0=OP.add, op1=OP.subtract)
            fl3 = flo[:].rearrange("p (s r) -> p s r", s=NSUB)
            frac = qpool.tile([128, NPX], F32, tag="frac", name="frac")
            vec.tensor_tensor(out=frac[:].rearrange("p (s r) -> p s r", s=NSUB),
                              in0=pix, in1=fl3, op=OP.subtract)
            fr3 = frac[:].rearrange("p (s r) -> p s r", s=NSUB)
            # gcorn: (s, hp, ax, dx) — clamped corner coords (biased +64)
            gcorn = qpool.tile([128, NPX * 2], F32, tag="gcorn", name="gcorn")
            gc4 = gcorn[:].rearrange("p (s r d) -> p s r d", s=NSUB, d=2)
            vec.tensor_scalar(out=gc4[:, :, :, 0], in0=fl3,
                              scalar1=64.0, scalar2=111.0, op0=OP.max, op1=OP.min)
            g1m = qpool.tile([128, NPX], F32, tag="g1m", name="g1m")
            vec.tensor_scalar(out=g1m[:], in0=flo[:],
                              scalar1=63.0, scalar2=110.0, op0=OP.max, op1=OP.min)
            vec.tensor_scalar(out=gc4[:, :, :, 1], in0=g1m[:]
                              .rearrange("p (s r) -> p s r", s=NSUB),
                              scalar1=1.0, scalar2=None, op0=OP.add)
            # validity via clip-equality
            v0 = qpool.tile([128, NPX], F32, tag="v0", name="v0")
            vec.tensor_tensor(out=v0[:].rearrange("p (s r) -> p s r", s=NSUB),
                              in0=gc4[:, :, :, 0], in1=fl3, op=OP.is_equal)
            v1 = qpool.tile([128, NPX], F32, tag="v1", name="v1")
            vec.tensor_tensor(out=v1[:], in0=g1m[:], in1=flo[:], op=OP.is_equal)
            # corner weights (x/y rows used; z rows ignored later)
            om = qpool.tile([128, NPX], F32, tag="om", name="om")
            vec.tensor_scalar(out=om[:], in0=frac[:], scalar1=-1.0, scalar2=1.0,
                              op0=OP.mult, op1=OP.add)
            wcorn = qpool.tile([128, NPX * 2], F32, tag="wcorn", name="wcorn")
            wc4 = wcorn[:].rearrange("p (s r d) -> p s r d", s=NSUB, d=2)
            vec.tensor_tensor(out=wc4[:, :, :, 0],
                              in0=om[:].rearrange("p (s r) -> p s r", s=NSUB),
                              in1=v0[:].rearrange("p (s r) -> p s r", s=NSUB),
                              op=OP.mult)
            vec.tensor_tensor(out=wc4[:, :, :, 1],
                              in0=fr3,
                              in1=v1[:].rearrange("p (s r) -> p s r", s=NSUB),
                              op=OP.mult)

            # z window: rzq = floor((gz-64)/4) in [0,11]; W-slot hat weights
            gc6 = gcorn[:].rearrange("p (s hp a d) -> p s hp a d",
                                     s=NSUB, hp=10, a=3)
            gz = gc6[:, :, :, 2, 0]
            pz = co[:, :, 0:30].rearrange("p s (hp a) -> p s hp a", a=3)[:, :, :, 2]
            rzq = qpool.tile([128, NSUB * 10], F32, tag="rzq", name="rzq")
            tq = qpool.tile([128, NSUB * 10], F32, tag="tq", name="tq")
            vec.tensor_scalar(out=tq[:].rearrange("p (s h) -> p s h", s=NSUB),
                              in0=gz, scalar1=0.25, scalar2=16.375,
                              op0=OP.mult, op1=OP.subtract)
            vec.tensor_scalar(out=rzq[:], in0=tq[:],
                              scalar1=8388624.0, scalar2=8388624.0,
                              op0=OP.add, op1=OP.subtract)
            # d0 = (4*rzq + 64) - pz ; dk = d0 + k
            zb4 = qpool.tile([128, NSUB * 10], F32, tag="zb4", name="zb4")
            vec.tensor_scalar(out=zb4[:], in0=rzq[:], scalar1=4.0, scalar2=64.0,
                              op0=OP.mult, op1=OP.add)
            d0 = qpool.tile([128, NSUB * 10], F32, tag="d0", name="d0")
            vec.tensor_tensor(out=d0[:].rearrange("p (s h) -> p s h", s=NSUB),
                              in0=zb4[:].rearrange("p (s h) -> p s h", s=NSUB),
                              in1=pz, op=OP.subtract)
            dk = qpool.tile([128, NSUB * 10 * W], F32, tag="dk", name="dk")
            vec.tensor_tensor(
                out=dk[:].rearrange("p (sh k) -> p sh k", k=W),
                in0=d0[:].unsqueeze(2).to_broadcast([128, NSUB * 10, W]),
                in1=zoff_sb[:].unsqueeze(1).to_broadcast([128, NSUB * 10, W]),
                op=OP.add)
            adk = qpool.tile([128, NSUB * 10 * W], F32, tag="adk", name="adk")
            act.activation(out=adk[:], in_=dk[:], func=ACT.Abs)
            hat = qpool.tile([128, NSUB * 10 * W], F32, tag="hat", name="hat")
            act.activation(out=hat[:], in_=adk[:], func=ACT.Relu,
                           scale=-1.0, bias=1.0)
            # upper bound: slot z 4*rzq+64+k <= 111  <=>  dk <= 111 - pz
            ub = qpool.tile([128, NSUB * 10], F32, tag="ub", name="ub")
            vec.tensor_scalar(out=ub[:].rearrange("p (s h) -> p s h", s=NSUB),
                              in0=pz, scalar1=-1.0, scalar2=111.0,
                              op0=OP.mult, op1=OP.add)
            vub = qpool.tile([128, NSUB * 10 * W], F32, tag="vub", name="vub")
            vec.tensor_tensor(
                out=vub[:].rearrange("p (sh k) -> p sh k", k=W),
                in0=dk[:].rearrange("p (sh k) -> p sh k", k=W),
                in1=ub[:].unsqueeze(2).to_broadcast([128, NSUB * 10, W]),
                op=OP.is_le)
            wz = qpool.tile([128, NSUB * 10 * W], F32, tag="wz", name="wz")
            vec.tensor_tensor(out=wz[:], in0=hat[:], in1=vub[:], op=OP.mult)

            # mask assembly: m[s, hp, dx, dy, k]
            wc6 = wcorn[:].rearrange("p (s hp a d) -> p s hp a d",
                                     s=NSUB, hp=10, a=3)
            wx = wc6[:, :, :, 0, :]                     # (s, hp, dx)
            wy = wc6[:, :, :, 1, :]                     # (s, hp, dy)
            t1 = qpool.tile([128, NSUB * 40], F32, tag="t1", name="t1")
            vec.tensor_tensor(
                out=t1[:].rearrange("p (s hp x y) -> p s hp x y",
                                    s=NSUB, hp=10, x=2),
                in0=wx.unsqueeze(4).to_broadcast([128, NSUB, 10, 2, 2]),
                in1=wy.unsqueeze(3).to_broadcast([128, NSUB, 10, 2, 2]),
                op=OP.mult)
            t2 = qpool.tile([128, NSUB * 40], F32, tag="t2", name="t2")
            aw4 = aw[:].rearrange("p (s hp) -> p s hp", s=NSUB) \
                .unsqueeze(3).to_broadcast([128, NSUB, 10, 4])
            vec.tensor_tensor(
                out=t2[:].rearrange("p (s hp xy) -> p s hp xy", s=NSUB, hp=10),
                in0=t1[:].rearrange("p (s hp xy) -> p s hp xy", s=NSUB, hp=10),
                in1=aw4, op=OP.mult)
            vec.tensor_tensor(
                out=maskb_all[:, g * MCOL:(g + 1) * MCOL]
                    .rearrange("p (shp xy k) -> p shp xy k", xy=4, k=W),
                in0=t2[:].rearrange("p (shp xy) -> p shp xy", xy=4)
                    .unsqueeze(3).to_broadcast([128, NSUB * 10, 4, W]),
                in1=wz[:].rearrange("p (shp k) -> p shp k", k=W)
                    .unsqueeze(2).to_broadcast([128, NSUB * 10, 4, W]),
                op=OP.mult)

            # gather row indices: 576*(gx-64) + 12*(gy-64) + rzq
            gx = gc6[:, :, :, 0, :]                     # (s, hp, dx)
            gy = gc6[:, :, :, 1, :]                     # (s, hp, dy)
            ga = qpool.tile([128, NSUB * 20], F32, tag="ga", name="ga")
            vec.tensor_scalar(out=ga[:].rearrange("p (s hp d) -> p s hp d",
                                                  s=NSUB, hp=10),
                              in0=gx, scalar1=576.0, scalar2=37632.0,
                              op0=OP.mult, op1=OP.subtract)
            gb = qpool.tile([128, NSUB * 20], F32, tag="gb", name="gb")
            vec.tensor_scalar(out=gb[:].rearrange("p (s hp d) -> p s hp d",
                                                  s=NSUB, hp=10),
                              in0=gy, scalar1=12.0, scalar2=None, op0=OP.mult)
            t4 = qpool.tile([128, NSUB * 40], F32, tag="t4", name="t4")
            vec.tensor_tensor(
                out=t4[:].rearrange("p (s hp x y) -> p s hp x y",
                                    s=NSUB, hp=10, x=2),
                in0=ga[:].rearrange("p (s hp x) -> p s hp x", s=NSUB, hp=10)
                    .unsqueeze(4).to_broadcast([128, NSUB, 10, 2, 2]),
                in1=gb[:].rearrange("p (s hp y) -> p s hp y", s=NSUB, hp=10)
                    .unsqueeze(3).to_broadcast([128, NSUB, 10, 2, 2]),
                op=OP.add)
            idxf = qpool.tile([128, NSUB * 40], F32, tag="idxf", name="idxf")
            vec.tensor_tensor(
                out=idxf[:].rearrange("p (s hp xy) -> p s hp xy", s=NSUB, hp=10),
                in0=t4[:].rearrange("p (s hp xy) -> p s hp xy", s=NSUB, hp=10),
                in1=rzq[:].rearrange("p (s hp) -> p s hp", s=NSUB)
                    .unsqueeze(3).to_broadcast([128, NSUB, 10, 4]),
                op=OP.add)
            idx16 = qpool.tile([128, NSUB * 40], I16, tag="idx16", name="idx16")
            vec.tensor_copy(out=idx16[:], in_=idxf[:])

            # idx fold into the gather's 16-partition-wrapped layout:
            # bounce through DRAM with contiguous 320B packets, then a DVE
            # column permute (s2-major -> s2-interleaved).
            nc.sync.dma_start(out=idxscr[g * 128:(g + 1) * 128, :], in_=idx16[:])
            idxr = gpool.tile([128, ICOL], I16, tag="idxr", name="idxr")
            scr = idxscr[g * 128:(g + 1) * 128, :]
            for gg in range(8):
                # idxr[16*gg + p16, s2*160 + f] = idx16[s2*16 + p16, f]
                src = bass.AP(scr.tensor, scr.offset,
                              [[160, 16], [16 * 160, 8], [1, 160]])
                nc.sync.dma_start(
                    out=idxr[gg * 16:(gg + 1) * 16, :]
                        .rearrange("p (s2 f) -> p s2 f", s2=8),
                    in_=src)
            # permute cols: (s2, subhl, r) -> (subhl, r, s2)
            vec.tensor_copy(
                out=idxw_all[:, g * ICOL:(g + 1) * ICOL]
                    .rearrange("p (sh r s2) -> p sh r s2", sh=8, r=20),
                in_=idxr[:].rearrange("p (s2 sh r) -> p sh r s2", s2=8, sh=8))

        # ---- stage V: value projection (bf16) ----
        # zero the pad voxels at the end of each head block
        zpad = const.tile([8, 32], VDT, tag="zpad", name="zpad")
        vec.memset(zpad[:], 0.0)
        vflat_r = vflat[:].rearrange("(v c) -> v c", c=32)
        for hl in range(2):
            nc.sync.dma_start(
                out=vflat_r[hl * BSV + NVOX:hl * BSV + NVOX + 8, :], in_=zpad[:])
        for vg in range(NVSUP // NR):          # flush groups of NR supertiles
            vb = [vpool.tile([128, NR * 256], VDT, tag=f"vb{hl}", name=f"vb{hl}")
                  for hl in range(2)]
            for i in range(NR):
                vt = vg * NR + i
                vin = [vpool.tile([128, VSUP], VDT, tag=f"vin{k}", name=f"vin{k}")
                       for k in range(2)]
                for k in range(2):
                    nc.sync.dma_start(
                        out=vin[k][:],
                        in_=value_in[k * 128:(k + 1) * 128,
                                     vt * VSUP:(vt + 1) * VSUP])
                psv = ps_v.tile([128, 512], F32, tag="psv", name="psv")
                for s in range(8):
                    lhs0 = vin[0][:].rearrange("p (v e) -> p e v", e=8)[:, s, :]
                    lhs1 = vin[1][:].rearrange("p (v e) -> p e v", e=8)[:, s, :]
                    nc.tensor.matmul(psv[:, s * 64:(s + 1) * 64], lhs0,
                                     wval_sb[0][:], start=True, stop=False)
                    nc.tensor.matmul(psv[:, s * 64:(s + 1) * 64], lhs1,
                                     wval_sb[1][:], start=False, stop=True)
                # split heads, add bias, pack (blk, c, v4) per 4-voxel block
                for hl in range(2):
                    src = psv[:].rearrange("p (blk v4 hc) -> p blk hc v4",
                                           blk=2, v4=4)[:, :, hl * 32:(hl + 1) * 32, :]
                    bv = bval_sb[:, hl * 32:(hl + 1) * 32] \
                        .unsqueeze(1).unsqueeze(3).to_broadcast([128, 2, 32, 4])
                    vec.tensor_tensor(
                        out=vb[hl][:, i * 256:(i + 1) * 256]
                            .rearrange("p (blk c v4) -> p blk c v4", blk=2, c=32),
                        in0=src, in1=bv, op=OP.add)
            # flush NR supertiles (NR*1024 voxels) per head
            for hl in range(2):
                base = (hl * BSV + vg * NR * VSUP) * 32
                dst = bass.AP(vflat[:].tensor, base,
                              [[256, 128], [VSUP * 32, NR], [1, 256]])
                nc.sync.dma_start(out=dst, in_=vb[hl][:]
                                  .rearrange("p (i x) -> p i x", i=NR))

        # ---- stage G: gather + weighted reduce per (supertile, subtile, head) ----
        for g in range(NSUP):
            q0 = g * TQ
            S = qpool.tile([128, NSUB * 64], F32, tag="S", name="S")
            for s in range(NSUB):
                for hl in range(2):
                    G = gpool.tile([128, 20 * W * 32], VDT, tag="G", name="G")
                    in_g = bass.AP(vflat[:].tensor, hl * BSV * 32,
                                   [[SROW * 32, NROWH], [1, W * 32]])
                    nc.gpsimd.dma_gather(
                        out_ap=G[:].rearrange("p (i e) -> p i e", i=20),
                        in_ap=in_g,
                        idxs_ap=idxw_all[:, (g * 8 + s * 2 + hl) * 160:
                                         (g * 8 + s * 2 + hl + 1) * 160],
                        num_idxs=NIDX, num_idxs_reg=NIDX,
                        elem_size=W * 32, elem_step=SROW * 32,
                        single_packet=False)
                    # P = G * mask in place; mask cols (rb, v4) bcast over c
                    moff = g * MCOL + s * (40 * W) + hl * (5 * 4 * W)
                    mg = maskb_all[:, moff:moff + 160] \
                        .rearrange("p (rb v4) -> p rb v4", v4=4) \
                        .unsqueeze(2).to_broadcast([128, 40, 32, 4])
                    gv = G[:].rearrange("p (rb c v4) -> p rb c v4", rb=40, c=32)
                    vec.tensor_tensor(out=gv, in0=gv, in1=mg, op=OP.mult)
                    # two-step reduce: over v4 (contiguous), then over rb
                    PtA = gpool.tile([128, 40 * 32], F32, tag="PtA", name="PtA")
                    vec.tensor_reduce(
                        out=PtA[:].rearrange("p (rb c) -> p rb c", rb=40),
                        in_=gv, axis=AX.X, op=OP.add)
                    vec.tensor_reduce(
                        out=S[:, s * 64 + hl * 32:s * 64 + hl * 32 + 32],
                        in_=PtA[:].rearrange("p (rb c) -> p c rb", rb=40),
                        axis=AX.X, op=OP.add)

            # transpose S [128, 64] -> [64, 128] per subtile
            for s in range(NSUB):
                pst = ps_t.tile([64, 128], F32, tag="pst", name="pst")
                nc.tensor.transpose(pst[:], S[:, s * 64:(s + 1) * 64], ident[:])
                act.activation(out=st_sb[:, q0 + s * 128:q0 + (s + 1) * 128],
                               in_=pst[:], func=ACT.Copy)

        # ---- GEMM2: outT = wout^T @ ST (float32r moving, N=512) ----
        for mc in range(2):
            for ntile in range(NQ // 512):
                ps2 = ps_c.tile([128, 512], F32, tag="ps2", name="ps2")
                nc.tensor.matmul(ps2[:],
                                 wout_sb[:, mc * 128:(mc + 1) * 128],
                                 st_sb[:, ntile * 512:(ntile + 1) * 512],
                                 start=True, stop=True)
                ob = opool.tile([128, 512], F32, tag="ob", name="ob")
                vec.tensor_copy(out=ob[:], in_=ps2[:])
                nc.sync.dma_start(
                    out=outp[mc * 128:(mc + 1) * 128,
                             ntile * 512:(ntile + 1) * 512],
                    in_=ob[:])

    nc.compile()
    return nc


def _prep_core_inputs(inputs, b, j):
    import ml_dtypes
    q = np.ascontiguousarray(inputs["query"][b].T, np.float32)
    p = np.ascontiguousarray(inputs["pos"][b].T, np.float32)
    r = np.concatenate([inputs["reference_points"][b].T,
                        np.ones((1, NQ), np.float32)]).astype(np.float32)
    r = np.ascontiguousarray(r)
    value = np.ascontiguousarray(
        inputs["value"][b].reshape(C, NVOX)).astype(ml_dtypes.bfloat16)

    W_off, b_off = inputs["W_off"], inputs["b_off"]
    W_attn, b_attn = inputs["W_attn"], inputs["b_attn"]
    heads = [2 * j, 2 * j + 1]
    rows, biases, refr = [], [], []
    for h in heads:
        for pp in range(P):
            for ax in range(3):
                rows.append(W_off[(h * P + pp) * 3 + ax])
                biases.append(b_off[(h * P + pp) * 3 + ax] - 0.5 + 64.0)
                e = np.zeros(3, np.float32)
                e[ax] = GRID
                refr.append(e)
    for h in heads:
        for pp in range(P):
            rows.append(W_attn[h * P + pp])
            biases.append(b_attn[h * P + pp])
            refr.append(np.zeros(3, np.float32))
    wcat = np.ascontiguousarray(np.stack(rows).T, np.float32)       # (256, 40)
    ref_rhs = np.concatenate(
        [np.stack(refr).T, np.asarray(biases, np.float32)[None, :]])
    ref_rhs = np.ascontiguousarray(ref_rhs, np.float32)             # (4, 40)

    wval = np.ascontiguousarray(
        inputs["W_val"][64 * j:64 * j + 64].T).astype(ml_dtypes.bfloat16)
    bval = np.ascontiguousarray(
        np.repeat(inputs["b_val"][64 * j:64 * j + 64][None, :], 128, axis=0),
        np.float32)
    wout = np.ascontiguousarray(inputs["W_out"][:, 64 * j:64 * j + 64].T,
                                np.float32)
    zoffs = np.repeat(np.arange(W, dtype=np.float32)[None, :], 128, axis=0)
    return {
        "value_in": value, "qT": q, "pT": p, "refT": r,
        "wcat": wcat, "ref_rhs": ref_rhs,
        "wval": wval, "bval": bval, "wout": wout, "zoff": zoffs,
    }


def get_nc():
    global _NC_CACHE
    if _NC_CACHE is None:
        _NC_CACHE = build_nc()
    return _NC_CACHE


def kernel(**inputs):
    from concourse.bass_utils import run_bass_kernel_spmd

    inputs = {k: np.asarray(v) for k, v in inputs.items()}
    nc = get_nc()
    in_maps = [_prep_core_inputs(inputs, core // 4, core % 4) for core in range(8)]
    res = run_bass_kernel_spmd(nc, in_maps, list(range(8)))
    bs = inputs["query"].shape[0]
    out = np.zeros((bs, NQ, C), np.float32)
    for core in range(8):
        out[core // 4] += res.results[core]["outp"].T
    out += inputs["b_out"][None, None, :].astype(np.float32)
    return out


# revision 7
# speedup vs baseline: 2.5242x; 1.0333x over previous
"""Trainium2 Bass kernel for 3D deformable attention (8 NeuronCores).

Sharding: core i handles batch b = i // 4 and head-pair j = i % 4
(heads 2j, 2j+1, i.e. value/out channels [64j, 64j+64)).

Per-core device pipeline (emission order = coords first so DVE mask work
overlaps the value-projection DMA/PE stage):
  C. per query-supertile (512 q): coords = qs^T @ Wcat^T + [48*ref | b] (PE);
     softmax over 5 points, clamped trilinear corner weights, z-window "hat"
     weights, combined mask m = aw*wx*wy*wz (DVE, bf16); int16 gather row
     indices; idx fold into dma_gather's 16-partition-wrapped layout via a
     contiguous DRAM bounce (320B packets) + DVE column permute.
  V. value projection v = W_val[64j:64j+64] @ value[b] in bf16 (PE,
     voxel-stationary), + b_val; packed per 4-voxel block as (c, v4) and
     flushed to DRAM as two head blocks of [NVOX+8 vox, 32ch] rows.
  G. per (query-subtile, head): one dma_gather of 2560 rows (8 vox x 32ch
     bf16 = 512B each); P = G * mask (DVE bf16, contiguous); two-step
     reduce (over v4 contiguous, then over (pt,xy,blk)) -> S[q, 64].
  O. PE transpose of S, then out^T = Wout^T @ S^T (float32r), DMA out.
Host combines: out[b] = sum_j outp_j^T + b_out.
"""
import numpy as np

import concourse.bass as bass
import concourse.mybir as mybir
from concourse import bacc, tile
from concourse.masks import make_identity
from contextlib import ExitStack

F32 = mybir.dt.float32
F32R = mybir.dt.float32r
I16 = mybir.dt.int16
AX = mybir.AxisListType
OP = mybir.AluOpType
ACT = mybir.ActivationFunctionType

H, P = 8, 5
NQ, C, GRID = 4096, 256, 48
NVOX = GRID ** 3            # 110592
NSUB = 4                    # query subtiles (of 128) per supertile
TQ = 128 * NSUB             # 512
NSUP = NQ // TQ             # 8
VSUP = 1024                 # voxels per value-proj supertile
NVSUP = NVOX // VSUP        # 108
NR = 4                      # value supertiles per DRAM flush

VDT = mybir.dt.bfloat16
W = 8                       # voxels per gathered row
SROW = 4                    # voxels per row step (4*32*2B = 256B)
NROWH = NVOX // SROW        # 27648 rows per head (< 32768 for int16)
BSV = NVOX + 8              # voxels per head block incl pad
NIDX = 20 * 128             # rows per (subtile, head) gather
MCOL = NSUB * 40 * W        # mask columns per supertile (1280)
ICOL = NSUB * 2 * 160       # idx columns per supertile (1280)

_NC_CACHE = None


def build_nc():
    nc = bacc.Bacc("TRN2", target_bir_lowering=False, debug=False, num_devices=8,
                   num_swdge_queues=4)

    value_in = nc.dram_tensor("value_in", [C, NVOX], VDT, kind="ExternalInput")
    qT = nc.dram_tensor("qT", [C, NQ], F32, kind="ExternalInput")
    pT = nc.dram_tensor("pT", [C, NQ], F32, kind="ExternalInput")
    refT = nc.dram_tensor("refT", [4, NQ], F32, kind="ExternalInput")
    wcat = nc.dram_tensor("wcat", [C, 40], F32, kind="ExternalInput")
    ref_rhs = nc.dram_tensor("ref_rhs", [4, 40], F32, kind="ExternalInput")
    wval = nc.dram_tensor("wval", [C, 64], VDT, kind="ExternalInput")
    bval = nc.dram_tensor("bval", [128, 64], F32, kind="ExternalInput")
    wout = nc.dram_tensor("wout", [64, C], F32, kind="ExternalInput")
    zoff = nc.dram_tensor("zoff", [128, W], F32, kind="ExternalInput")
    outp = nc.dram_tensor("outp", [C, NQ], F32, kind="ExternalOutput")
    vflat = nc.dram_tensor("vflat", [2 * BSV * 32], VDT)
    idxscr = nc.dram_tensor("idxscr", [NSUP * 128, 160], I16)

    vec = nc.vector
    act = nc.scalar

    with tile.TileContext(nc) as tc, ExitStack() as ctx:
        const = ctx.enter_context(tc.tile_pool(name="const", bufs=1))
        vpool = ctx.enter_context(tc.tile_pool(name="vpool", bufs=2))
        qpool = ctx.enter_context(tc.tile_pool(name="qpool", bufs=2))
        gpool = ctx.enter_context(tc.tile_pool(name="gpool", bufs=2))
        opool = ctx.enter_context(tc.tile_pool(name="opool", bufs=2))
        ps_v = ctx.enter_context(tc.tile_pool(name="ps_v", bufs=2, space="PSUM"))
        ps_c = ctx.enter_context(tc.tile_pool(name="ps_c", bufs=2, space="PSUM"))
        ps_t = ctx.enter_context(tc.tile_pool(name="ps_t", bufs=2, space="PSUM"))

        # ---- constants into SBUF ----
        wcat_sb = [const.tile([128, 40], F32, tag=f"wcat{k}", name=f"wcat{k}")
                   for k in range(2)]
        for k in range(2):
            nc.sync.dma_start(out=wcat_sb[k][:], in_=wcat[k * 128:(k + 1) * 128, :])
        refrhs_sb = const.tile([4, 40], F32, tag="refrhs", name="refrhs")
        nc.sync.dma_start(out=refrhs_sb[:], in_=ref_rhs[:])
        wval_sb = [const.tile([128, 64], VDT, tag=f"wval{k}", name=f"wval{k}")
                   for k in range(2)]
        for k in range(2):
            nc.sync.dma_start(out=wval_sb[k][:], in_=wval[k * 128:(k + 1) * 128, :])
        bval_sb = const.tile([128, 64], F32, tag="bval", name="bval")
        nc.sync.dma_start(out=bval_sb[:], in_=bval[:])
        wout_sb = const.tile([64, C], F32, tag="wout", name="wout")
        nc.sync.dma_start(out=wout_sb[:], in_=wout[:])
        zoff_sb = const.tile([128, W], F32, tag="zoff", name="zoff")
        nc.sync.dma_start(out=zoff_sb[:], in_=zoff[:])
        ident = const.tile([128, 128], F32, tag="ident", name="ident")
        make_identity(nc, ident[:])

        # persistent big buffers
        qs_sb = [const.tile([128, NQ], F32, tag=f"qs{k}", name=f"qs{k}")
                 for k in range(2)]
        ref_sb = const.tile([4, NQ], F32, tag="refq", name="refq")
        st_sb = const.tile([64, NQ], F32, tag="st", name="st")
        maskb_all = const.tile([128, NSUP * MCOL], VDT, tag="maskb", name="maskb")
        idxw_all = const.tile([128, NSUP * ICOL], I16, tag="idxw", name="idxw")

        # ---- stage Q0: load q, pos, ref; qs = q + p ----
        for k in range(2):
            for half in range(4):
                sl = slice(half * (NQ // 4), (half + 1) * (NQ // 4))
                ptmp = qpool.tile([128, NQ // 4], F32, tag="ptmp", name="ptmp")
                nc.sync.dma_start(out=qs_sb[k][:, sl],
                                  in_=qT[k * 128:(k + 1) * 128, sl])
                nc.sync.dma_start(out=ptmp[:], in_=pT[k * 128:(k + 1) * 128, sl])
                vec.tensor_tensor(out=qs_sb[k][:, sl], in0=qs_sb[k][:, sl],
                                  in1=ptmp[:], op=OP.add)
        nc.sync.dma_start(out=ref_sb[:], in_=refT[:])

        # ---- stage C: coords / masks / gather indices, all supertiles ----
        for g in range(NSUP):
            q0 = g * TQ
            psc = ps_c.tile([128, 160], F32, tag="psc", name="psc")
            for s in range(NSUB):
                qsl = slice(q0 + s * 128, q0 + (s + 1) * 128)
                nc.tensor.matmul(psc[:, s * 40:(s + 1) * 40],
                                 qs_sb[0][:, qsl], wcat_sb[0][:],
                                 start=True, stop=False)
                nc.tensor.matmul(psc[:, s * 40:(s + 1) * 40],
                                 qs_sb[1][:, qsl], wcat_sb[1][:],
                                 start=False, stop=False)
                nc.tensor.matmul(psc[:, s * 40:(s + 1) * 40],
                                 ref_sb[:, qsl], refrhs_sb[:],
                                 start=False, stop=True)
            coords = qpool.tile([128, 160], F32, tag="coords", name="coords")
            act.activation(out=coords[:], in_=psc[:], func=ACT.Copy)

            co = coords[:].rearrange("p (s r) -> p s r", s=NSUB)
            pix = co[:, :, 0:30]                        # (s, hp*ax)
            logit = co[:, :, 30:40]                     # (s, hp)

            # softmax over P
            exlog = qpool.tile([128, NSUB * 10], F32, tag="exlog", name="exlog")
            act.activation(out=exlog[:], in_=logit, func=ACT.Exp)
            ex4 = exlog[:].rearrange("p (s h q) -> p s h q", s=NSUB, h=2)
            sums = qpool.tile([128, NSUB * 2], F32, tag="sums", name="sums")
            vec.tensor_reduce(out=sums[:].rearrange("p (s h) -> p s h", s=NSUB),
                              in_=ex4, axis=AX.X, op=OP.add)
            rsum = qpool.tile([128, NSUB * 2], F32, tag="rsum", name="rsum")
            vec.reciprocal(out=rsum[:], in_=sums[:])
            aw = qpool.tile([128, NSUB * 10], F32, tag="aw", name="aw")
            vec.tensor_tensor(
                out=aw[:].rearrange("p (sh q) -> p sh q", q=5),
                in0=exlog[:].rearrange("p (sh q) -> p sh q", q=5),
                in1=rsum[:].unsqueeze(2).to_broadcast([128, NSUB * 2, 5]),
                op=OP.mult)

            # corner math on the 30 pixel rows
            NPX = NSUB * 30
            # flo = round(pix - 0.5) via the 2^23 magic add (== floor except
            # exactly-integer pix, where the phantom corner gets zero weight)
            flo = qpool.tile([128, NPX], F32, tag="flo", name="flo")
            vec.tensor_scalar(out=flo[:].rearrange("p (s r) -> p s r", s=NSUB),
                              in0=pix, scalar1=8388607.5, scalar2=8388608.0,
                              op0=OP.add, op1=OP.subtract)
            fl3 = flo[:].rearrange("p (s r) -> p s r", s=NSUB)
            frac = qpool.tile([128, NPX], F32, tag="frac", name="frac")
            vec.tensor_tensor(out=frac[:].rearrange("p (s r) -> p s r", s=NSUB),
                              in0=pix, in1=fl3, op=OP.subtract)
            fr3 = frac[:].rearrange("p (s r) -> p s r", s=NSUB)
            # gcorn: (s, hp, ax, dx) — clamped corner coords (biased +64)
            gcorn = qpool.tile([128, NPX * 2], F32, tag="gcorn", name="gcorn")
            gc4 = gcorn[:].rearrange("p (s r d) -> p s r d", s=NSUB, d=2)
            vec.tensor_scalar(out=gc4[:, :, :, 0], in0=fl3,
                              scalar1=64.0, scalar2=111.0, op0=OP.max, op1=OP.min)
            g1m = qpool.tile([128, NPX], F32, tag="g1m", name="g1m")
            vec.tensor_scalar(out=g1m[:], in0=flo[:],
                              scalar1=63.0, scalar2=110.0, op0=OP.max, op1=OP.min)
            vec.tensor_scalar(out=gc4[:, :, :, 1], in0=g1m[:]
                              .rearrange("p (s r) -> p s r", s=NSUB),
                              scalar1=1.0, scalar2=None, op0=OP.add)
            # validity via clip-equality
            v0 = qpool.tile([128, NPX], F32, tag="v0", name="v0")
            vec.tensor_tensor(out=v0[:].rearrange("p (s r) -> p s r", s=NSUB),
                              in0=gc4[:, :, :, 0], in1=fl3, op=OP.is_equal)
            v1 = qpool.tile([128, NPX], F32, tag="v1", name="v1")
            vec.tensor_tensor(out=v1[:], in0=g1m[:], in1=flo[:], op=OP.is_equal)
            # corner weights (x/y rows used; z rows ignored later)
            om = qpool.tile([128, NPX], F32, tag="om", name="om")
            vec.tensor_scalar(out=om[:], in0=frac[:], scalar1=-1.0, scalar2=1.0,
                              op0=OP.mult, op1=OP.add)
            wcorn = qpool.tile([128, NPX * 2], F32, tag="wcorn", name="wcorn")
            wc4 = wcorn[:].rearrange("p (s r d) -> p s r d", s=NSUB, d=2)
            vec.tensor_tensor(out=wc4[:, :, :, 0],
                              in0=om[:].rearrange("p (s r) -> p s r", s=NSUB),
                              in1=v0[:].rearrange("p (s r) -> p s r", s=NSUB),
                              op=OP.mult)
            vec.tensor_tensor(out=wc4[:, :, :, 1],
                              in0=fr3,
                              in1=v1[:].rearrange("p (s r) -> p s r", s=NSUB),
                              op=OP.mult)

            # z window: rzq = floor((gz-64)/4) in [0,11]; W-slot hat weights
            gc6 = gcorn[:].rearrange("p (s hp a d) -> p s hp a d",
                                     s=NSUB, hp=10, a=3)
            gz = gc6[:, :, :, 2, 0]
            pz = co[:, :, 0:30].rearrange("p s (hp a) -> p s hp a", a=3)[:, :, :, 2]
            rzq = qpool.tile([128, NSUB * 10], F32, tag="rzq", name="rzq")
            tq = qpool.tile([128, NSUB * 10], F32, tag="tq", name="tq")
            vec.tensor_scalar(out=tq[:].rearrange("p (s h) -> p s h", s=NSUB),
                              in0=gz, scalar1=0.25, scalar2=16.375,
                              op0=OP.mult, op1=OP.subtract)
            vec.tensor_scalar(out=rzq[:], in0=tq[:],
                              scalar1=8388624.0, scalar2=8388624.0,
                              op0=OP.add, op1=OP.subtract)
            # d0 = (4*rzq + 64) - pz ; dk = d0 + k
            zb4 = qpool.tile([128, NSUB * 10], F32, tag="zb4", name="zb4")
            vec.tensor_scalar(out=zb4[:], in0=rzq[:], scalar1=4.0, scalar2=64.0,
                              op0=OP.mult, op1=OP.add)
            d0 = qpool.tile([128, NSUB * 10], F32, tag="d0", name="d0")
            vec.tensor_tensor(out=d0[:].rearrange("p (s h) -> p s h", s=NSUB),
                              in0=zb4[:].rearrange("p (s h) -> p s h", s=NSUB),
                              in1=pz, op=OP.subtract)
            dk = qpool.tile([128, NSUB * 10 * W], F32, tag="dk", name="dk")
            vec.tensor_tensor(
                out=dk[:].rearrange("p (sh k) -> p sh k", k=W),
                in0=d0[:].unsqueeze(2).to_broadcast([128, NSUB * 10, W]),
                in1=zoff_sb[:].unsqueeze(1).to_broadcast([128, NSUB * 10, W]),
                op=OP.add)
            adk = qpool.tile([128, NSUB * 10 * W], F32, tag="adk", name="adk")
            act.activation(out=adk[:], in_=dk[:], func=ACT.Abs)
            hat = qpool.tile([128, NSUB * 10 * W], F32, tag="hat", name="hat")
            act.activation(out=hat[:], in_=adk[:], func=ACT.Relu,
                           scale=-1.0, bias=1.0)
            # upper bound: slot z 4*rzq+64+k <= 111  <=>  dk <= 111 - pz
            ub = qpool.tile([128, NSUB * 10], F32, tag="ub", name="ub")
            vec.tensor_scalar(out=ub[:].rearrange("p (s h) -> p s h", s=NSUB),
                              in0=pz, scalar1=-1.0, scalar2=111.0,
                              op0=OP.mult, op1=OP.add)
            vub = qpool.tile([128, NSUB * 10 * W], F32, tag="vub", name="vub")
            vec.tensor_tensor(
                out=vub[:].rearrange("p (sh k) -> p sh k", k=W),
                in0=dk[:].rearrange("p (sh k) -> p sh k", k=W),
                in1=ub[:].unsqueeze(2).to_broadcast([128, NSUB * 10, W]),
                op=OP.is_le)
            wz = qpool.tile([128, NSUB * 10 * W], F32, tag="wz", name="wz")
            vec.tensor_tensor(out=wz[:], in0=hat[:], in1=vub[:], op=OP.mult)

            # mask assembly: m[s, hp, dx, dy, k]
            wc6 = wcorn[:].rearrange("p (s hp a d) -> p s hp a d",
                                     s=NSUB, hp=10, a=3)
            wx = wc6[:, :, :, 0, :]                     # (s, hp, dx)
            wy = wc6[:, :, :, 1, :]                     # (s, hp, dy)
            t1 = qpool.tile([128, NSUB * 40], F32, tag="t1", name="t1")
            vec.tensor_tensor(
                out=t1[:].rearrange("p (s hp x y) -> p s hp x y",
                                    s=NSUB, hp=10, x=2),
                in0=wx.unsqueeze(4).to_broadcast([128, NSUB, 10, 2, 2]),
                in1=wy.unsqueeze(3).to_broadcast([128, NSUB, 10, 2, 2]),
                op=OP.mult)
            t2 = qpool.tile([128, NSUB * 40], F32, tag="t2", name="t2")
            aw4 = aw[:].rearrange("p (s hp) -> p s hp", s=NSUB) \
                .unsqueeze(3).to_broadcast([128, NSUB, 10, 4])
            vec.tensor_tensor(
                out=t2[:].rearrange("p (s hp xy) -> p s hp xy", s=NSUB, hp=10),
                in0=t1[:].rearrange("p (s hp xy) -> p s hp xy", s=NSUB, hp=10),
                in1=aw4, op=OP.mult)
            vec.tensor_tensor(
                out=maskb_all[:, g * MCOL:(g + 1) * MCOL]
                    .rearrange("p (shp xy k) -> p shp xy k", xy=4, k=W),
                in0=t2[:].rearrange("p (shp xy) -> p shp xy", xy=4)
                    .unsqueeze(3).to_broadcast([128, NSUB * 10, 4, W]),
                in1=wz[:].rearrange("p (shp k) -> p shp k", k=W)
                    .unsqueeze(2).to_broadcast([128, NSUB * 10, 4, W]),
                op=OP.mult)

            # gather row indices: 576*(gx-64) + 12*(gy-64) + rzq
            gx = gc6[:, :, :, 0, :]                     # (s, hp, dx)
            gy = gc6[:, :, :, 1, :]                     # (s, hp, dy)
            ga = qpool.tile([128, NSUB * 20], F32, tag="ga", name="ga")
            vec.tensor_scalar(out=ga[:].rearrange("p (s hp d) -> p s hp d",
                                                  s=NSUB, hp=10),
                              in0=gx, scalar1=576.0, scalar2=37632.0,
                              op0=OP.mult, op1=OP.subtract)
            gb = qpool.tile([128, NSUB * 20], F32, tag="gb", name="gb")
            vec.tensor_scalar(out=gb[:].rearrange("p (s hp d) -> p s hp d",
                                                  s=NSUB, hp=10),
                              in0=gy, scalar1=12.0, scalar2=None, op0=OP.mult)
            t4 = qpool.tile([128, NSUB * 40], F32, tag="t4", name="t4")
            vec.tensor_tensor(
                out=t4[:].rearrange("p (s hp x y) -> p s hp x y",
                                    s=NSUB, hp=10, x=2),
                in0=ga[:].rearrange("p (s hp x) -> p s hp x", s=NSUB, hp=10)
                    .unsqueeze(4).to_broadcast([128, NSUB, 10, 2, 2]),
                in1=gb[:].rearrange("p (s hp y) -> p s hp y", s=NSUB, hp=10)
                    .unsqueeze(3).to_broadcast([128, NSUB, 10, 2, 2]),
                op=OP.add)
            idxf = qpool.tile([128, NSUB * 40], F32, tag="idxf", name="idxf")
            vec.tensor_tensor(
                out=idxf[:].rearrange("p (s hp xy) -> p s hp xy", s=NSUB, hp=10),
                in0=t4[:].rearrange("p (s hp xy) -> p s hp xy", s=NSUB, hp=10),
                in1=rzq[:].rearrange("p (s hp) -> p s hp", s=NSUB)
                    .unsqueeze(3).to_broadcast([128, NSUB, 10, 4]),
                op=OP.add)
            idx16 = qpool.tile([128, NSUB * 40], I16, tag="idx16", name="idx16")
            vec.tensor_copy(out=idx16[:], in_=idxf[:])

            # idx fold into the gather's 16-partition-wrapped layout:
            # bounce through DRAM with contiguous 320B packets, then a DVE
            # column permute (s2-major -> s2-interleaved).
            nc.sync.dma_start(out=idxscr[g * 128:(g + 1) * 128, :], in_=idx16[:])
            idxr = gpool.tile([128, ICOL], I16, tag="idxr", name="idxr")
            scr = idxscr[g * 128:(g + 1) * 128, :]
            for gg in range(8):
                # idxr[16*gg + p16, s2*160 + f] = idx16[s2*16 + p16, f]
                src = bass.AP(scr.tensor, scr.offset,
                              [[160, 16], [16 * 160, 8], [1, 160]])
                nc.sync.dma_start(
                    out=idxr[gg * 16:(gg + 1) * 16, :]
                        .rearrange("p (s2 f) -> p s2 f", s2=8),
                    in_=src)
            # permute cols: (s2, subhl, r) -> (subhl, r, s2)
            vec.tensor_copy(
                out=idxw_all[:, g * ICOL:(g + 1) * ICOL]
                    .rearrange("p (sh r s2) -> p sh r s2", sh=8, r=20),
                in_=idxr[:].rearrange("p (s2 sh r) -> p sh r s2", s2=8, sh=8))

        # ---- stage V: value projection (bf16) ----
        # zero the pad voxels at the end of each head block
        zpad = const.tile([8, 32], VDT, tag="zpad", name="zpad")
        vec.memset(zpad[:], 0.0)
        vflat_r = vflat[:].rearrange("(v c) -> v c", c=32)
        for hl in range(2):
            nc.sync.dma_start(
                out=vflat_r[hl * BSV + NVOX:hl * BSV + NVOX + 8, :], in_=zpad[:])
        for vg in range(NVSUP // NR):          # flush groups of NR supertiles
            vb = [vpool.tile([128, NR * 256], VDT, tag=f"vb{hl}", name=f"vb{hl}")
                  for hl in range(2)]
            for i in range(NR):
                vt = vg * NR + i
                vin = [vpool.tile([128, VSUP], VDT, tag=f"vin{k}", name=f"vin{k}")
                       for k in range(2)]
                for k in range(2):
                    nc.sync.dma_start(
                        out=vin[k][:],
                        in_=value_in[k * 128:(k + 1) * 128,
                                     vt * VSUP:(vt + 1) * VSUP])
                psv = ps_v.tile([128, 512], F32, tag="psv", name="psv")
                for s in range(8):
                    lhs0 = vin[0][:].rearrange("p (v e) -> p e v", e=8)[:, s, :]
                    lhs1 = vin[1][:].rearrange("p (v e) -> p e v", e=8)[:, s, :]
                    nc.tensor.matmul(psv[:, s * 64:(s + 1) * 64], lhs0,
                                     wval_sb[0][:], start=True, stop=False)
                    nc.tensor.matmul(psv[:, s * 64:(s + 1) * 64], lhs1,
                                     wval_sb[1][:], start=False, stop=True)
                # split heads, add bias, pack (blk, c, v4) per 4-voxel block
                for hl in range(2):
                    src = psv[:].rearrange("p (blk v4 hc) -> p blk hc v4",
                                           blk=2, v4=4)[:, :, hl * 32:(hl + 1) * 32, :]
                    bv = bval_sb[:, hl * 32:(hl + 1) * 32] \
                        .unsqueeze(1).unsqueeze(3).to_broadcast([128, 2, 32, 4])
                    vec.tensor_tensor(
                        out=vb[hl][:, i * 256:(i + 1) * 256]
                            .rearrange("p (blk c v4) -> p blk c v4", blk=2, c=32),
                        in0=src, in1=bv, op=OP.add)
            # flush NR supertiles (NR*1024 voxels) per head
            for hl in range(2):
                base = (hl * BSV + vg * NR * VSUP) * 32
                dst = bass.AP(vflat[:].tensor, base,
                              [[256, 128], [VSUP * 32, NR], [1, 256]])
                nc.sync.dma_start(out=dst, in_=vb[hl][:]
                                  .rearrange("p (i x) -> p i x", i=NR))

        # ---- stage G: gather + weighted reduce per (supertile, subtile, head) ----
        for g in range(NSUP):
            q0 = g * TQ
            S = qpool.tile([128, NSUB * 64], F32, tag="S", name="S")
            for s in range(NSUB):
                for hl in range(2):
                    G = gpool.tile([128, 20 * W * 32], VDT, tag="G", name="G")
                    in_g = bass.AP(vflat[:].tensor, hl * BSV * 32,
                                   [[SROW * 32, NROWH], [1, W * 32]])
                    nc.gpsimd.dma_gather(
                        out_ap=G[:].rearrange("p (i e) -> p i e", i=20),
                        in_ap=in_g,
                        idxs_ap=idxw_all[:, (g * 8 + s * 2 + hl) * 160:
                                         (g * 8 + s * 2 + hl + 1) * 160],
                        num_idxs=NIDX, num_idxs_reg=NIDX,
                        elem_size=W * 32, elem_step=SROW * 32,
                        single_packet=False,
                        queue_num=(s * 2 + hl) % 4)
                    # P = G * mask in place; mask cols (rb, v4) bcast over c
                    moff = g * MCOL + s * (40 * W) + hl * (5 * 4 * W)
                    mg = maskb_all[:, moff:moff + 160] \
                        .rearrange("p (rb v4) -> p rb v4", v4=4) \
                        .unsqueeze(2).to_broadcast([128, 40, 32, 4])
                    gv = G[:].rearrange("p (rb c v4) -> p rb c v4", rb=40, c=32)
                    vec.tensor_tensor(out=gv, in0=gv, in1=mg, op=OP.mult)
                    # two-step reduce: over v4 (contiguous), then over rb
                    PtA = gpool.tile([128, 40 * 32], F32, tag="PtA", name="PtA")
                    vec.tensor_reduce(
                        out=PtA[:].rearrange("p (rb c) -> p rb c", rb=40),
                        in_=gv, axis=AX.X, op=OP.add)
                    vec.tensor_reduce(
                        out=S[:, s * 64 + hl * 32:s * 64 + hl * 32 + 32],
                        in_=PtA[:].rearrange("p (rb c) -> p c rb", rb=40),
                        axis=AX.X, op=OP.add)

            # transpose S [128, 64] -> [64, 128] per subtile
            for s in range(NSUB):
                pst = ps_t.tile([64, 128], F32, tag="pst", name="pst")
                nc.tensor.transpose(pst[:], S[:, s * 64:(s + 1) * 64], ident[:])
                act.activation(out=st_sb[:, q0 + s * 128:q0 + (s + 1) * 128],
                               in_=pst[:], func=ACT.Copy)

        # ---- GEMM2: outT = wout^T @ ST (float32r moving, N=512) ----
        for mc in range(2):
            for ntile in range(NQ // 512):
                ps2 = ps_c.tile([128, 512], F32, tag="ps2", name="ps2")
                nc.tensor.matmul(ps2[:],
                                 wout_sb[:, mc * 128:(mc + 1) * 128],
                                 st_sb[:, ntile * 512:(ntile + 1) * 512],
                                 start=True, stop=True)
                ob = opool.tile([128, 512], F32, tag="ob", name="ob")
                vec.tensor_copy(out=ob[:], in_=ps2[:])
                nc.sync.dma_start(
                    out=outp[mc * 128:(mc + 1) * 128,
                             ntile * 512:(ntile + 1) * 512],
                    in_=ob[:])

    nc.compile()
    return nc


def _prep_core_inputs(inputs, b, j):
    import ml_dtypes
    q = np.ascontiguousarray(inputs["query"][b].T, np.float32)
    p = np.ascontiguousarray(inputs["pos"][b].T, np.float32)
    r = np.concatenate([inputs["reference_points"][b].T,
                        np.ones((1, NQ), np.float32)]).astype(np.float32)
    r = np.ascontiguousarray(r)
    value = np.ascontiguousarray(
        inputs["value"][b].reshape(C, NVOX)).astype(ml_dtypes.bfloat16)

    W_off, b_off = inputs["W_off"], inputs["b_off"]
    W_attn, b_attn = inputs["W_attn"], inputs["b_attn"]
    heads = [2 * j, 2 * j + 1]
    rows, biases, refr = [], [], []
    for h in heads:
        for pp in range(P):
            for ax in range(3):
                rows.append(W_off[(h * P + pp) * 3 + ax])
                biases.append(b_off[(h * P + pp) * 3 + ax] - 0.5 + 64.0)
                e = np.zeros(3, np.float32)
                e[ax] = GRID
                refr.append(e)
    for h in heads:
        for pp in range(P):
            rows.append(W_attn[h * P + pp])
            biases.append(b_attn[h * P + pp])
            refr.append(np.zeros(3, np.float32))
    wcat = np.ascontiguousarray(np.stack(rows).T, np.float32)       # (256, 40)
    ref_rhs = np.concatenate(
        [np.stack(refr).T, np.asarray(biases, np.float32)[None, :]])
    ref_rhs = np.ascontiguousarray(ref_rhs, np.float32)             # (4, 40)

    wval = np.ascontiguousarray(
        inputs["W_val"][64 * j:64 * j + 64].T).astype(ml_dtypes.bfloat16)
    bval = np.ascontiguousarray(
        np.repeat(inputs["b_val"][64 * j:64 * j + 64][None, :], 128, axis=0),
        np.float32)
    wout = np.ascontiguousarray(inputs["W_out"][:, 64 * j:64 * j + 64].T,
                                np.float32)
    zoffs = np.repeat(np.arange(W, dtype=np.float32)[None, :], 128, axis=0)
    return {
        "value_in": value, "qT": q, "pT": p, "refT": r,
        "wcat": wcat, "ref_rhs": ref_rhs,
        "wval": wval, "bval": bval, "wout": wout, "zoff": zoffs,
    }


def get_nc():
    global _NC_CACHE
    if _NC_CACHE is None:
        _NC_CACHE = build_nc()
    return _NC_CACHE


def kernel(**inputs):
    from concourse.bass_utils import run_bass_kernel_spmd

    inputs = {k: np.asarray(v) for k, v in inputs.items()}
    nc = get_nc()
    in_maps = [_prep_core_inputs(inputs, core // 4, core % 4) for core in range(8)]
    res = run_bass_kernel_spmd(nc, in_maps, list(range(8)))
    bs = inputs["query"].shape[0]
    out = np.zeros((bs, NQ, C), np.float32)
    for core in range(8):
        out[core // 4] += res.results[core]["outp"].T
    out += inputs["b_out"][None, None, :].astype(np.float32)
    return out


# revision 22
# speedup vs baseline: 2.9859x; 1.1829x over previous
"""Trainium2 Bass kernel for 3D deformable attention (8 NeuronCores).

Sharding: core i handles batch b = i // 4 and head-pair j = i % 4
(heads 2j, 2j+1, i.e. value/out channels [64j, 64j+64)).

Per-core device pipeline (emission order = coords first so DVE mask work
overlaps the value-projection DMA/PE stage):
  C. per query-supertile (512 q): coords = qs^T @ Wcat^T + [48*ref | b] (PE);
     softmax over 5 points, clamped trilinear corner weights, z-window "hat"
     weights, combined mask m = aw*wx*wy*wz (DVE, bf16); int16 gather row
     indices; idx fold into dma_gather's 16-partition-wrapped layout via a
     contiguous DRAM bounce (320B packets) + DVE column permute.
  V. value projection v = W_val[64j:64j+64] @ value[b] in bf16 (PE,
     voxel-stationary), + b_val; packed per 4-voxel block as (c, v4) and
     flushed to DRAM as two head blocks of [NVOX+8 vox, 32ch] rows.
  G. per (query-subtile, head): one dma_gather of 2560 rows (8 vox x 32ch
     bf16 = 512B each); P = G * mask (DVE bf16, contiguous); two-step
     reduce (over v4 contiguous, then over (pt,xy,blk)) -> S[q, 64].
  O. PE transpose of S, then out^T = Wout^T @ S^T (float32r), DMA out.
Host combines: out[b] = sum_j outp_j^T + b_out.
"""
import numpy as np

import concourse.bass as bass
import concourse.mybir as mybir
from concourse import bacc, tile
from concourse.masks import make_identity
from contextlib import ExitStack

F32 = mybir.dt.float32
F32R = mybir.dt.float32r
I16 = mybir.dt.int16
AX = mybir.AxisListType
OP = mybir.AluOpType
ACT = mybir.ActivationFunctionType

H, P = 8, 5
NQ, C, GRID = 4096, 256, 48
NVOX = GRID ** 3            # 110592
NSUB = 4                    # query subtiles (of 128) per supertile
TQ = 128 * NSUB             # 512
NSUP = NQ // TQ             # 8
VSUP = 1024                 # voxels per value-proj supertile
NVSUP = NVOX // VSUP        # 108
NR = 4                      # value supertiles per DRAM flush

VDT = mybir.dt.bfloat16
W = 8                       # z-window voxels per gathered row
# vexp: quad-interleaved expanded volume. Per head, blocks (x0, y0) of
# 12 z-units; unit = (xp, yp, c, z4) = 512 els (4-z slab x 4 quadrants x
# 32ch). A gather row = 2 consecutive units = the full 2x2x(8z) trilinear
# neighborhood of one sample point. unit(vblock) = vblock - xp*576 - yp*12
# is linear in vblock, so the 4 write passes keep 256B-contiguous runs.
NUNIT = 48 * 48 * 12        # 27648 addressable units per head
G0 = 588                    # front guard units (absorbs xp/yp shifts)
G1 = 16                     # back guard units (zeroed; z-window overrun)
VHEAD = (G0 + NUNIT + G1) * 512
NIDX = 5 * 128              # rows per (subtile, head) gather
MCOL = NSUB * 40 * W        # mask columns per supertile (1280)
ICOL = NSUB * 2 * 40        # idx columns per supertile (320)

_NC_CACHE = None


def build_nc():
    nc = bacc.Bacc("TRN2", target_bir_lowering=False, debug=False, num_devices=8,
                   num_swdge_queues=4)

    value_in = nc.dram_tensor("value_in", [C, NVOX], VDT, kind="ExternalInput")
    qT = nc.dram_tensor("qT", [C, NQ], F32, kind="ExternalInput")
    pT = nc.dram_tensor("pT", [C, NQ], F32, kind="ExternalInput")
    refT = nc.dram_tensor("refT", [4, NQ], F32, kind="ExternalInput")
    wcat = nc.dram_tensor("wcat", [C, 40], F32, kind="ExternalInput")
    ref_rhs = nc.dram_tensor("ref_rhs", [4, 40], F32, kind="ExternalInput")
    wval = nc.dram_tensor("wval", [C, 64], VDT, kind="ExternalInput")
    bval = nc.dram_tensor("bval", [128, 64], F32, kind="ExternalInput")
    wout = nc.dram_tensor("wout", [64, C], F32, kind="ExternalInput")
    zoff = nc.dram_tensor("zoff", [128, W], F32, kind="ExternalInput")
    outp = nc.dram_tensor("outp", [C, NQ], F32, kind="ExternalOutput")
    vexp = nc.dram_tensor("vexp", [2 * VHEAD], VDT)
    idxscr = nc.dram_tensor("idxscr", [NSUP * 128, 40], I16)

    vec = nc.vector
    act = nc.scalar

    with tile.TileContext(nc) as tc, ExitStack() as ctx:
        const = ctx.enter_context(tc.tile_pool(name="const", bufs=1))
        vpool = ctx.enter_context(tc.tile_pool(name="vpool", bufs=2))
        qpool = ctx.enter_context(tc.tile_pool(name="qpool", bufs=2))
        gpool = ctx.enter_context(tc.tile_pool(name="gpool", bufs=2))
        opool = ctx.enter_context(tc.tile_pool(name="opool", bufs=2))
        ps_v = ctx.enter_context(tc.tile_pool(name="ps_v", bufs=2, space="PSUM"))
        ps_c = ctx.enter_context(tc.tile_pool(name="ps_c", bufs=2, space="PSUM"))
        ps_t = ctx.enter_context(tc.tile_pool(name="ps_t", bufs=2, space="PSUM"))

        # ---- constants into SBUF ----
        wcat_sb = [const.tile([128, 40], F32, tag=f"wcat{k}", name=f"wcat{k}")
                   for k in range(2)]
        for k in range(2):
            nc.sync.dma_start(out=wcat_sb[k][:], in_=wcat[k * 128:(k + 1) * 128, :])
        refrhs_sb = const.tile([4, 40], F32, tag="refrhs", name="refrhs")
        nc.sync.dma_start(out=refrhs_sb[:], in_=ref_rhs[:])
        wval_sb = [const.tile([128, 64], VDT, tag=f"wval{k}", name=f"wval{k}")
                   for k in range(2)]
        for k in range(2):
            nc.sync.dma_start(out=wval_sb[k][:], in_=wval[k * 128:(k + 1) * 128, :])
        bval_sb = const.tile([128, 64], F32, tag="bval", name="bval")
        nc.sync.dma_start(out=bval_sb[:], in_=bval[:])
        wout_sb = const.tile([64, C], F32, tag="wout", name="wout")
        nc.sync.dma_start(out=wout_sb[:], in_=wout[:])
        zoff_sb = const.tile([128, W], F32, tag="zoff", name="zoff")
        nc.sync.dma_start(out=zoff_sb[:], in_=zoff[:])
        ident = const.tile([128, 128], F32, tag="ident", name="ident")
        make_identity(nc, ident[:])

        # persistent big buffers
        qs_sb = [const.tile([128, NQ], F32, tag=f"qs{k}", name=f"qs{k}")
                 for k in range(2)]
        ref_sb = const.tile([4, NQ], F32, tag="refq", name="refq")
        st_sb = const.tile([64, NQ], F32, tag="st", name="st")
        maskb_all = const.tile([128, NSUP * MCOL], VDT, tag="maskb", name="maskb")
        idxw_all = const.tile([128, NSUP * ICOL], I16, tag="idxw", name="idxw")

        # ---- stage Q0: load q, pos, ref; qs = q + p ----
        for k in range(2):
            for half in range(4):
                sl = slice(half * (NQ // 4), (half + 1) * (NQ // 4))
                ptmp = qpool.tile([128, NQ // 4], F32, tag="ptmp", name="ptmp")
                nc.sync.dma_start(out=qs_sb[k][:, sl],
                                  in_=qT[k * 128:(k + 1) * 128, sl])
                nc.sync.dma_start(out=ptmp[:], in_=pT[k * 128:(k + 1) * 128, sl])
                vec.tensor_tensor(out=qs_sb[k][:, sl], in0=qs_sb[k][:, sl],
                                  in1=ptmp[:], op=OP.add)
        nc.sync.dma_start(out=ref_sb[:], in_=refT[:])

        # ---- stage C: coords / masks / gather indices, all supertiles ----
        for g in range(NSUP):
            q0 = g * TQ
            psc = ps_c.tile([128, 160], F32, tag="psc", name="psc")
            for s in range(NSUB):
                qsl = slice(q0 + s * 128, q0 + (s + 1) * 128)
                nc.tensor.matmul(psc[:, s * 40:(s + 1) * 40],
                                 qs_sb[0][:, qsl], wcat_sb[0][:],
                                 start=True, stop=False)
                nc.tensor.matmul(psc[:, s * 40:(s + 1) * 40],
                                 qs_sb[1][:, qsl], wcat_sb[1][:],
                                 start=False, stop=False)
                nc.tensor.matmul(psc[:, s * 40:(s + 1) * 40],
                                 ref_sb[:, qsl], refrhs_sb[:],
                                 start=False, stop=True)
            coords = qpool.tile([128, 160], F32, tag="coords", name="coords")
            act.activation(out=coords[:], in_=psc[:], func=ACT.Copy)

            co = coords[:].rearrange("p (s r) -> p s r", s=NSUB)
            pix = co[:, :, 0:30]                        # (s, hp*ax)
            logit = co[:, :, 30:40]                     # (s, hp)

            # softmax over P
            exlog = qpool.tile([128, NSUB * 10], F32, tag="exlog", name="exlog")
            act.activation(out=exlog[:], in_=logit, func=ACT.Exp)
            ex4 = exlog[:].rearrange("p (s h q) -> p s h q", s=NSUB, h=2)
            sums = qpool.tile([128, NSUB * 2], F32, tag="sums", name="sums")
            vec.tensor_reduce(out=sums[:].rearrange("p (s h) -> p s h", s=NSUB),
                              in_=ex4, axis=AX.X, op=OP.add)
            rsum = qpool.tile([128, NSUB * 2], F32, tag="rsum", name="rsum")
            vec.reciprocal(out=rsum[:], in_=sums[:])
            aw = qpool.tile([128, NSUB * 10], F32, tag="aw", name="aw")
            vec.tensor_tensor(
                out=aw[:].rearrange("p (sh q) -> p sh q", q=5),
                in0=exlog[:].rearrange("p (sh q) -> p sh q", q=5),
                in1=rsum[:].unsqueeze(2).to_broadcast([128, NSUB * 2, 5]),
                op=OP.mult)

            # corner math on the 30 pixel rows
            NPX = NSUB * 30
            # flo = round(pix - 0.5) via the 2^23 magic add (== floor except
            # exactly-integer pix, where the phantom corner gets zero weight)
            flo = qpool.tile([128, NPX], F32, tag="flo", name="flo")
            vec.tensor_scalar(out=flo[:].rearrange("p (s r) -> p s r", s=NSUB),
                              in0=pix, scalar1=8388607.5, scalar2=8388608.0,
                              op0=OP.add, op1=OP.subtract)
            fl3 = flo[:].rearrange("p (s r) -> p s r", s=NSUB)
            frac = qpool.tile([128, NPX], F32, tag="frac", name="frac")
            vec.tensor_tensor(out=frac[:].rearrange("p (s r) -> p s r", s=NSUB),
                              in0=pix, in1=fl3, op=OP.subtract)
            fr3 = frac[:].rearrange("p (s r) -> p s r", s=NSUB)
            # gcorn: (s, hp, ax, dx) — clamped corner coords (biased +64)
            gcorn = qpool.tile([128, NPX * 2], F32, tag="gcorn", name="gcorn")
            gc4 = gcorn[:].rearrange("p (s r d) -> p s r d", s=NSUB, d=2)
            vec.tensor_scalar(out=gc4[:, :, :, 0], in0=fl3,
                              scalar1=64.0, scalar2=111.0, op0=OP.max, op1=OP.min)
            g1m = qpool.tile([128, NPX], F32, tag="g1m", name="g1m")
            vec.tensor_scalar(out=g1m[:], in0=flo[:],
                              scalar1=63.0, scalar2=110.0, op0=OP.max, op1=OP.min)
            vec.tensor_scalar(out=gc4[:, :, :, 1], in0=g1m[:]
                              .rearrange("p (s r) -> p s r", s=NSUB),
                              scalar1=1.0, scalar2=None, op0=OP.add)
            # validity via clip-equality
            v0 = qpool.tile([128, NPX], F32, tag="v0", name="v0")
            vec.tensor_tensor(out=v0[:].rearrange("p (s r) -> p s r", s=NSUB),
                              in0=gc4[:, :, :, 0], in1=fl3, op=OP.is_equal)
            v1 = qpool.tile([128, NPX], F32, tag="v1", name="v1")
            vec.tensor_tensor(out=v1[:], in0=g1m[:], in1=flo[:], op=OP.is_equal)
            # corner weights (x/y rows used; z rows ignored later)
            om = qpool.tile([128, NPX], F32, tag="om", name="om")
            vec.tensor_scalar(out=om[:], in0=frac[:], scalar1=-1.0, scalar2=1.0,
                              op0=OP.mult, op1=OP.add)
            wcorn = qpool.tile([128, NPX * 2], F32, tag="wcorn", name="wcorn")
            wc4 = wcorn[:].rearrange("p (s r d) -> p s r d", s=NSUB, d=2)
            vec.tensor_tensor(out=wc4[:, :, :, 0],
                              in0=om[:].rearrange("p (s r) -> p s r", s=NSUB),
                              in1=v0[:].rearrange("p (s r) -> p s r", s=NSUB),
                              op=OP.mult)
            vec.tensor_tensor(out=wc4[:, :, :, 1],
                              in0=fr3,
                              in1=v1[:].rearrange("p (s r) -> p s r", s=NSUB),
                              op=OP.mult)

            # z window: rzq = floor((gz-64)/4) in [0,11]; W-slot hat weights
            gc6 = gcorn[:].rearrange("p (s hp a d) -> p s hp a d",
                                     s=NSUB, hp=10, a=3)
            gz = gc6[:, :, :, 2, 0]
            pz = co[:, :, 0:30].rearrange("p s (hp a) -> p s hp a", a=3)[:, :, :, 2]
            rzq = qpool.tile([128, NSUB * 10], F32, tag="rzq", name="rzq")
            tq = qpool.tile([128, NSUB * 10], F32, tag="tq", name="tq")
            vec.tensor_scalar(out=tq[:].rearrange("p (s h) -> p s h", s=NSUB),
                              in0=gz, scalar1=0.25, scalar2=16.375,
                              op0=OP.mult, op1=OP.subtract)
            vec.tensor_scalar(out=rzq[:], in0=tq[:],
                              scalar1=8388624.0, scalar2=8388624.0,
                              op0=OP.add, op1=OP.subtract)
            # d0 = (4*rzq + 64) - pz ; dk = d0 + k
            zb4 = qpool.tile([128, NSUB * 10], F32, tag="zb4", name="zb4")
            vec.tensor_scalar(out=zb4[:], in0=rzq[:], scalar1=4.0, scalar2=64.0,
                              op0=OP.mult, op1=OP.add)
            d0 = qpool.tile([128, NSUB * 10], F32, tag="d0", name="d0")
            vec.tensor_tensor(out=d0[:].rearrange("p (s h) -> p s h", s=NSUB),
                              in0=zb4[:].rearrange("p (s h) -> p s h", s=NSUB),
                              in1=pz, op=OP.subtract)
            dk = qpool.tile([128, NSUB * 10 * W], F32, tag="dk", name="dk")
            vec.tensor_tensor(
                out=dk[:].rearrange("p (sh k) -> p sh k", k=W),
                in0=d0[:].unsqueeze(2).to_broadcast([128, NSUB * 10, W]),
                in1=zoff_sb[:].unsqueeze(1).to_broadcast([128, NSUB * 10, W]),
                op=OP.add)
            adk = qpool.tile([128, NSUB * 10 * W], F32, tag="adk", name="adk")
            act.activation(out=adk[:], in_=dk[:], func=ACT.Abs)
            hat = qpool.tile([128, NSUB * 10 * W], F32, tag="hat", name="hat")
            act.activation(out=hat[:], in_=adk[:], func=ACT.Relu,
                           scale=-1.0, bias=1.0)
            # upper bound: slot z 4*rzq+64+k <= 111  <=>  dk <= 111 - pz
            ub = qpool.tile([128, NSUB * 10], F32, tag="ub", name="ub")
            vec.tensor_scalar(out=ub[:].rearrange("p (s h) -> p s h", s=NSUB),
                              in0=pz, scalar1=-1.0, scalar2=111.0,
                              op0=OP.mult, op1=OP.add)
            vub = qpool.tile([128, NSUB * 10 * W], F32, tag="vub", name="vub")
            vec.tensor_tensor(
                out=vub[:].rearrange("p (sh k) -> p sh k", k=W),
                in0=dk[:].rearrange("p (sh k) -> p sh k", k=W),
                in1=ub[:].unsqueeze(2).to_broadcast([128, NSUB * 10, W]),
                op=OP.is_le)
            wz = qpool.tile([128, NSUB * 10 * W], F32, tag="wz", name="wz")
            vec.tensor_tensor(out=wz[:], in0=hat[:], in1=vub[:], op=OP.mult)

            # slot weights: block bx = min(gx0, 110); slot s holds x = bx+s.
            # When gx0 == 111 (x >= 47) the corner-0 weight moves to slot 1.
            wc6 = wcorn[:].rearrange("p (s hp a d) -> p s hp a d",
                                     s=NSUB, hp=10, a=3)
            ws = qpool.tile([128, NSUB * 40], F32, tag="ws", name="ws")
            ws4 = ws[:].rearrange("p (s hp a d) -> p s hp a d", s=NSUB, hp=10, a=2)
            hi = qpool.tile([128, NSUB * 20], F32, tag="hi", name="hi")
            hi3 = hi[:].rearrange("p (s hp a) -> p s hp a", s=NSUB, hp=10)
            hit = qpool.tile([128, NSUB * 20], F32, tag="hit", name="hit")
            hit3 = hit[:].rearrange("p (s hp a) -> p s hp a", s=NSUB, hp=10)
            # hi = (gc0 == 111): corner-0 weight moves to slot 1.
            # lo = (g1m == 63): corner-1 weight (position 64) moves to slot 0.
            vec.tensor_scalar(out=hi3, in0=gc6[:, :, :, 0:2, 0],
                              scalar1=111.0, scalar2=None, op0=OP.is_equal)
            vec.tensor_tensor(out=hit3, in0=wc6[:, :, :, 0:2, 0], in1=hi3,
                              op=OP.mult)
            lo = qpool.tile([128, NSUB * 20], F32, tag="lo", name="lo")
            lo3 = lo[:].rearrange("p (s hp a) -> p s hp a", s=NSUB, hp=10)
            lot = qpool.tile([128, NSUB * 20], F32, tag="lot", name="lot")
            lot3 = lot[:].rearrange("p (s hp a) -> p s hp a", s=NSUB, hp=10)
            g1m3 = g1m[:].rearrange("p (s hp a) -> p s hp a", s=NSUB, hp=10)
            vec.tensor_scalar(out=lo3, in0=g1m3[:, :, :, 0:2],
                              scalar1=63.0, scalar2=None, op0=OP.is_equal)
            vec.tensor_tensor(out=lot3, in0=wc6[:, :, :, 0:2, 1], in1=lo3,
                              op=OP.mult)
            vec.tensor_tensor(out=ws4[:, :, :, :, 0],
                              in0=wc6[:, :, :, 0:2, 0], in1=hit3,
                              op=OP.subtract)
            vec.tensor_tensor(out=ws4[:, :, :, :, 0],
                              in0=ws4[:, :, :, :, 0], in1=lot3, op=OP.add)
            vec.tensor_tensor(out=ws4[:, :, :, :, 1],
                              in0=wc6[:, :, :, 0:2, 1], in1=hit3, op=OP.add)
            vec.tensor_tensor(out=ws4[:, :, :, :, 1],
                              in0=ws4[:, :, :, :, 1], in1=lot3, op=OP.subtract)
            wxs = ws4[:, :, :, 0, :]                    # (s, hp, xslot)
            wys = ws4[:, :, :, 1, :]                    # (s, hp, yslot)

            # mask: m[(s hp), zb, xp, yp, z4] = aw*wxs*wys*wz
            m1 = qpool.tile([128, NSUB * 20], F32, tag="m1", name="m1")
            vec.tensor_tensor(
                out=m1[:].rearrange("p (sh xp) -> p sh xp", xp=2),
                in0=aw[:].unsqueeze(2).to_broadcast([128, NSUB * 10, 2]),
                in1=wxs.rearrange("p s hp xp -> p (s hp) xp"), op=OP.mult)
            m2 = qpool.tile([128, NSUB * 40], F32, tag="m2", name="m2")
            vec.tensor_tensor(
                out=m2[:].rearrange("p (sh xp yp) -> p sh xp yp", xp=2, yp=2),
                in0=m1[:].rearrange("p (sh xp) -> p sh xp", xp=2)
                    .unsqueeze(3).to_broadcast([128, NSUB * 10, 2, 2]),
                in1=wys.rearrange("p s hp yp -> p (s hp) yp")
                    .unsqueeze(2).to_broadcast([128, NSUB * 10, 2, 2]),
                op=OP.mult)
            # mtmp layout (sh, zb, z4, xy); all three APs are 3-free-dim
            mtmp = qpool.tile([128, NSUB * 10 * 32], F32, tag="mtmp",
                              name="mtmp")
            vec.tensor_tensor(
                out=mtmp[:].rearrange("p (sh zz xy) -> p sh zz xy",
                                      zz=8, xy=4),
                in0=m2[:].rearrange("p (sh xy) -> p sh xy", xy=4)
                    .unsqueeze(2).to_broadcast([128, NSUB * 10, 8, 4]),
                in1=wz[:].rearrange("p (sh zz) -> p sh zz", zz=8)
                    .unsqueeze(3).to_broadcast([128, NSUB * 10, 8, 4]),
                op=OP.mult)
            # permute (zb, z4, xy) -> (zb, xy, z4); (sh, zb) merges both sides
            mv = maskb_all[:, g * MCOL:(g + 1) * MCOL]
            vec.tensor_copy(
                out=mv.rearrange("p (szb xy z4) -> p szb xy z4", xy=4, z4=4),
                in_=mtmp[:].rearrange("p (szb z4 xy) -> p szb xy z4",
                                      z4=4, xy=4))

            # gather row indices: 576*(bx-64) + 12*(by-64) + rzq
            bx = qpool.tile([128, NSUB * 10], F32, tag="bx", name="bx")
            vec.tensor_scalar(out=bx[:].rearrange("p (s h) -> p s h", s=NSUB),
                              in0=gc6[:, :, :, 0, 0], scalar1=110.0,
                              scalar2=576.0, op0=OP.min, op1=OP.mult)
            by = qpool.tile([128, NSUB * 10], F32, tag="by", name="by")
            vec.tensor_scalar(out=by[:].rearrange("p (s h) -> p s h", s=NSUB),
                              in0=gc6[:, :, :, 1, 0], scalar1=110.0,
                              scalar2=12.0, op0=OP.min, op1=OP.mult)
            t4 = qpool.tile([128, NSUB * 10], F32, tag="t4", name="t4")
            vec.tensor_tensor(out=t4[:], in0=bx[:], in1=by[:], op=OP.add)
            idxf = qpool.tile([128, NSUB * 10], F32, tag="idxf", name="idxf")
            vec.tensor_scalar(out=idxf[:], in0=t4[:],
                              scalar1=37632.0, scalar2=None, op0=OP.subtract)
            vec.tensor_tensor(out=idxf[:], in0=idxf[:], in1=rzq[:], op=OP.add)
            idx16 = qpool.tile([128, NSUB * 10], I16, tag="idx16", name="idx16")
            vec.tensor_copy(out=idx16[:], in_=idxf[:])

            # idx fold into the gather's 16-partition-wrapped layout:
            # bounce through DRAM with contiguous 320B packets, then a DVE
            # column permute (s2-major -> s2-interleaved).
            nc.sync.dma_start(out=idxscr[g * 128:(g + 1) * 128, :], in_=idx16[:])
            idxr = gpool.tile([128, ICOL], I16, tag="idxr", name="idxr")
            scr = idxscr[g * 128:(g + 1) * 128, :]
            for gg in range(8):
                # idxr[16*gg + p16, s2*40 + f] = idx16[s2*16 + p16, f]
                src = bass.AP(scr.tensor, scr.offset,
                              [[40, 16], [16 * 40, 8], [1, 40]])
                nc.sync.dma_start(
                    out=idxr[gg * 16:(gg + 1) * 16, :]
                        .rearrange("p (s2 f) -> p s2 f", s2=8),
                    in_=src)
            # permute cols: (s2, subhl, r) -> (subhl, r, s2)
            vec.tensor_copy(
                out=idxw_all[:, g * ICOL:(g + 1) * ICOL]
                    .rearrange("p (sh r s2) -> p sh r s2", sh=8, r=5),
                in_=idxr[:].rearrange("p (s2 sh r) -> p sh r s2", s2=8, sh=8))

        # ---- stage V: value projection (bf16) ----
        # Zero the 12 units at block (46, 47) whose (xp=1, yp=1) slots no
        # write pass covers but the z-window overrun can read, plus the back
        # guard.
        zpad = const.tile([16, 512], VDT, tag="zpad", name="zpad")
        vec.memset(zpad[:], 0.0)
        for hl in range(2):
            zb46 = (46 * 576 + 47 * 12)
            nc.sync.dma_start(
                out=bass.AP(vexp[:].tensor,
                            hl * VHEAD + (G0 + zb46) * 512 + 256 + 128,
                            [[512, 12], [1, 128]]),
                in_=zpad[0:12, 0:128])
            nc.sync.dma_start(
                out=bass.AP(vexp[:].tensor, hl * VHEAD + (G0 + NUNIT) * 512,
                            [[512, 16], [1, 512]]),
                in_=zpad[:])
        for vg in range(NVSUP // NR):          # flush groups of NR supertiles
            vb = [vpool.tile([128, NR * 256], VDT, tag=f"vb{hl}", name=f"vb{hl}")
                  for hl in range(2)]
            for i in range(NR):
                vt = vg * NR + i
                vin = [vpool.tile([128, VSUP], VDT, tag=f"vin{k}", name=f"vin{k}")
                       for k in range(2)]
                for k in range(2):
                    nc.sync.dma_start(
                        out=vin[k][:],
                        in_=value_in[k * 128:(k + 1) * 128,
                                     vt * VSUP:(vt + 1) * VSUP])
                psv = ps_v.tile([128, 512], F32, tag="psv", name="psv")
                for s in range(8):
                    lhs0 = vin[0][:].rearrange("p (v e) -> p e v", e=8)[:, s, :]
                    lhs1 = vin[1][:].rearrange("p (v e) -> p e v", e=8)[:, s, :]
                    nc.tensor.matmul(psv[:, s * 64:(s + 1) * 64], lhs0,
                                     wval_sb[0][:], start=True, stop=False)
                    nc.tensor.matmul(psv[:, s * 64:(s + 1) * 64], lhs1,
                                     wval_sb[1][:], start=False, stop=True)
                # split heads, add bias, pack (blk, c, v4) per 4-voxel block
                for hl in range(2):
                    src = psv[:].rearrange("p (blk v4 hc) -> p blk hc v4",
                                           blk=2, v4=4)[:, :, hl * 32:(hl + 1) * 32, :]
                    bv = bval_sb[:, hl * 32:(hl + 1) * 32] \
                        .unsqueeze(1).unsqueeze(3).to_broadcast([128, 2, 32, 4])
                    vec.tensor_tensor(
                        out=vb[hl][:, i * 256:(i + 1) * 256]
                            .rearrange("p (blk c v4) -> p blk c v4", blk=2, c=32),
                        in0=src, in1=bv, op=OP.add)
            # flush NR supertiles (NR*1024 voxels) per head: 4 quadrant
            # passes; pass (xp, yp) lands vblock at unit vblock-xp*576-yp*12
            # slot (xp, yp). Runs are 256B ((c, z4) per vblock).
            for hl in range(2):
                for xp in range(2):
                    for yp in range(2):
                        base = (hl * VHEAD
                                + (G0 + vg * NR * 256 - xp * 576 - yp * 12)
                                * 512 + xp * 256 + yp * 128)
                        for blk in range(2):
                            dst = bass.AP(vexp[:].tensor, base + blk * 512,
                                          [[1024, 128], [256 * 512, NR],
                                           [1, 128]])
                            src = vb[hl][:] \
                                .rearrange("p (i blk x) -> p i blk x",
                                           i=NR, blk=2)[:, :, blk, :]
                            eng = nc.sync if (xp * 2 + yp) % 2 == 0 \
                                else nc.scalar
                            eng.dma_start(out=dst, in_=src)

        # ---- stage G: gather + weighted reduce per (supertile, subtile, head) ----
        for g in range(NSUP):
            q0 = g * TQ
            S = qpool.tile([128, NSUB * 64], F32, tag="S", name="S")
            for s in range(NSUB):
                for hl in range(2):
                    G = gpool.tile([128, 5 * 1024], VDT, tag="G", name="G")
                    in_g = bass.AP(vexp[:].tensor, hl * VHEAD + G0 * 512,
                                   [[512, NUNIT], [1, 1024]])
                    nc.gpsimd.dma_gather(
                        out_ap=G[:].rearrange("p (i e) -> p i e", i=5),
                        in_ap=in_g,
                        idxs_ap=idxw_all[:, (g * 8 + s * 2 + hl) * 40:
                                         (g * 8 + s * 2 + hl + 1) * 40],
                        num_idxs=NIDX, num_idxs_reg=NIDX,
                        elem_size=1024, elem_step=512,
                        single_packet=False,
                        queue_num=(s * 2 + hl) % 4)
                    # P = G * mask in place; G row = (zb, xp, yp, c, z4),
                    # mask cols (pt, zb, xy, z4) bcast over c
                    moff = g * MCOL + (s * 2 + hl) * 160
                    mg = maskb_all[:, moff:moff + 160] \
                        .rearrange("p (rb z4) -> p rb z4", z4=4) \
                        .unsqueeze(2).to_broadcast([128, 40, 32, 4])
                    gv = G[:].rearrange("p (rb c z4) -> p rb c z4",
                                        rb=40, c=32)
                    vec.tensor_tensor(out=gv, in0=gv, in1=mg, op=OP.mult)
                    # two-step reduce: over z4 (contiguous), then over rb
                    PtA = gpool.tile([128, 40 * 32], F32, tag="PtA", name="PtA")
                    vec.tensor_reduce(
                        out=PtA[:].rearrange("p (rb c) -> p rb c", rb=40),
                        in_=gv, axis=AX.X, op=OP.add)
                    vec.tensor_reduce(
                        out=S[:, s * 64 + hl * 32:s * 64 + hl * 32 + 32],
                        in_=PtA[:].rearrange("p (rb c) -> p c rb", rb=40),
                        axis=AX.X, op=OP.add)

            # transpose S [128, 64] -> [64, 128] per subtile
            for s in range(NSUB):
                pst = ps_t.tile([64, 128], F32, tag="pst", name="pst")
                nc.tensor.transpose(pst[:], S[:, s * 64:(s + 1) * 64], ident[:])
                act.activation(out=st_sb[:, q0 + s * 128:q0 + (s + 1) * 128],
                               in_=pst[:], func=ACT.Copy)

        # ---- GEMM2: outT = wout^T @ ST (float32r moving, N=512) ----
        for mc in range(2):
            for ntile in range(NQ // 512):
                ps2 = ps_c.tile([128, 512], F32, tag="ps2", name="ps2")
                nc.tensor.matmul(ps2[:],
                                 wout_sb[:, mc * 128:(mc + 1) * 128],
                                 st_sb[:, ntile * 512:(ntile + 1) * 512],
                                 start=True, stop=True)
                ob = opool.tile([128, 512], F32, tag="ob", name="ob")
                vec.tensor_copy(out=ob[:], in_=ps2[:])
                nc.sync.dma_start(
                    out=outp[mc * 128:(mc + 1) * 128,
                             ntile * 512:(ntile + 1) * 512],
                    in_=ob[:])

    nc.compile()
    return nc


def _prep_core_inputs(inputs, b, j):
    import ml_dtypes
    q = np.ascontiguousarray(inputs["query"][b].T, np.float32)
    p = np.ascontiguousarray(inputs["pos"][b].T, np.float32)
    r = np.concatenate([inputs["reference_points"][b].T,
                        np.ones((1, NQ), np.float32)]).astype(np.float32)
    r = np.ascontiguousarray(r)
    value = np.ascontiguousarray(
        inputs["value"][b].reshape(C, NVOX)).astype(ml_dtypes.bfloat16)

    W_off, b_off = inputs["W_off"], inputs["b_off"]
    W_attn, b_attn = inputs["W_attn"], inputs["b_attn"]
    heads = [2 * j, 2 * j + 1]
    rows, biases, refr = [], [], []
    for h in heads:
        for pp in range(P):
            for ax in range(3):
                rows.append(W_off[(h * P + pp) * 3 + ax])
                biases.append(b_off[(h * P + pp) * 3 + ax] - 0.5 + 64.0)
                e = np.zeros(3, np.float32)
                e[ax] = GRID
                refr.append(e)
    for h in heads:
        for pp in range(P):
            rows.append(W_attn[h * P + pp])
            biases.append(b_attn[h * P + pp])
            refr.append(np.zeros(3, np.float32))
    wcat = np.ascontiguousarray(np.stack(rows).T, np.float32)       # (256, 40)
    ref_rhs = np.concatenate(
        [np.stack(refr).T, np.asarray(biases, np.float32)[None, :]])
    ref_rhs = np.ascontiguousarray(ref_rhs, np.float32)             # (4, 40)

    wval = np.ascontiguousarray(
        inputs["W_val"][64 * j:64 * j + 64].T).astype(ml_dtypes.bfloat16)
    bval = np.ascontiguousarray(
        np.repeat(inputs["b_val"][64 * j:64 * j + 64][None, :], 128, axis=0),
        np.float32)
    wout = np.ascontiguousarray(inputs["W_out"][:, 64 * j:64 * j + 64].T,
                                np.float32)
    zoffs = np.repeat(np.arange(W, dtype=np.float32)[None, :], 128, axis=0)
    return {
        "value_in": value, "qT": q, "pT": p, "refT": r,
        "wcat": wcat, "ref_rhs": ref_rhs,
        "wval": wval, "bval": bval, "wout": wout, "zoff": zoffs,
    }


def get_nc():
    global _NC_CACHE
    if _NC_CACHE is None:
        _NC_CACHE = build_nc()
    return _NC_CACHE


def kernel(**inputs):
    from concourse.bass_utils import run_bass_kernel_spmd

    inputs = {k: np.asarray(v) for k, v in inputs.items()}
    nc = get_nc()
    in_maps = [_prep_core_inputs(inputs, core // 4, core % 4) for core in range(8)]
    res = run_bass_kernel_spmd(nc, in_maps, list(range(8)))
    bs = inputs["query"].shape[0]
    out = np.zeros((bs, NQ, C), np.float32)
    for core in range(8):
        out[core // 4] += res.results[core]["outp"].T
    out += inputs["b_out"][None, None, :].astype(np.float32)
    return out


# revision 23
# speedup vs baseline: 3.1067x; 1.0405x over previous
"""Trainium2 Bass kernel for 3D deformable attention (8 NeuronCores).

Sharding: core i handles batch b = i // 4 and head-pair j = i % 4
(heads 2j, 2j+1, i.e. value/out channels [64j, 64j+64)).

Per-core device pipeline (emission order = coords first so DVE mask work
overlaps the value-projection DMA/PE stage):
  C. per query-supertile (512 q): coords = qs^T @ Wcat^T + [48*ref | b] (PE);
     softmax over 5 points, clamped trilinear corner weights, z-window "hat"
     weights, combined mask m = aw*wx*wy*wz (DVE, bf16); int16 gather row
     indices; idx fold into dma_gather's 16-partition-wrapped layout via a
     contiguous DRAM bounce (320B packets) + DVE column permute.
  V. value projection v = W_val[64j:64j+64] @ value[b] in bf16 (PE,
     voxel-stationary), + b_val; packed per 4-voxel block as (c, v4) and
     flushed to DRAM as two head blocks of [NVOX+8 vox, 32ch] rows.
  G. per (query-subtile, head): one dma_gather of 2560 rows (8 vox x 32ch
     bf16 = 512B each); P = G * mask (DVE bf16, contiguous); two-step
     reduce (over v4 contiguous, then over (pt,xy,blk)) -> S[q, 64].
  O. PE transpose of S, then out^T = Wout^T @ S^T (float32r), DMA out.
Host combines: out[b] = sum_j outp_j^T + b_out.
"""
import numpy as np

import concourse.bass as bass
import concourse.mybir as mybir
from concourse import bacc, tile
from concourse.masks import make_identity
from contextlib import ExitStack

F32 = mybir.dt.float32
F32R = mybir.dt.float32r
I16 = mybir.dt.int16
AX = mybir.AxisListType
OP = mybir.AluOpType
ACT = mybir.ActivationFunctionType

H, P = 8, 5
NQ, C, GRID = 4096, 256, 48
NVOX = GRID ** 3            # 110592
NSUB = 4                    # query subtiles (of 128) per supertile
TQ = 128 * NSUB             # 512
NSUP = NQ // TQ             # 8
VSUP = 1024                 # voxels per value-proj supertile
NVSUP = NVOX // VSUP        # 108
NR = 4                      # value supertiles per DRAM flush

VDT = mybir.dt.bfloat16
W = 8                       # z-window voxels per gathered row
# vexp: quad-interleaved expanded volume. Per head, blocks (x0, y0) of
# 12 z-units; unit = (xp, yp, c, z4) = 512 els (4-z slab x 4 quadrants x
# 32ch). A gather row = 2 consecutive units = the full 2x2x(8z) trilinear
# neighborhood of one sample point. unit(vblock) = vblock - xp*576 - yp*12
# is linear in vblock, so the 4 write passes keep 256B-contiguous runs.
NUNIT = 48 * 48 * 12        # 27648 addressable units per head
G0 = 588                    # front guard units (absorbs xp/yp shifts)
G1 = 16                     # back guard units (zeroed; z-window overrun)
VHEAD = (G0 + NUNIT + G1) * 512
NIDX = 5 * 128              # rows per (subtile, head) gather
MCOL = NSUB * 40 * W        # mask columns per supertile (1280)
ICOL = NSUB * 2 * 40        # idx columns per supertile (320)

_NC_CACHE = None


def build_nc():
    nc = bacc.Bacc("TRN2", target_bir_lowering=False, debug=False, num_devices=8,
                   num_swdge_queues=4)

    value_in = nc.dram_tensor("value_in", [C, NVOX], VDT, kind="ExternalInput")
    qT = nc.dram_tensor("qT", [C, NQ], F32, kind="ExternalInput")
    pT = nc.dram_tensor("pT", [C, NQ], F32, kind="ExternalInput")
    refT = nc.dram_tensor("refT", [4, NQ], F32, kind="ExternalInput")
    wcat = nc.dram_tensor("wcat", [C, 40], F32, kind="ExternalInput")
    ref_rhs = nc.dram_tensor("ref_rhs", [4, 40], F32, kind="ExternalInput")
    wval = nc.dram_tensor("wval", [C, 64], VDT, kind="ExternalInput")
    bval = nc.dram_tensor("bval", [128, 64], F32, kind="ExternalInput")
    wout = nc.dram_tensor("wout", [64, C], F32, kind="ExternalInput")
    zoff = nc.dram_tensor("zoff", [128, W], F32, kind="ExternalInput")
    outp = nc.dram_tensor("outp", [C, NQ], F32, kind="ExternalOutput")
    vexp = nc.dram_tensor("vexp", [2 * VHEAD], VDT)
    idxscr = nc.dram_tensor("idxscr", [NSUP * 128, 40], I16)

    vec = nc.vector
    act = nc.scalar

    with tile.TileContext(nc) as tc, ExitStack() as ctx:
        const = ctx.enter_context(tc.tile_pool(name="const", bufs=1))
        vpool = ctx.enter_context(tc.tile_pool(name="vpool", bufs=2))
        qpool = ctx.enter_context(tc.tile_pool(name="qpool", bufs=2))
        gpool = ctx.enter_context(tc.tile_pool(name="gpool", bufs=2))
        opool = ctx.enter_context(tc.tile_pool(name="opool", bufs=2))
        ps_v = ctx.enter_context(tc.tile_pool(name="ps_v", bufs=2, space="PSUM"))
        ps_c = ctx.enter_context(tc.tile_pool(name="ps_c", bufs=2, space="PSUM"))
        ps_t = ctx.enter_context(tc.tile_pool(name="ps_t", bufs=2, space="PSUM"))

        # ---- constants into SBUF ----
        wcat_sb = [const.tile([128, 40], F32, tag=f"wcat{k}", name=f"wcat{k}")
                   for k in range(2)]
        for k in range(2):
            nc.sync.dma_start(out=wcat_sb[k][:], in_=wcat[k * 128:(k + 1) * 128, :])
        refrhs_sb = const.tile([4, 40], F32, tag="refrhs", name="refrhs")
        nc.sync.dma_start(out=refrhs_sb[:], in_=ref_rhs[:])
        wval_sb = [const.tile([128, 64], VDT, tag=f"wval{k}", name=f"wval{k}")
                   for k in range(2)]
        for k in range(2):
            nc.sync.dma_start(out=wval_sb[k][:], in_=wval[k * 128:(k + 1) * 128, :])
        bval_sb = const.tile([128, 64], F32, tag="bval", name="bval")
        nc.sync.dma_start(out=bval_sb[:], in_=bval[:])
        wout_sb = const.tile([64, C], F32, tag="wout", name="wout")
        nc.sync.dma_start(out=wout_sb[:], in_=wout[:])
        zoff_sb = const.tile([128, W], F32, tag="zoff", name="zoff")
        nc.sync.dma_start(out=zoff_sb[:], in_=zoff[:])
        ident = const.tile([128, 128], F32, tag="ident", name="ident")
        make_identity(nc, ident[:])

        # persistent big buffers
        qs_sb = [const.tile([128, NQ], F32, tag=f"qs{k}", name=f"qs{k}")
                 for k in range(2)]
        ref_sb = const.tile([4, NQ], F32, tag="refq", name="refq")
        st_sb = const.tile([64, NQ], F32, tag="st", name="st")
        maskb_all = const.tile([128, NSUP * MCOL], VDT, tag="maskb", name="maskb")
        idxw_all = const.tile([128, NSUP * ICOL], I16, tag="idxw", name="idxw")

        # ---- stage Q0: load q, pos, ref; qs = q + p ----
        for k in range(2):
            for half in range(4):
                sl = slice(half * (NQ // 4), (half + 1) * (NQ // 4))
                ptmp = qpool.tile([128, NQ // 4], F32, tag="ptmp", name="ptmp")
                nc.sync.dma_start(out=qs_sb[k][:, sl],
                                  in_=qT[k * 128:(k + 1) * 128, sl])
                nc.sync.dma_start(out=ptmp[:], in_=pT[k * 128:(k + 1) * 128, sl])
                vec.tensor_tensor(out=qs_sb[k][:, sl], in0=qs_sb[k][:, sl],
                                  in1=ptmp[:], op=OP.add)
        nc.sync.dma_start(out=ref_sb[:], in_=refT[:])

        # ---- stage C: coords / masks / gather indices, all supertiles ----
        for g in range(NSUP):
            q0 = g * TQ
            psc = ps_c.tile([128, 160], F32, tag="psc", name="psc")
            for s in range(NSUB):
                qsl = slice(q0 + s * 128, q0 + (s + 1) * 128)
                nc.tensor.matmul(psc[:, s * 40:(s + 1) * 40],
                                 qs_sb[0][:, qsl], wcat_sb[0][:],
                                 start=True, stop=False)
                nc.tensor.matmul(psc[:, s * 40:(s + 1) * 40],
                                 qs_sb[1][:, qsl], wcat_sb[1][:],
                                 start=False, stop=False)
                nc.tensor.matmul(psc[:, s * 40:(s + 1) * 40],
                                 ref_sb[:, qsl], refrhs_sb[:],
                                 start=False, stop=True)
            coords = qpool.tile([128, 160], F32, tag="coords", name="coords")
            act.activation(out=coords[:], in_=psc[:], func=ACT.Copy)

            co = coords[:].rearrange("p (s r) -> p s r", s=NSUB)
            pix = co[:, :, 0:30]                        # (s, hp*ax)
            logit = co[:, :, 30:40]                     # (s, hp)

            # softmax over P
            exlog = qpool.tile([128, NSUB * 10], F32, tag="exlog", name="exlog")
            act.activation(out=exlog[:], in_=logit, func=ACT.Exp)
            ex4 = exlog[:].rearrange("p (s h q) -> p s h q", s=NSUB, h=2)
            sums = qpool.tile([128, NSUB * 2], F32, tag="sums", name="sums")
            vec.tensor_reduce(out=sums[:].rearrange("p (s h) -> p s h", s=NSUB),
                              in_=ex4, axis=AX.X, op=OP.add)
            rsum = qpool.tile([128, NSUB * 2], F32, tag="rsum", name="rsum")
            vec.reciprocal(out=rsum[:], in_=sums[:])
            aw = qpool.tile([128, NSUB * 10], F32, tag="aw", name="aw")
            vec.tensor_tensor(
                out=aw[:].rearrange("p (sh q) -> p sh q", q=5),
                in0=exlog[:].rearrange("p (sh q) -> p sh q", q=5),
                in1=rsum[:].unsqueeze(2).to_broadcast([128, NSUB * 2, 5]),
                op=OP.mult)

            # corner math on the 30 pixel rows
            NPX = NSUB * 30
            # flo = round(pix - 0.5) via the 2^23 magic add (== floor except
            # exactly-integer pix, where the phantom corner gets zero weight)
            flo = qpool.tile([128, NPX], F32, tag="flo", name="flo")
            vec.tensor_scalar(out=flo[:].rearrange("p (s r) -> p s r", s=NSUB),
                              in0=pix, scalar1=8388607.5, scalar2=8388608.0,
                              op0=OP.add, op1=OP.subtract)
            fl3 = flo[:].rearrange("p (s r) -> p s r", s=NSUB)
            frac = qpool.tile([128, NPX], F32, tag="frac", name="frac")
            vec.tensor_tensor(out=frac[:].rearrange("p (s r) -> p s r", s=NSUB),
                              in0=pix, in1=fl3, op=OP.subtract)
            fr3 = frac[:].rearrange("p (s r) -> p s r", s=NSUB)
            # gcorn: (s, hp, ax, dx) — clamped corner coords (biased +64)
            gcorn = qpool.tile([128, NPX * 2], F32, tag="gcorn", name="gcorn")
            gc4 = gcorn[:].rearrange("p (s r d) -> p s r d", s=NSUB, d=2)
            vec.tensor_scalar(out=gc4[:, :, :, 0], in0=fl3,
                              scalar1=64.0, scalar2=111.0, op0=OP.max, op1=OP.min)
            g1m = qpool.tile([128, NPX], F32, tag="g1m", name="g1m")
            vec.tensor_scalar(out=g1m[:], in0=flo[:],
                              scalar1=63.0, scalar2=110.0, op0=OP.max, op1=OP.min)
            vec.tensor_scalar(out=gc4[:, :, :, 1], in0=g1m[:]
                              .rearrange("p (s r) -> p s r", s=NSUB),
                              scalar1=1.0, scalar2=None, op0=OP.add)
            # validity via clip-equality
            v0 = qpool.tile([128, NPX], F32, tag="v0", name="v0")
            vec.tensor_tensor(out=v0[:].rearrange("p (s r) -> p s r", s=NSUB),
                              in0=gc4[:, :, :, 0], in1=fl3, op=OP.is_equal)
            v1 = qpool.tile([128, NPX], F32, tag="v1", name="v1")
            vec.tensor_tensor(out=v1[:], in0=g1m[:], in1=flo[:], op=OP.is_equal)
            # corner weights (x/y rows used; z rows ignored later)
            om = qpool.tile([128, NPX], F32, tag="om", name="om")
            vec.tensor_scalar(out=om[:], in0=frac[:], scalar1=-1.0, scalar2=1.0,
                              op0=OP.mult, op1=OP.add)
            wcorn = qpool.tile([128, NPX * 2], F32, tag="wcorn", name="wcorn")
            wc4 = wcorn[:].rearrange("p (s r d) -> p s r d", s=NSUB, d=2)
            vec.tensor_tensor(out=wc4[:, :, :, 0],
                              in0=om[:].rearrange("p (s r) -> p s r", s=NSUB),
                              in1=v0[:].rearrange("p (s r) -> p s r", s=NSUB),
                              op=OP.mult)
            vec.tensor_tensor(out=wc4[:, :, :, 1],
                              in0=fr3,
                              in1=v1[:].rearrange("p (s r) -> p s r", s=NSUB),
                              op=OP.mult)

            # z window: rzq = floor((gz-64)/4) in [0,11]; W-slot hat weights
            gc6 = gcorn[:].rearrange("p (s hp a d) -> p s hp a d",
                                     s=NSUB, hp=10, a=3)
            gz = gc6[:, :, :, 2, 0]
            pz = co[:, :, 0:30].rearrange("p s (hp a) -> p s hp a", a=3)[:, :, :, 2]
            rzq = qpool.tile([128, NSUB * 10], F32, tag="rzq", name="rzq")
            tq = qpool.tile([128, NSUB * 10], F32, tag="tq", name="tq")
            vec.tensor_scalar(out=tq[:].rearrange("p (s h) -> p s h", s=NSUB),
                              in0=gz, scalar1=0.25, scalar2=16.375,
                              op0=OP.mult, op1=OP.subtract)
            vec.tensor_scalar(out=rzq[:], in0=tq[:],
                              scalar1=8388624.0, scalar2=8388624.0,
                              op0=OP.add, op1=OP.subtract)
            # d0 = (4*rzq + 64) - pz ; dk = d0 + k
            zb4 = qpool.tile([128, NSUB * 10], F32, tag="zb4", name="zb4")
            vec.tensor_scalar(out=zb4[:], in0=rzq[:], scalar1=4.0, scalar2=64.0,
                              op0=OP.mult, op1=OP.add)
            d0 = qpool.tile([128, NSUB * 10], F32, tag="d0", name="d0")
            vec.tensor_tensor(out=d0[:].rearrange("p (s h) -> p s h", s=NSUB),
                              in0=zb4[:].rearrange("p (s h) -> p s h", s=NSUB),
                              in1=pz, op=OP.subtract)
            dk = qpool.tile([128, NSUB * 10 * W], F32, tag="dk", name="dk")
            vec.tensor_tensor(
                out=dk[:].rearrange("p (sh k) -> p sh k", k=W),
                in0=d0[:].unsqueeze(2).to_broadcast([128, NSUB * 10, W]),
                in1=zoff_sb[:].unsqueeze(1).to_broadcast([128, NSUB * 10, W]),
                op=OP.add)
            adk = qpool.tile([128, NSUB * 10 * W], F32, tag="adk", name="adk")
            act.activation(out=adk[:], in_=dk[:], func=ACT.Abs)
            hat = qpool.tile([128, NSUB * 10 * W], F32, tag="hat", name="hat")
            act.activation(out=hat[:], in_=adk[:], func=ACT.Relu,
                           scale=-1.0, bias=1.0)
            # upper bound: slot z 4*rzq+64+k <= 111  <=>  dk <= 111 - pz
            ub = qpool.tile([128, NSUB * 10], F32, tag="ub", name="ub")
            vec.tensor_scalar(out=ub[:].rearrange("p (s h) -> p s h", s=NSUB),
                              in0=pz, scalar1=-1.0, scalar2=111.0,
                              op0=OP.mult, op1=OP.add)
            vub = qpool.tile([128, NSUB * 10 * W], F32, tag="vub", name="vub")
            vec.tensor_tensor(
                out=vub[:].rearrange("p (sh k) -> p sh k", k=W),
                in0=dk[:].rearrange("p (sh k) -> p sh k", k=W),
                in1=ub[:].unsqueeze(2).to_broadcast([128, NSUB * 10, W]),
                op=OP.is_le)
            wz = qpool.tile([128, NSUB * 10 * W], F32, tag="wz", name="wz")
            vec.tensor_tensor(out=wz[:], in0=hat[:], in1=vub[:], op=OP.mult)

            # slot weights: block bx = min(gx0, 110); slot s holds x = bx+s.
            # When gx0 == 111 (x >= 47) the corner-0 weight moves to slot 1.
            wc6 = wcorn[:].rearrange("p (s hp a d) -> p s hp a d",
                                     s=NSUB, hp=10, a=3)
            ws = qpool.tile([128, NSUB * 40], F32, tag="ws", name="ws")
            ws4 = ws[:].rearrange("p (s hp a d) -> p s hp a d", s=NSUB, hp=10, a=2)
            hi = qpool.tile([128, NSUB * 20], F32, tag="hi", name="hi")
            hi3 = hi[:].rearrange("p (s hp a) -> p s hp a", s=NSUB, hp=10)
            hit = qpool.tile([128, NSUB * 20], F32, tag="hit", name="hit")
            hit3 = hit[:].rearrange("p (s hp a) -> p s hp a", s=NSUB, hp=10)
            # hi = (gc0 == 111): corner-0 weight moves to slot 1.
            # lo = (g1m == 63): corner-1 weight (position 64) moves to slot 0.
            vec.tensor_scalar(out=hi3, in0=gc6[:, :, :, 0:2, 0],
                              scalar1=111.0, scalar2=None, op0=OP.is_equal)
            vec.tensor_tensor(out=hit3, in0=wc6[:, :, :, 0:2, 0], in1=hi3,
                              op=OP.mult)
            lo = qpool.tile([128, NSUB * 20], F32, tag="lo", name="lo")
            lo3 = lo[:].rearrange("p (s hp a) -> p s hp a", s=NSUB, hp=10)
            lot = qpool.tile([128, NSUB * 20], F32, tag="lot", name="lot")
            lot3 = lot[:].rearrange("p (s hp a) -> p s hp a", s=NSUB, hp=10)
            g1m3 = g1m[:].rearrange("p (s hp a) -> p s hp a", s=NSUB, hp=10)
            vec.tensor_scalar(out=lo3, in0=g1m3[:, :, :, 0:2],
                              scalar1=63.0, scalar2=None, op0=OP.is_equal)
            vec.tensor_tensor(out=lot3, in0=wc6[:, :, :, 0:2, 1], in1=lo3,
                              op=OP.mult)
            vec.tensor_tensor(out=ws4[:, :, :, :, 0],
                              in0=wc6[:, :, :, 0:2, 0], in1=hit3,
                              op=OP.subtract)
            vec.tensor_tensor(out=ws4[:, :, :, :, 0],
                              in0=ws4[:, :, :, :, 0], in1=lot3, op=OP.add)
            vec.tensor_tensor(out=ws4[:, :, :, :, 1],
                              in0=wc6[:, :, :, 0:2, 1], in1=hit3, op=OP.add)
            vec.tensor_tensor(out=ws4[:, :, :, :, 1],
                              in0=ws4[:, :, :, :, 1], in1=lot3, op=OP.subtract)
            wxs = ws4[:, :, :, 0, :]                    # (s, hp, xslot)
            wys = ws4[:, :, :, 1, :]                    # (s, hp, yslot)

            # mask: m[(s hp), zb, xp, yp, z4] = aw*wxs*wys*wz
            m1 = qpool.tile([128, NSUB * 20], F32, tag="m1", name="m1")
            vec.tensor_tensor(
                out=m1[:].rearrange("p (sh xp) -> p sh xp", xp=2),
                in0=aw[:].unsqueeze(2).to_broadcast([128, NSUB * 10, 2]),
                in1=wxs.rearrange("p s hp xp -> p (s hp) xp"), op=OP.mult)
            m2 = qpool.tile([128, NSUB * 40], F32, tag="m2", name="m2")
            vec.tensor_tensor(
                out=m2[:].rearrange("p (sh xp yp) -> p sh xp yp", xp=2, yp=2),
                in0=m1[:].rearrange("p (sh xp) -> p sh xp", xp=2)
                    .unsqueeze(3).to_broadcast([128, NSUB * 10, 2, 2]),
                in1=wys.rearrange("p s hp yp -> p (s hp) yp")
                    .unsqueeze(2).to_broadcast([128, NSUB * 10, 2, 2]),
                op=OP.mult)
            # mtmp layout (sh, zb, z4, xy); all three APs are 3-free-dim
            mtmp = qpool.tile([128, NSUB * 10 * 32], F32, tag="mtmp",
                              name="mtmp")
            vec.tensor_tensor(
                out=mtmp[:].rearrange("p (sh zz xy) -> p sh zz xy",
                                      zz=8, xy=4),
                in0=m2[:].rearrange("p (sh xy) -> p sh xy", xy=4)
                    .unsqueeze(2).to_broadcast([128, NSUB * 10, 8, 4]),
                in1=wz[:].rearrange("p (sh zz) -> p sh zz", zz=8)
                    .unsqueeze(3).to_broadcast([128, NSUB * 10, 8, 4]),
                op=OP.mult)
            # permute (zb, z4, xy) -> (zb, xy, z4); (sh, zb) merges both sides
            mv = maskb_all[:, g * MCOL:(g + 1) * MCOL]
            vec.tensor_copy(
                out=mv.rearrange("p (szb xy z4) -> p szb xy z4", xy=4, z4=4),
                in_=mtmp[:].rearrange("p (szb z4 xy) -> p szb xy z4",
                                      z4=4, xy=4))

            # gather row indices: 576*(bx-64) + 12*(by-64) + rzq
            bx = qpool.tile([128, NSUB * 10], F32, tag="bx", name="bx")
            vec.tensor_scalar(out=bx[:].rearrange("p (s h) -> p s h", s=NSUB),
                              in0=gc6[:, :, :, 0, 0], scalar1=110.0,
                              scalar2=576.0, op0=OP.min, op1=OP.mult)
            by = qpool.tile([128, NSUB * 10], F32, tag="by", name="by")
            vec.tensor_scalar(out=by[:].rearrange("p (s h) -> p s h", s=NSUB),
                              in0=gc6[:, :, :, 1, 0], scalar1=110.0,
                              scalar2=12.0, op0=OP.min, op1=OP.mult)
            t4 = qpool.tile([128, NSUB * 10], F32, tag="t4", name="t4")
            vec.tensor_tensor(out=t4[:], in0=bx[:], in1=by[:], op=OP.add)
            idxf = qpool.tile([128, NSUB * 10], F32, tag="idxf", name="idxf")
            vec.tensor_scalar(out=idxf[:], in0=t4[:],
                              scalar1=37632.0, scalar2=None, op0=OP.subtract)
            vec.tensor_tensor(out=idxf[:], in0=idxf[:], in1=rzq[:], op=OP.add)
            idx16 = qpool.tile([128, NSUB * 10], I16, tag="idx16", name="idx16")
            vec.tensor_copy(out=idx16[:], in_=idxf[:])

            # idx fold into the gather's 16-partition-wrapped layout:
            # bounce through DRAM with contiguous 320B packets, then a DVE
            # column permute (s2-major -> s2-interleaved).
            nc.sync.dma_start(out=idxscr[g * 128:(g + 1) * 128, :], in_=idx16[:])
            idxr = gpool.tile([128, ICOL], I16, tag="idxr", name="idxr")
            scr = idxscr[g * 128:(g + 1) * 128, :]
            for gg in range(8):
                # idxr[16*gg + p16, s2*40 + f] = idx16[s2*16 + p16, f]
                src = bass.AP(scr.tensor, scr.offset,
                              [[40, 16], [16 * 40, 8], [1, 40]])
                nc.sync.dma_start(
                    out=idxr[gg * 16:(gg + 1) * 16, :]
                        .rearrange("p (s2 f) -> p s2 f", s2=8),
                    in_=src)
            # permute cols: (s2, subhl, r) -> (subhl, r, s2)
            vec.tensor_copy(
                out=idxw_all[:, g * ICOL:(g + 1) * ICOL]
                    .rearrange("p (sh r s2) -> p sh r s2", sh=8, r=5),
                in_=idxr[:].rearrange("p (s2 sh r) -> p sh r s2", s2=8, sh=8))

        # ---- stage V: value projection (bf16) ----
        # Zero the 12 units at block (46, 47) whose (xp=1, yp=1) slots no
        # write pass covers but the z-window overrun can read, plus the back
        # guard.
        zpad = const.tile([16, 512], VDT, tag="zpad", name="zpad")
        vec.memset(zpad[:], 0.0)
        for hl in range(2):
            zb46 = (46 * 576 + 47 * 12)
            nc.sync.dma_start(
                out=bass.AP(vexp[:].tensor,
                            hl * VHEAD + (G0 + zb46) * 512 + 256 + 128,
                            [[512, 12], [1, 128]]),
                in_=zpad[0:12, 0:128])
            nc.sync.dma_start(
                out=bass.AP(vexp[:].tensor, hl * VHEAD + (G0 + NUNIT) * 512,
                            [[512, 16], [1, 512]]),
                in_=zpad[:])
        for vg in range(NVSUP // NR):          # flush groups of NR supertiles
            vb = [vpool.tile([128, NR * 256], VDT, tag=f"vb{hl}", name=f"vb{hl}")
                  for hl in range(2)]
            for i in range(NR):
                vt = vg * NR + i
                vin = [vpool.tile([128, VSUP], VDT, tag=f"vin{k}", name=f"vin{k}")
                       for k in range(2)]
                for k in range(2):
                    nc.sync.dma_start(
                        out=vin[k][:],
                        in_=value_in[k * 128:(k + 1) * 128,
                                     vt * VSUP:(vt + 1) * VSUP])
                psv = ps_v.tile([128, 512], F32, tag="psv", name="psv")
                for s in range(8):
                    lhs0 = vin[0][:].rearrange("p (v e) -> p e v", e=8)[:, s, :]
                    lhs1 = vin[1][:].rearrange("p (v e) -> p e v", e=8)[:, s, :]
                    nc.tensor.matmul(psv[:, s * 64:(s + 1) * 64], lhs0,
                                     wval_sb[0][:], start=True, stop=False)
                    nc.tensor.matmul(psv[:, s * 64:(s + 1) * 64], lhs1,
                                     wval_sb[1][:], start=False, stop=True)
                # split heads, add bias, pack (blk, c, v4) per 4-voxel block
                for hl in range(2):
                    src = psv[:].rearrange("p (blk v4 hc) -> p blk hc v4",
                                           blk=2, v4=4)[:, :, hl * 32:(hl + 1) * 32, :]
                    bv = bval_sb[:, hl * 32:(hl + 1) * 32] \
                        .unsqueeze(1).unsqueeze(3).to_broadcast([128, 2, 32, 4])
                    vec.tensor_tensor(
                        out=vb[hl][:, i * 256:(i + 1) * 256]
                            .rearrange("p (blk c v4) -> p blk c v4", blk=2, c=32),
                        in0=src, in1=bv, op=OP.add)
            # flush NR supertiles (NR*1024 voxels) per head: 4 quadrant
            # passes; pass (xp, yp) lands vblock at unit vblock-xp*576-yp*12
            # slot (xp, yp). Runs are 256B ((c, z4) per vblock).
            for hl in range(2):
                for xp in range(2):
                    for yp in range(2):
                        base = (hl * VHEAD
                                + (G0 + vg * NR * 256 - xp * 576 - yp * 12)
                                * 512 + xp * 256 + yp * 128)
                        for blk in range(2):
                            dst = bass.AP(vexp[:].tensor, base + blk * 512,
                                          [[1024, 128], [256 * 512, NR],
                                           [1, 128]])
                            src = vb[hl][:] \
                                .rearrange("p (i blk x) -> p i blk x",
                                           i=NR, blk=2)[:, :, blk, :]
                            eng = nc.sync if (xp * 2 + yp) % 2 == 0 \
                                else nc.scalar
                            eng.dma_start(out=dst, in_=src)

        # ---- stage G: gather + weighted reduce per (supertile, subtile, head) ----
        for g in range(NSUP):
            q0 = g * TQ
            S = qpool.tile([128, NSUB * 64], F32, tag="S", name="S")
            for s in range(NSUB):
                for hl in range(2):
                    G = gpool.tile([128, 5 * 1024], VDT, tag="G", name="G")
                    in_g = bass.AP(vexp[:].tensor, hl * VHEAD + G0 * 512,
                                   [[512, NUNIT], [1, 1024]])
                    nc.gpsimd.dma_gather(
                        out_ap=G[:].rearrange("p (i e) -> p i e", i=5),
                        in_ap=in_g,
                        idxs_ap=idxw_all[:, (g * 8 + s * 2 + hl) * 40:
                                         (g * 8 + s * 2 + hl + 1) * 40],
                        num_idxs=NIDX, num_idxs_reg=NIDX,
                        elem_size=1024, elem_step=512,
                        single_packet=False,
                        queue_num=(s * 2 + hl) % 4)
                    # P = G * mask in place; G row = (zb, xp, yp, c, z4),
                    # mask cols (pt, zb, xy, z4) bcast over c
                    moff = g * MCOL + (s * 2 + hl) * 160
                    mg = maskb_all[:, moff:moff + 160] \
                        .rearrange("p (rb z4) -> p rb z4", z4=4) \
                        .unsqueeze(2).to_broadcast([128, 40, 32, 4])
                    gv = G[:].rearrange("p (rb c z4) -> p rb c z4",
                                        rb=40, c=32)
                    vec.tensor_tensor(out=gv, in0=gv, in1=mg, op=OP.mult)
                    # single fused reduce over (rb, z4), keeping c
                    vec.tensor_reduce(
                        out=S[:, s * 64 + hl * 32:s * 64 + hl * 32 + 32],
                        in_=G[:].rearrange("p (rb c z4) -> p c rb z4",
                                           rb=40, c=32),
                        axis=AX.XY, op=OP.add)

            # transpose S [128, 64] -> [64, 128] per subtile
            for s in range(NSUB):
                pst = ps_t.tile([64, 128], F32, tag="pst", name="pst")
                nc.tensor.transpose(pst[:], S[:, s * 64:(s + 1) * 64], ident[:])
                act.activation(out=st_sb[:, q0 + s * 128:q0 + (s + 1) * 128],
                               in_=pst[:], func=ACT.Copy)

        # ---- GEMM2: outT = wout^T @ ST (float32r moving, N=512) ----
        for mc in range(2):
            for ntile in range(NQ // 512):
                ps2 = ps_c.tile([128, 512], F32, tag="ps2", name="ps2")
                nc.tensor.matmul(ps2[:],
                                 wout_sb[:, mc * 128:(mc + 1) * 128],
                                 st_sb[:, ntile * 512:(ntile + 1) * 512],
                                 start=True, stop=True)
                ob = opool.tile([128, 512], F32, tag="ob", name="ob")
                vec.tensor_copy(out=ob[:], in_=ps2[:])
                nc.sync.dma_start(
                    out=outp[mc * 128:(mc + 1) * 128,
                             ntile * 512:(ntile + 1) * 512],
                    in_=ob[:])

    nc.compile()
    return nc


def _prep_core_inputs(inputs, b, j):
    import ml_dtypes
    q = np.ascontiguousarray(inputs["query"][b].T, np.float32)
    p = np.ascontiguousarray(inputs["pos"][b].T, np.float32)
    r = np.concatenate([inputs["reference_points"][b].T,
                        np.ones((1, NQ), np.float32)]).astype(np.float32)
    r = np.ascontiguousarray(r)
    value = np.ascontiguousarray(
        inputs["value"][b].reshape(C, NVOX)).astype(ml_dtypes.bfloat16)

    W_off, b_off = inputs["W_off"], inputs["b_off"]
    W_attn, b_attn = inputs["W_attn"], inputs["b_attn"]
    heads = [2 * j, 2 * j + 1]
    rows, biases, refr = [], [], []
    for h in heads:
        for pp in range(P):
            for ax in range(3):
                rows.append(W_off[(h * P + pp) * 3 + ax])
                biases.append(b_off[(h * P + pp) * 3 + ax] - 0.5 + 64.0)
                e = np.zeros(3, np.float32)
                e[ax] = GRID
                refr.append(e)
    for h in heads:
        for pp in range(P):
            rows.append(W_attn[h * P + pp])
            biases.append(b_attn[h * P + pp])
            refr.append(np.zeros(3, np.float32))
    wcat = np.ascontiguousarray(np.stack(rows).T, np.float32)       # (256, 40)
    ref_rhs = np.concatenate(
        [np.stack(refr).T, np.asarray(biases, np.float32)[None, :]])
    ref_rhs = np.ascontiguousarray(ref_rhs, np.float32)             # (4, 40)

    wval = np.ascontiguousarray(
        inputs["W_val"][64 * j:64 * j + 64].T).astype(ml_dtypes.bfloat16)
    bval = np.ascontiguousarray(
        np.repeat(inputs["b_val"][64 * j:64 * j + 64][None, :], 128, axis=0),
        np.float32)
    wout = np.ascontiguousarray(inputs["W_out"][:, 64 * j:64 * j + 64].T,
                                np.float32)
    zoffs = np.repeat(np.arange(W, dtype=np.float32)[None, :], 128, axis=0)
    return {
        "value_in": value, "qT": q, "pT": p, "refT": r,
        "wcat": wcat, "ref_rhs": ref_rhs,
        "wval": wval, "bval": bval, "wout": wout, "zoff": zoffs,
    }


def get_nc():
    global _NC_CACHE
    if _NC_CACHE is None:
        _NC_CACHE = build_nc()
    return _NC_CACHE


def kernel(**inputs):
    from concourse.bass_utils import run_bass_kernel_spmd

    inputs = {k: np.asarray(v) for k, v in inputs.items()}
    nc = get_nc()
    in_maps = [_prep_core_inputs(inputs, core // 4, core % 4) for core in range(8)]
    res = run_bass_kernel_spmd(nc, in_maps, list(range(8)))
    bs = inputs["query"].shape[0]
    out = np.zeros((bs, NQ, C), np.float32)
    for core in range(8):
        out[core // 4] += res.results[core]["outp"].T
    out += inputs["b_out"][None, None, :].astype(np.float32)
    return out


# revision 27
# speedup vs baseline: 3.1172x; 1.0034x over previous
"""Trainium2 Bass kernel for 3D deformable attention (8 NeuronCores).

Sharding: core i handles batch b = i // 4 and head-pair j = i % 4
(heads 2j, 2j+1, i.e. value/out channels [64j, 64j+64)).

Per-core device pipeline (emission order = coords first so DVE mask work
overlaps the value-projection DMA/PE stage):
  C. per query-supertile (512 q): coords = qs^T @ Wcat^T + [48*ref | b] (PE);
     softmax over 5 points, clamped trilinear corner weights, z-window "hat"
     weights, combined mask m = aw*wx*wy*wz (DVE, bf16); int16 gather row
     indices; idx fold into dma_gather's 16-partition-wrapped layout via a
     contiguous DRAM bounce (320B packets) + DVE column permute.
  V. value projection v = W_val[64j:64j+64] @ value[b] in bf16 (PE,
     voxel-stationary), + b_val; packed per 4-voxel block as (c, v4) and
     flushed to DRAM as two head blocks of [NVOX+8 vox, 32ch] rows.
  G. per (query-subtile, head): one dma_gather of 2560 rows (8 vox x 32ch
     bf16 = 512B each); P = G * mask (DVE bf16, contiguous); two-step
     reduce (over v4 contiguous, then over (pt,xy,blk)) -> S[q, 64].
  O. PE transpose of S, then out^T = Wout^T @ S^T (float32r), DMA out.
Host combines: out[b] = sum_j outp_j^T + b_out.
"""
import numpy as np

import concourse.bass as bass
import concourse.mybir as mybir
from concourse import bacc, tile
from concourse.masks import make_identity
from contextlib import ExitStack

F32 = mybir.dt.float32
F32R = mybir.dt.float32r
I16 = mybir.dt.int16
AX = mybir.AxisListType
OP = mybir.AluOpType
ACT = mybir.ActivationFunctionType

H, P = 8, 5
NQ, C, GRID = 4096, 256, 48
NVOX = GRID ** 3            # 110592
NSUB = 4                    # query subtiles (of 128) per supertile
TQ = 128 * NSUB             # 512
NSUP = NQ // TQ             # 8
VSUP = 1024                 # voxels per value-proj supertile
NVSUP = NVOX // VSUP        # 108
NR = 4                      # value supertiles per DRAM flush

VDT = mybir.dt.bfloat16
W = 8                       # z-window voxels per gathered row
# vexp: quad-interleaved expanded volume. Per head, blocks (x0, y0) of
# 12 z-units; unit = (xp, yp, c, z4) = 512 els (4-z slab x 4 quadrants x
# 32ch). A gather row = 2 consecutive units = the full 2x2x(8z) trilinear
# neighborhood of one sample point. unit(vblock) = vblock - xp*576 - yp*12
# is linear in vblock, so the 4 write passes keep 256B-contiguous runs.
NUNIT = 48 * 48 * 12        # 27648 addressable units per head
G0 = 588                    # front guard units (absorbs xp/yp shifts)
G1 = 16                     # back guard units (zeroed; z-window overrun)
VHEAD = (G0 + NUNIT + G1) * 512
NIDX = 5 * 128              # rows per (subtile, head) gather
MCOL = NSUB * 40 * W        # mask columns per supertile (1280)
ICOL = NSUB * 2 * 40        # idx columns per supertile (320)

_NC_CACHE = None


def build_nc():
    nc = bacc.Bacc("TRN2", target_bir_lowering=False, debug=False, num_devices=8,
                   num_swdge_queues=4)

    value_in = nc.dram_tensor("value_in", [C, NVOX], VDT, kind="ExternalInput")
    qT = nc.dram_tensor("qT", [C, NQ], F32, kind="ExternalInput")
    pT = nc.dram_tensor("pT", [C, NQ], F32, kind="ExternalInput")
    refT = nc.dram_tensor("refT", [4, NQ], F32, kind="ExternalInput")
    wcat = nc.dram_tensor("wcat", [C, 40], F32, kind="ExternalInput")
    ref_rhs = nc.dram_tensor("ref_rhs", [4, 40], F32, kind="ExternalInput")
    wval = nc.dram_tensor("wval", [C, 64], VDT, kind="ExternalInput")
    bval = nc.dram_tensor("bval", [128, 64], F32, kind="ExternalInput")
    wout = nc.dram_tensor("wout", [64, C], F32, kind="ExternalInput")
    zoff = nc.dram_tensor("zoff", [128, W], F32, kind="ExternalInput")
    outp = nc.dram_tensor("outp", [C, NQ], F32, kind="ExternalOutput")
    vexp = nc.dram_tensor("vexp", [2 * VHEAD], VDT)
    idxscr = nc.dram_tensor("idxscr", [NSUP * 128, 40], I16)

    vec = nc.vector
    act = nc.scalar

    with tile.TileContext(nc) as tc, ExitStack() as ctx:
        const = ctx.enter_context(tc.tile_pool(name="const", bufs=1))
        vpool = ctx.enter_context(tc.tile_pool(name="vpool", bufs=2))
        qpool = ctx.enter_context(tc.tile_pool(name="qpool", bufs=2))
        gpool = ctx.enter_context(tc.tile_pool(name="gpool", bufs=2))
        opool = ctx.enter_context(tc.tile_pool(name="opool", bufs=2))
        ps_v = ctx.enter_context(tc.tile_pool(name="ps_v", bufs=2, space="PSUM"))
        ps_c = ctx.enter_context(tc.tile_pool(name="ps_c", bufs=2, space="PSUM"))
        ps_t = ctx.enter_context(tc.tile_pool(name="ps_t", bufs=2, space="PSUM"))

        # ---- constants into SBUF ----
        wcat_sb = [const.tile([128, 40], F32, tag=f"wcat{k}", name=f"wcat{k}")
                   for k in range(2)]
        for k in range(2):
            nc.sync.dma_start(out=wcat_sb[k][:], in_=wcat[k * 128:(k + 1) * 128, :])
        refrhs_sb = const.tile([4, 40], F32, tag="refrhs", name="refrhs")
        nc.sync.dma_start(out=refrhs_sb[:], in_=ref_rhs[:])
        wval_sb = [const.tile([128, 64], VDT, tag=f"wval{k}", name=f"wval{k}")
                   for k in range(2)]
        for k in range(2):
            nc.sync.dma_start(out=wval_sb[k][:], in_=wval[k * 128:(k + 1) * 128, :])
        bval_sb = const.tile([128, 64], F32, tag="bval", name="bval")
        nc.sync.dma_start(out=bval_sb[:], in_=bval[:])
        wout_sb = const.tile([64, C], F32, tag="wout", name="wout")
        nc.sync.dma_start(out=wout_sb[:], in_=wout[:])
        zoff_sb = const.tile([128, W], F32, tag="zoff", name="zoff")
        nc.sync.dma_start(out=zoff_sb[:], in_=zoff[:])
        ident = const.tile([128, 128], F32, tag="ident", name="ident")
        make_identity(nc, ident[:])

        # persistent big buffers
        qs_sb = [const.tile([128, NQ], F32, tag=f"qs{k}", name=f"qs{k}")
                 for k in range(2)]
        ref_sb = const.tile([4, NQ], F32, tag="refq", name="refq")
        st_sb = const.tile([64, NQ], F32, tag="st", name="st")
        maskb_all = const.tile([128, NSUP * MCOL], VDT, tag="maskb", name="maskb")
        idxw_all = const.tile([128, NSUP * ICOL], I16, tag="idxw", name="idxw")

        # ---- stage Q0: load q, pos, ref; qs = q + p ----
        for k in range(2):
            for half in range(4):
                sl = slice(half * (NQ // 4), (half + 1) * (NQ // 4))
                ptmp = qpool.tile([128, NQ // 4], F32, tag="ptmp", name="ptmp")
                nc.sync.dma_start(out=qs_sb[k][:, sl],
                                  in_=qT[k * 128:(k + 1) * 128, sl])
                nc.sync.dma_start(out=ptmp[:], in_=pT[k * 128:(k + 1) * 128, sl])
                vec.tensor_tensor(out=qs_sb[k][:, sl], in0=qs_sb[k][:, sl],
                                  in1=ptmp[:], op=OP.add)
        nc.sync.dma_start(out=ref_sb[:], in_=refT[:])

        # ---- stage C: coords / masks / gather indices, all supertiles ----
        for g in range(NSUP):
            q0 = g * TQ
            psc = ps_c.tile([128, 160], F32, tag="psc", name="psc")
            for s in range(NSUB):
                qsl = slice(q0 + s * 128, q0 + (s + 1) * 128)
                nc.tensor.matmul(psc[:, s * 40:(s + 1) * 40],
                                 qs_sb[0][:, qsl], wcat_sb[0][:],
                                 start=True, stop=False)
                nc.tensor.matmul(psc[:, s * 40:(s + 1) * 40],
                                 qs_sb[1][:, qsl], wcat_sb[1][:],
                                 start=False, stop=False)
                nc.tensor.matmul(psc[:, s * 40:(s + 1) * 40],
                                 ref_sb[:, qsl], refrhs_sb[:],
                                 start=False, stop=True)
            coords = qpool.tile([128, 160], F32, tag="coords", name="coords")
            act.activation(out=coords[:], in_=psc[:], func=ACT.Copy)

            co = coords[:].rearrange("p (s r) -> p s r", s=NSUB)
            pix = co[:, :, 0:30]                        # (s, hp*ax)
            logit = co[:, :, 30:40]                     # (s, hp)

            # softmax over P
            exlog = qpool.tile([128, NSUB * 10], F32, tag="exlog", name="exlog")
            act.activation(out=exlog[:], in_=logit, func=ACT.Exp)
            ex4 = exlog[:].rearrange("p (s h q) -> p s h q", s=NSUB, h=2)
            sums = qpool.tile([128, NSUB * 2], F32, tag="sums", name="sums")
            vec.tensor_reduce(out=sums[:].rearrange("p (s h) -> p s h", s=NSUB),
                              in_=ex4, axis=AX.X, op=OP.add)
            rsum = qpool.tile([128, NSUB * 2], F32, tag="rsum", name="rsum")
            vec.reciprocal(out=rsum[:], in_=sums[:])
            aw = qpool.tile([128, NSUB * 10], F32, tag="aw", name="aw")
            vec.tensor_tensor(
                out=aw[:].rearrange("p (sh q) -> p sh q", q=5),
                in0=exlog[:].rearrange("p (sh q) -> p sh q", q=5),
                in1=rsum[:].unsqueeze(2).to_broadcast([128, NSUB * 2, 5]),
                op=OP.mult)

            # corner math on the 30 pixel rows
            NPX = NSUB * 30
            # flo = round(pix - 0.5) via the 2^23 magic add (== floor except
            # exactly-integer pix, where the phantom corner gets zero weight)
            flo = qpool.tile([128, NPX], F32, tag="flo", name="flo")
            vec.tensor_scalar(out=flo[:].rearrange("p (s r) -> p s r", s=NSUB),
                              in0=pix, scalar1=8388607.5, scalar2=8388608.0,
                              op0=OP.add, op1=OP.subtract)
            fl3 = flo[:].rearrange("p (s r) -> p s r", s=NSUB)
            frac = qpool.tile([128, NPX], F32, tag="frac", name="frac")
            vec.tensor_tensor(out=frac[:].rearrange("p (s r) -> p s r", s=NSUB),
                              in0=pix, in1=fl3, op=OP.subtract)
            fr3 = frac[:].rearrange("p (s r) -> p s r", s=NSUB)
            # gcorn: (s, hp, ax, dx) — clamped corner coords (biased +64)
            gcorn = qpool.tile([128, NPX * 2], F32, tag="gcorn", name="gcorn")
            gc4 = gcorn[:].rearrange("p (s r d) -> p s r d", s=NSUB, d=2)
            vec.tensor_scalar(out=gc4[:, :, :, 0], in0=fl3,
                              scalar1=64.0, scalar2=111.0, op0=OP.max, op1=OP.min)
            g1m = qpool.tile([128, NPX], F32, tag="g1m", name="g1m")
            vec.tensor_scalar(out=g1m[:], in0=flo[:],
                              scalar1=63.0, scalar2=110.0, op0=OP.max, op1=OP.min)
            vec.tensor_scalar(out=gc4[:, :, :, 1], in0=g1m[:]
                              .rearrange("p (s r) -> p s r", s=NSUB),
                              scalar1=1.0, scalar2=None, op0=OP.add)
            # validity via clip-equality
            v0 = qpool.tile([128, NPX], F32, tag="v0", name="v0")
            vec.tensor_tensor(out=v0[:].rearrange("p (s r) -> p s r", s=NSUB),
                              in0=gc4[:, :, :, 0], in1=fl3, op=OP.is_equal)
            v1 = qpool.tile([128, NPX], F32, tag="v1", name="v1")
            vec.tensor_tensor(out=v1[:], in0=g1m[:], in1=flo[:], op=OP.is_equal)
            # corner weights (x/y rows used; z rows ignored later)
            om = qpool.tile([128, NPX], F32, tag="om", name="om")
            vec.tensor_scalar(out=om[:], in0=frac[:], scalar1=-1.0, scalar2=1.0,
                              op0=OP.mult, op1=OP.add)
            wcorn = qpool.tile([128, NPX * 2], F32, tag="wcorn", name="wcorn")
            wc4 = wcorn[:].rearrange("p (s r d) -> p s r d", s=NSUB, d=2)
            vec.tensor_tensor(out=wc4[:, :, :, 0],
                              in0=om[:].rearrange("p (s r) -> p s r", s=NSUB),
                              in1=v0[:].rearrange("p (s r) -> p s r", s=NSUB),
                              op=OP.mult)
            vec.tensor_tensor(out=wc4[:, :, :, 1],
                              in0=fr3,
                              in1=v1[:].rearrange("p (s r) -> p s r", s=NSUB),
                              op=OP.mult)

            # z window: rzq = floor((gz-64)/4) in [0,11]; W-slot hat weights
            gc6 = gcorn[:].rearrange("p (s hp a d) -> p s hp a d",
                                     s=NSUB, hp=10, a=3)
            gz = gc6[:, :, :, 2, 0]
            pz = co[:, :, 0:30].rearrange("p s (hp a) -> p s hp a", a=3)[:, :, :, 2]
            rzq = qpool.tile([128, NSUB * 10], F32, tag="rzq", name="rzq")
            tq = qpool.tile([128, NSUB * 10], F32, tag="tq", name="tq")
            vec.tensor_scalar(out=tq[:].rearrange("p (s h) -> p s h", s=NSUB),
                              in0=gz, scalar1=0.25, scalar2=16.375,
                              op0=OP.mult, op1=OP.subtract)
            vec.tensor_scalar(out=rzq[:], in0=tq[:],
                              scalar1=8388624.0, scalar2=8388624.0,
                              op0=OP.add, op1=OP.subtract)
            # d0 = (4*rzq + 64) - pz ; dk = d0 + k
            zb4 = qpool.tile([128, NSUB * 10], F32, tag="zb4", name="zb4")
            vec.tensor_scalar(out=zb4[:], in0=rzq[:], scalar1=4.0, scalar2=64.0,
                              op0=OP.mult, op1=OP.add)
            d0 = qpool.tile([128, NSUB * 10], F32, tag="d0", name="d0")
            vec.tensor_tensor(out=d0[:].rearrange("p (s h) -> p s h", s=NSUB),
                              in0=zb4[:].rearrange("p (s h) -> p s h", s=NSUB),
                              in1=pz, op=OP.subtract)
            dk = qpool.tile([128, NSUB * 10 * W], F32, tag="dk", name="dk")
            vec.tensor_tensor(
                out=dk[:].rearrange("p (sh k) -> p sh k", k=W),
                in0=d0[:].unsqueeze(2).to_broadcast([128, NSUB * 10, W]),
                in1=zoff_sb[:].unsqueeze(1).to_broadcast([128, NSUB * 10, W]),
                op=OP.add)
            adk = qpool.tile([128, NSUB * 10 * W], F32, tag="adk", name="adk")
            act.activation(out=adk[:], in_=dk[:], func=ACT.Abs)
            hat = qpool.tile([128, NSUB * 10 * W], F32, tag="hat", name="hat")
            act.activation(out=hat[:], in_=adk[:], func=ACT.Relu,
                           scale=-1.0, bias=1.0)
            # upper bound: slot z 4*rzq+64+k <= 111  <=>  dk <= 111 - pz
            ub = qpool.tile([128, NSUB * 10], F32, tag="ub", name="ub")
            vec.tensor_scalar(out=ub[:].rearrange("p (s h) -> p s h", s=NSUB),
                              in0=pz, scalar1=-1.0, scalar2=111.0,
                              op0=OP.mult, op1=OP.add)
            vub = qpool.tile([128, NSUB * 10 * W], F32, tag="vub", name="vub")
            vec.tensor_tensor(
                out=vub[:].rearrange("p (sh k) -> p sh k", k=W),
                in0=dk[:].rearrange("p (sh k) -> p sh k", k=W),
                in1=ub[:].unsqueeze(2).to_broadcast([128, NSUB * 10, W]),
                op=OP.is_le)
            wz = qpool.tile([128, NSUB * 10 * W], F32, tag="wz", name="wz")
            vec.tensor_tensor(out=wz[:], in0=hat[:], in1=vub[:], op=OP.mult)

            # slot weights: block bx = min(gx0, 110); slot s holds x = bx+s.
            # When gx0 == 111 (x >= 47) the corner-0 weight moves to slot 1.
            wc6 = wcorn[:].rearrange("p (s hp a d) -> p s hp a d",
                                     s=NSUB, hp=10, a=3)
            ws = qpool.tile([128, NSUB * 40], F32, tag="ws", name="ws")
            ws4 = ws[:].rearrange("p (s hp a d) -> p s hp a d", s=NSUB, hp=10, a=2)
            hi = qpool.tile([128, NSUB * 20], F32, tag="hi", name="hi")
            hi3 = hi[:].rearrange("p (s hp a) -> p s hp a", s=NSUB, hp=10)
            hit = qpool.tile([128, NSUB * 20], F32, tag="hit", name="hit")
            hit3 = hit[:].rearrange("p (s hp a) -> p s hp a", s=NSUB, hp=10)
            # hi = (gc0 == 111): corner-0 weight moves to slot 1.
            # lo = (g1m == 63): corner-1 weight (position 64) moves to slot 0.
            vec.tensor_scalar(out=hi3, in0=gc6[:, :, :, 0:2, 0],
                              scalar1=111.0, scalar2=None, op0=OP.is_equal)
            vec.tensor_tensor(out=hit3, in0=wc6[:, :, :, 0:2, 0], in1=hi3,
                              op=OP.mult)
            lo = qpool.tile([128, NSUB * 20], F32, tag="lo", name="lo")
            lo3 = lo[:].rearrange("p (s hp a) -> p s hp a", s=NSUB, hp=10)
            lot = qpool.tile([128, NSUB * 20], F32, tag="lot", name="lot")
            lot3 = lot[:].rearrange("p (s hp a) -> p s hp a", s=NSUB, hp=10)
            g1m3 = g1m[:].rearrange("p (s hp a) -> p s hp a", s=NSUB, hp=10)
            vec.tensor_scalar(out=lo3, in0=g1m3[:, :, :, 0:2],
                              scalar1=63.0, scalar2=None, op0=OP.is_equal)
            vec.tensor_tensor(out=lot3, in0=wc6[:, :, :, 0:2, 1], in1=lo3,
                              op=OP.mult)
            vec.tensor_tensor(out=ws4[:, :, :, :, 0],
                              in0=wc6[:, :, :, 0:2, 0], in1=hit3,
                              op=OP.subtract)
            vec.tensor_tensor(out=ws4[:, :, :, :, 0],
                              in0=ws4[:, :, :, :, 0], in1=lot3, op=OP.add)
            vec.tensor_tensor(out=ws4[:, :, :, :, 1],
                              in0=wc6[:, :, :, 0:2, 1], in1=hit3, op=OP.add)
            vec.tensor_tensor(out=ws4[:, :, :, :, 1],
                              in0=ws4[:, :, :, :, 1], in1=lot3, op=OP.subtract)
            wxs = ws4[:, :, :, 0, :]                    # (s, hp, xslot)
            wys = ws4[:, :, :, 1, :]                    # (s, hp, yslot)

            # mask: m[(s hp), zb, xp, yp, z4] = aw*wxs*wys*wz
            m1 = qpool.tile([128, NSUB * 20], F32, tag="m1", name="m1")
            vec.tensor_tensor(
                out=m1[:].rearrange("p (sh xp) -> p sh xp", xp=2),
                in0=aw[:].unsqueeze(2).to_broadcast([128, NSUB * 10, 2]),
                in1=wxs.rearrange("p s hp xp -> p (s hp) xp"), op=OP.mult)
            m2 = qpool.tile([128, NSUB * 40], F32, tag="m2", name="m2")
            vec.tensor_tensor(
                out=m2[:].rearrange("p (sh xp yp) -> p sh xp yp", xp=2, yp=2),
                in0=m1[:].rearrange("p (sh xp) -> p sh xp", xp=2)
                    .unsqueeze(3).to_broadcast([128, NSUB * 10, 2, 2]),
                in1=wys.rearrange("p s hp yp -> p (s hp) yp")
                    .unsqueeze(2).to_broadcast([128, NSUB * 10, 2, 2]),
                op=OP.mult)
            # mtmp layout (sh, zb, z4, xy); all three APs are 3-free-dim
            mtmp = qpool.tile([128, NSUB * 10 * 32], F32, tag="mtmp",
                              name="mtmp")
            vec.tensor_tensor(
                out=mtmp[:].rearrange("p (sh zz xy) -> p sh zz xy",
                                      zz=8, xy=4),
                in0=m2[:].rearrange("p (sh xy) -> p sh xy", xy=4)
                    .unsqueeze(2).to_broadcast([128, NSUB * 10, 8, 4]),
                in1=wz[:].rearrange("p (sh zz) -> p sh zz", zz=8)
                    .unsqueeze(3).to_broadcast([128, NSUB * 10, 8, 4]),
                op=OP.mult)
            # permute (zb, z4, xy) -> (zb, xy, z4); (sh, zb) merges both sides
            mv = maskb_all[:, g * MCOL:(g + 1) * MCOL]
            vec.tensor_copy(
                out=mv.rearrange("p (szb xy z4) -> p szb xy z4", xy=4, z4=4),
                in_=mtmp[:].rearrange("p (szb z4 xy) -> p szb xy z4",
                                      z4=4, xy=4))

            # gather row indices: 576*(bx-64) + 12*(by-64) + rzq
            bx = qpool.tile([128, NSUB * 10], F32, tag="bx", name="bx")
            vec.tensor_scalar(out=bx[:].rearrange("p (s h) -> p s h", s=NSUB),
                              in0=gc6[:, :, :, 0, 0], scalar1=110.0,
                              scalar2=576.0, op0=OP.min, op1=OP.mult)
            by = qpool.tile([128, NSUB * 10], F32, tag="by", name="by")
            vec.tensor_scalar(out=by[:].rearrange("p (s h) -> p s h", s=NSUB),
                              in0=gc6[:, :, :, 1, 0], scalar1=110.0,
                              scalar2=12.0, op0=OP.min, op1=OP.mult)
            t4 = qpool.tile([128, NSUB * 10], F32, tag="t4", name="t4")
            vec.tensor_tensor(out=t4[:], in0=bx[:], in1=by[:], op=OP.add)
            idxf = qpool.tile([128, NSUB * 10], F32, tag="idxf", name="idxf")
            vec.tensor_scalar(out=idxf[:], in0=t4[:],
                              scalar1=37632.0, scalar2=None, op0=OP.subtract)
            vec.tensor_tensor(out=idxf[:], in0=idxf[:], in1=rzq[:], op=OP.add)
            idx16 = qpool.tile([128, NSUB * 10], I16, tag="idx16", name="idx16")
            vec.tensor_copy(out=idx16[:], in_=idxf[:])

            # idx fold into the gather's 16-partition-wrapped layout:
            # bounce through DRAM with contiguous 320B packets, then a DVE
            # column permute (s2-major -> s2-interleaved).
            nc.sync.dma_start(out=idxscr[g * 128:(g + 1) * 128, :], in_=idx16[:])
            idxr = gpool.tile([128, ICOL], I16, tag="idxr", name="idxr")
            scr = idxscr[g * 128:(g + 1) * 128, :]
            for gg in range(8):
                # idxr[16*gg + p16, s2*40 + f] = idx16[s2*16 + p16, f]
                src = bass.AP(scr.tensor, scr.offset,
                              [[40, 16], [16 * 40, 8], [1, 40]])
                nc.sync.dma_start(
                    out=idxr[gg * 16:(gg + 1) * 16, :]
                        .rearrange("p (s2 f) -> p s2 f", s2=8),
                    in_=src)
            # permute cols: (s2, subhl, r) -> (subhl, r, s2)
            vec.tensor_copy(
                out=idxw_all[:, g * ICOL:(g + 1) * ICOL]
                    .rearrange("p (sh r s2) -> p sh r s2", sh=8, r=5),
                in_=idxr[:].rearrange("p (s2 sh r) -> p sh r s2", s2=8, sh=8))

        # ---- stage V: value projection (bf16) ----
        # Zero the 12 units at block (46, 47) whose (xp=1, yp=1) slots no
        # write pass covers but the z-window overrun can read, plus the back
        # guard.
        zpad = const.tile([16, 512], VDT, tag="zpad", name="zpad")
        vec.memset(zpad[:], 0.0)
        for hl in range(2):
            zb46 = (46 * 576 + 47 * 12)
            nc.sync.dma_start(
                out=bass.AP(vexp[:].tensor,
                            hl * VHEAD + (G0 + zb46) * 512 + 256 + 128,
                            [[512, 12], [1, 128]]),
                in_=zpad[0:12, 0:128])
            nc.sync.dma_start(
                out=bass.AP(vexp[:].tensor, hl * VHEAD + (G0 + NUNIT) * 512,
                            [[512, 16], [1, 512]]),
                in_=zpad[:])
        for vg in range(NVSUP // NR):          # flush groups of NR supertiles
            vb = [vpool.tile([128, NR * 256], VDT, tag=f"vb{hl}", name=f"vb{hl}")
                  for hl in range(2)]
            for i in range(NR):
                vt = vg * NR + i
                vin = [vpool.tile([128, VSUP], VDT, tag=f"vin{k}", name=f"vin{k}")
                       for k in range(2)]
                for k in range(2):
                    nc.sync.dma_start(
                        out=vin[k][:],
                        in_=value_in[k * 128:(k + 1) * 128,
                                     vt * VSUP:(vt + 1) * VSUP])
                psv = ps_v.tile([128, 512], F32, tag="psv", name="psv")
                for s in range(8):
                    lhs0 = vin[0][:].rearrange("p (v e) -> p e v", e=8)[:, s, :]
                    lhs1 = vin[1][:].rearrange("p (v e) -> p e v", e=8)[:, s, :]
                    nc.tensor.matmul(psv[:, s * 64:(s + 1) * 64], lhs0,
                                     wval_sb[0][:], start=True, stop=False)
                    nc.tensor.matmul(psv[:, s * 64:(s + 1) * 64], lhs1,
                                     wval_sb[1][:], start=False, stop=True)
                # split heads, add bias, pack (blk, c, v4) per 4-voxel block
                for hl in range(2):
                    src = psv[:].rearrange("p (blk v4 hc) -> p blk hc v4",
                                           blk=2, v4=4)[:, :, hl * 32:(hl + 1) * 32, :]
                    bv = bval_sb[:, hl * 32:(hl + 1) * 32] \
                        .unsqueeze(1).unsqueeze(3).to_broadcast([128, 2, 32, 4])
                    vec.tensor_tensor(
                        out=vb[hl][:, i * 256:(i + 1) * 256]
                            .rearrange("p (blk c v4) -> p blk c v4", blk=2, c=32),
                        in0=src, in1=bv, op=OP.add)
            # flush NR supertiles (NR*1024 voxels) per head: 4 quadrant
            # passes; pass (xp, yp) lands vblock at unit vblock-xp*576-yp*12
            # slot (xp, yp). Runs are 256B ((c, z4) per vblock).
            for hl in range(2):
                for xp in range(2):
                    for yp in range(2):
                        base = (hl * VHEAD
                                + (G0 + vg * NR * 256 - xp * 576 - yp * 12)
                                * 512 + xp * 256 + yp * 128)
                        for blk in range(2):
                            dst = bass.AP(vexp[:].tensor, base + blk * 512,
                                          [[1024, 128], [256 * 512, NR],
                                           [1, 128]])
                            src = vb[hl][:] \
                                .rearrange("p (i blk x) -> p i blk x",
                                           i=NR, blk=2)[:, :, blk, :]
                            eng = nc.sync if (xp * 2 + yp) % 2 == 0 \
                                else nc.scalar
                            eng.dma_start(out=dst, in_=src)

        # ---- stage G: gather + weighted reduce per (supertile, subtile, head) ----
        for g in range(NSUP):
            q0 = g * TQ
            S = qpool.tile([128, NSUB * 64], F32, tag="S", name="S")
            for s in range(NSUB):
                for hl in range(2):
                    G = gpool.tile([128, 5 * 1024], VDT, tag="G", name="G")
                    in_g = bass.AP(vexp[:].tensor, hl * VHEAD + G0 * 512,
                                   [[512, NUNIT], [1, 1024]])
                    nc.gpsimd.dma_gather(
                        out_ap=G[:].rearrange("p (i e) -> p i e", i=5),
                        in_ap=in_g,
                        idxs_ap=idxw_all[:, (g * 8 + s * 2 + hl) * 40:
                                         (g * 8 + s * 2 + hl + 1) * 40],
                        num_idxs=NIDX, num_idxs_reg=NIDX,
                        elem_size=1024, elem_step=512,
                        single_packet=False,
                        queue_num=(s * 2 + hl) % 4)
                    # P = G * mask in place; G row = (zb, xp, yp, c, z4),
                    # mask cols (pt, zb, xy, z4) bcast over c
                    moff = g * MCOL + (s * 2 + hl) * 160
                    mg = maskb_all[:, moff:moff + 160] \
                        .rearrange("p (rb z4) -> p rb z4", z4=4) \
                        .unsqueeze(2).to_broadcast([128, 40, 32, 4])
                    gv = G[:].rearrange("p (rb c z4) -> p rb c z4",
                                        rb=40, c=32)
                    vec.tensor_tensor(out=gv, in0=gv, in1=mg, op=OP.mult)
                    # single fused reduce over (rb, z4), keeping c
                    vec.tensor_reduce(
                        out=S[:, s * 64 + hl * 32:s * 64 + hl * 32 + 32],
                        in_=G[:].rearrange("p (rb c z4) -> p c rb z4",
                                           rb=40, c=32),
                        axis=AX.XY, op=OP.add)

            # transpose S [128, 64] -> [64, 128] per subtile
            for s in range(NSUB):
                pst = ps_t.tile([64, 128], F32, tag="pst", name="pst")
                nc.tensor.transpose(pst[:], S[:, s * 64:(s + 1) * 64], ident[:])
                act.activation(out=st_sb[:, q0 + s * 128:q0 + (s + 1) * 128],
                               in_=pst[:], func=ACT.Copy)

        # ---- GEMM2: outT = wout^T @ ST (float32r moving, N=512) ----
        for mc in range(2):
            for ntile in range(NQ // 512):
                ps2 = ps_c.tile([128, 512], F32, tag="ps2", name="ps2")
                nc.tensor.matmul(ps2[:],
                                 wout_sb[:, mc * 128:(mc + 1) * 128],
                                 st_sb[:, ntile * 512:(ntile + 1) * 512],
                                 start=True, stop=True)
                ob = opool.tile([128, 512], F32, tag="ob", name="ob")
                vec.tensor_copy(out=ob[:], in_=ps2[:])
                nc.sync.dma_start(
                    out=outp[mc * 128:(mc + 1) * 128,
                             ntile * 512:(ntile + 1) * 512],
                    in_=ob[:])

    nc.compile()
    return nc


def _prep_core_inputs(inputs, b, j):
    import ml_dtypes
    q = np.ascontiguousarray(inputs["query"][b].T, np.float32)
    p = np.ascontiguousarray(inputs["pos"][b].T, np.float32)
    r = np.concatenate([inputs["reference_points"][b].T,
                        np.ones((1, NQ), np.float32)]).astype(np.float32)
    r = np.ascontiguousarray(r)
    value = np.ascontiguousarray(
        inputs["value"][b].reshape(C, NVOX)).astype(ml_dtypes.bfloat16)

    W_off, b_off = inputs["W_off"], inputs["b_off"]
    W_attn, b_attn = inputs["W_attn"], inputs["b_attn"]
    heads = [2 * j, 2 * j + 1]
    rows, biases, refr = [], [], []
    for h in heads:
        for pp in range(P):
            for ax in range(3):
                rows.append(W_off[(h * P + pp) * 3 + ax])
                biases.append(b_off[(h * P + pp) * 3 + ax] - 0.5 + 64.0)
                e = np.zeros(3, np.float32)
                e[ax] = GRID
                refr.append(e)
    for h in heads:
        for pp in range(P):
            rows.append(W_attn[h * P + pp])
            biases.append(b_attn[h * P + pp])
            refr.append(np.zeros(3, np.float32))
    wcat = np.ascontiguousarray(np.stack(rows).T, np.float32)       # (256, 40)
    ref_rhs = np.concatenate(
        [np.stack(refr).T, np.asarray(biases, np.float32)[None, :]])
    ref_rhs = np.ascontiguousarray(ref_rhs, np.float32)             # (4, 40)

    wval = np.ascontiguousarray(
        inputs["W_val"][64 * j:64 * j + 64].T).astype(ml_dtypes.bfloat16)
    bval = np.ascontiguousarray(
        np.repeat(inputs["b_val"][64 * j:64 * j + 64][None, :], 128, axis=0),
        np.float32)
    wout = np.ascontiguousarray(inputs["W_out"][:, 64 * j:64 * j + 64].T,
                                np.float32)
    zoffs = np.repeat(np.arange(W, dtype=np.float32)[None, :], 128, axis=0)
    return {
        "value_in": value, "qT": q, "pT": p, "refT": r,
        "wcat": wcat, "ref_rhs": ref_rhs,
        "wval": wval, "bval": bval, "wout": wout, "zoff": zoffs,
    }


def get_nc():
    global _NC_CACHE
    if _NC_CACHE is None:
        _NC_CACHE = build_nc()
    return _NC_CACHE


def kernel(**inputs):
    from concourse.bass_utils import run_bass_kernel_spmd

    inputs = {k: np.asarray(v) for k, v in inputs.items()}
    nc = get_nc()
    in_maps = [_prep_core_inputs(inputs, core // 4, core % 4) for core in range(8)]
    res = run_bass_kernel_spmd(nc, in_maps, list(range(8)))
    bs = inputs["query"].shape[0]
    out = np.zeros((bs, NQ, C), np.float32)
    for core in range(8):
        out[core // 4] += res.results[core]["outp"].T
    out += inputs["b_out"][None, None, :].astype(np.float32)
    return out


# revision 28
# speedup vs baseline: 3.3592x; 1.0776x over previous
"""Trainium2 Bass kernel for 3D deformable attention (8 NeuronCores).

Sharding: core i handles batch b = i // 4 and head-pair j = i % 4
(heads 2j, 2j+1, i.e. value/out channels [64j, 64j+64)).

Per-core device pipeline (emission order = coords first so DVE mask work
overlaps the value-projection DMA/PE stage):
  C. per query-supertile (512 q): coords = qs^T @ Wcat^T + [48*ref | b] (PE);
     softmax over 5 points, clamped trilinear corner weights, z-window "hat"
     weights, combined mask m = aw*wx*wy*wz (DVE, bf16); int16 gather row
     indices; idx fold into dma_gather's 16-partition-wrapped layout via a
     contiguous DRAM bounce (320B packets) + DVE column permute.
  V. value projection v = W_val[64j:64j+64] @ value[b] in bf16 (PE,
     voxel-stationary), + b_val; packed per 4-voxel block as (c, v4) and
     flushed to DRAM as two head blocks of [NVOX+8 vox, 32ch] rows.
  G. per (query-subtile, head): one dma_gather of 2560 rows (8 vox x 32ch
     bf16 = 512B each); P = G * mask (DVE bf16, contiguous); two-step
     reduce (over v4 contiguous, then over (pt,xy,blk)) -> S[q, 64].
  O. PE transpose of S, then out^T = Wout^T @ S^T (float32r), DMA out.
Host combines: out[b] = sum_j outp_j^T + b_out.
"""
import numpy as np

import concourse.bass as bass
import concourse.mybir as mybir
from concourse import bacc, tile
from concourse.masks import make_identity
from contextlib import ExitStack

F32 = mybir.dt.float32
F32R = mybir.dt.float32r
I16 = mybir.dt.int16
AX = mybir.AxisListType
OP = mybir.AluOpType
ACT = mybir.ActivationFunctionType

H, P = 8, 5
NQ, C, GRID = 4096, 256, 48
NVOX = GRID ** 3            # 110592
NSUB = 4                    # query subtiles (of 128) per supertile
TQ = 128 * NSUB             # 512
NSUP = NQ // TQ             # 8
VSUP = 1024                 # voxels per value-proj supertile
NVSUP = NVOX // VSUP        # 108
NR = 4                      # value supertiles per DRAM flush

VDT = mybir.dt.bfloat16
W = 8                       # z-window voxels per gathered row
# vexp: quad-interleaved expanded volume. Per head, blocks (x0, y0) of
# 12 z-units; unit = (xp, yp, c, z4) = 512 els (4-z slab x 4 quadrants x
# 32ch). A gather row = 2 consecutive units = the full 2x2x(8z) trilinear
# neighborhood of one sample point. unit(vblock) = vblock - xp*576 - yp*12
# is linear in vblock, so the 4 write passes keep 256B-contiguous runs.
NUNIT = 48 * 48 * 12        # 27648 addressable units per head
G0 = 588                    # front guard units (absorbs xp/yp shifts)
G1 = 16                     # back guard units (zeroed; z-window overrun)
VHEAD = (G0 + NUNIT + G1) * 512
NIDX = 5 * 128              # rows per (subtile, head) gather
MCOL = NSUB * 40 * W        # mask columns per supertile (1280)
ICOL = NSUB * 2 * 40        # idx columns per supertile (320)

_NC_CACHE = None


def build_nc():
    nc = bacc.Bacc("TRN2", target_bir_lowering=False, debug=False, num_devices=8,
                   num_swdge_queues=4)

    value_in = nc.dram_tensor("value_in", [C, NVOX], VDT, kind="ExternalInput")
    qT = nc.dram_tensor("qT", [C, NQ], F32, kind="ExternalInput")
    pT = nc.dram_tensor("pT", [C, NQ], F32, kind="ExternalInput")
    refT = nc.dram_tensor("refT", [4, NQ], F32, kind="ExternalInput")
    wcat = nc.dram_tensor("wcat", [C, 40], F32, kind="ExternalInput")
    ref_rhs = nc.dram_tensor("ref_rhs", [4, 40], F32, kind="ExternalInput")
    wval = nc.dram_tensor("wval", [C, 64], VDT, kind="ExternalInput")
    bval = nc.dram_tensor("bval", [128, 64], F32, kind="ExternalInput")
    wout = nc.dram_tensor("wout", [64, C], F32, kind="ExternalInput")
    zoff = nc.dram_tensor("zoff", [128, W], F32, kind="ExternalInput")
    outp = nc.dram_tensor("outp", [C, NQ], F32, kind="ExternalOutput")
    vexp = nc.dram_tensor("vexp", [2 * VHEAD], VDT)
    idxscr = nc.dram_tensor("idxscr", [NSUP * 128, 40], I16)

    vec = nc.vector
    act = nc.scalar

    with tile.TileContext(nc) as tc, ExitStack() as ctx:
        const = ctx.enter_context(tc.tile_pool(name="const", bufs=1))
        vpool = ctx.enter_context(tc.tile_pool(name="vpool", bufs=2))
        qpool = ctx.enter_context(tc.tile_pool(name="qpool", bufs=2))
        gpool = ctx.enter_context(tc.tile_pool(name="gpool", bufs=3))
        opool = ctx.enter_context(tc.tile_pool(name="opool", bufs=2))
        ps_v = ctx.enter_context(tc.tile_pool(name="ps_v", bufs=2, space="PSUM"))
        ps_c = ctx.enter_context(tc.tile_pool(name="ps_c", bufs=2, space="PSUM"))
        ps_t = ctx.enter_context(tc.tile_pool(name="ps_t", bufs=2, space="PSUM"))

        # ---- constants into SBUF ----
        wcat_sb = [const.tile([128, 40], F32, tag=f"wcat{k}", name=f"wcat{k}")
                   for k in range(2)]
        for k in range(2):
            nc.sync.dma_start(out=wcat_sb[k][:], in_=wcat[k * 128:(k + 1) * 128, :])
        refrhs_sb = const.tile([4, 40], F32, tag="refrhs", name="refrhs")
        nc.sync.dma_start(out=refrhs_sb[:], in_=ref_rhs[:])
        wval_sb = [const.tile([128, 64], VDT, tag=f"wval{k}", name=f"wval{k}")
                   for k in range(2)]
        for k in range(2):
            nc.sync.dma_start(out=wval_sb[k][:], in_=wval[k * 128:(k + 1) * 128, :])
        bval_sb = const.tile([128, 64], F32, tag="bval", name="bval")
        nc.sync.dma_start(out=bval_sb[:], in_=bval[:])
        wout_sb = const.tile([64, C], F32, tag="wout", name="wout")
        nc.sync.dma_start(out=wout_sb[:], in_=wout[:])
        zoff_sb = const.tile([128, W], F32, tag="zoff", name="zoff")
        nc.sync.dma_start(out=zoff_sb[:], in_=zoff[:])
        ident = const.tile([128, 128], F32, tag="ident", name="ident")
        make_identity(nc, ident[:])

        # persistent big buffers
        qs_sb = [const.tile([128, NQ], F32, tag=f"qs{k}", name=f"qs{k}")
                 for k in range(2)]
        ref_sb = const.tile([4, NQ], F32, tag="refq", name="refq")
        st_sb = const.tile([64, NQ], F32, tag="st", name="st")
        maskb_all = const.tile([128, NSUP * MCOL], VDT, tag="maskb", name="maskb")
        idxw_all = const.tile([128, NSUP * ICOL], I16, tag="idxw", name="idxw")

        # ---- stage Q0: load q, pos, ref; qs = q + p ----
        for k in range(2):
            for half in range(4):
                sl = slice(half * (NQ // 4), (half + 1) * (NQ // 4))
                ptmp = qpool.tile([128, NQ // 4], F32, tag="ptmp", name="ptmp")
                nc.sync.dma_start(out=qs_sb[k][:, sl],
                                  in_=qT[k * 128:(k + 1) * 128, sl])
                nc.sync.dma_start(out=ptmp[:], in_=pT[k * 128:(k + 1) * 128, sl])
                vec.tensor_tensor(out=qs_sb[k][:, sl], in0=qs_sb[k][:, sl],
                                  in1=ptmp[:], op=OP.add)
        nc.sync.dma_start(out=ref_sb[:], in_=refT[:])

        # ---- stage C: coords / masks / gather indices, all supertiles ----
        for g in range(NSUP):
            q0 = g * TQ
            psc = ps_c.tile([128, 160], F32, tag="psc", name="psc")
            for s in range(NSUB):
                qsl = slice(q0 + s * 128, q0 + (s + 1) * 128)
                nc.tensor.matmul(psc[:, s * 40:(s + 1) * 40],
                                 qs_sb[0][:, qsl], wcat_sb[0][:],
                                 start=True, stop=False)
                nc.tensor.matmul(psc[:, s * 40:(s + 1) * 40],
                                 qs_sb[1][:, qsl], wcat_sb[1][:],
                                 start=False, stop=False)
                nc.tensor.matmul(psc[:, s * 40:(s + 1) * 40],
                                 ref_sb[:, qsl], refrhs_sb[:],
                                 start=False, stop=True)
            coords = qpool.tile([128, 160], F32, tag="coords", name="coords")
            act.activation(out=coords[:], in_=psc[:], func=ACT.Copy)

            co = coords[:].rearrange("p (s r) -> p s r", s=NSUB)
            pix = co[:, :, 0:30]                        # (s, hp*ax)
            logit = co[:, :, 30:40]                     # (s, hp)

            # softmax over P
            exlog = qpool.tile([128, NSUB * 10], F32, tag="exlog", name="exlog")
            act.activation(out=exlog[:], in_=logit, func=ACT.Exp)
            ex4 = exlog[:].rearrange("p (s h q) -> p s h q", s=NSUB, h=2)
            sums = qpool.tile([128, NSUB * 2], F32, tag="sums", name="sums")
            vec.tensor_reduce(out=sums[:].rearrange("p (s h) -> p s h", s=NSUB),
                              in_=ex4, axis=AX.X, op=OP.add)
            rsum = qpool.tile([128, NSUB * 2], F32, tag="rsum", name="rsum")
            vec.reciprocal(out=rsum[:], in_=sums[:])
            aw = qpool.tile([128, NSUB * 10], F32, tag="aw", name="aw")
            vec.tensor_tensor(
                out=aw[:].rearrange("p (sh q) -> p sh q", q=5),
                in0=exlog[:].rearrange("p (sh q) -> p sh q", q=5),
                in1=rsum[:].unsqueeze(2).to_broadcast([128, NSUB * 2, 5]),
                op=OP.mult)

            # corner math on the 30 pixel rows
            NPX = NSUB * 30
            # flo = round(pix - 0.5) via the 2^23 magic add (== floor except
            # exactly-integer pix, where the phantom corner gets zero weight)
            flo = qpool.tile([128, NPX], F32, tag="flo", name="flo")
            vec.tensor_scalar(out=flo[:].rearrange("p (s r) -> p s r", s=NSUB),
                              in0=pix, scalar1=8388607.5, scalar2=8388608.0,
                              op0=OP.add, op1=OP.subtract)
            fl3 = flo[:].rearrange("p (s r) -> p s r", s=NSUB)
            frac = qpool.tile([128, NPX], F32, tag="frac", name="frac")
            vec.tensor_tensor(out=frac[:].rearrange("p (s r) -> p s r", s=NSUB),
                              in0=pix, in1=fl3, op=OP.subtract)
            fr3 = frac[:].rearrange("p (s r) -> p s r", s=NSUB)
            # gcorn: (s, hp, ax, dx) — clamped corner coords (biased +64)
            gcorn = qpool.tile([128, NPX * 2], F32, tag="gcorn", name="gcorn")
            gc4 = gcorn[:].rearrange("p (s r d) -> p s r d", s=NSUB, d=2)
            vec.tensor_scalar(out=gc4[:, :, :, 0], in0=fl3,
                              scalar1=64.0, scalar2=111.0, op0=OP.max, op1=OP.min)
            g1m = qpool.tile([128, NPX], F32, tag="g1m", name="g1m")
            vec.tensor_scalar(out=g1m[:], in0=flo[:],
                              scalar1=63.0, scalar2=110.0, op0=OP.max, op1=OP.min)
            vec.tensor_scalar(out=gc4[:, :, :, 1], in0=g1m[:]
                              .rearrange("p (s r) -> p s r", s=NSUB),
                              scalar1=1.0, scalar2=None, op0=OP.add)
            # validity via clip-equality
            v0 = qpool.tile([128, NPX], F32, tag="v0", name="v0")
            vec.tensor_tensor(out=v0[:].rearrange("p (s r) -> p s r", s=NSUB),
                              in0=gc4[:, :, :, 0], in1=fl3, op=OP.is_equal)
            v1 = qpool.tile([128, NPX], F32, tag="v1", name="v1")
            vec.tensor_tensor(out=v1[:], in0=g1m[:], in1=flo[:], op=OP.is_equal)
            # corner weights (x/y rows used; z rows ignored later)
            om = qpool.tile([128, NPX], F32, tag="om", name="om")
            vec.tensor_scalar(out=om[:], in0=frac[:], scalar1=-1.0, scalar2=1.0,
                              op0=OP.mult, op1=OP.add)
            wcorn = qpool.tile([128, NPX * 2], F32, tag="wcorn", name="wcorn")
            wc4 = wcorn[:].rearrange("p (s r d) -> p s r d", s=NSUB, d=2)
            vec.tensor_tensor(out=wc4[:, :, :, 0],
                              in0=om[:].rearrange("p (s r) -> p s r", s=NSUB),
                              in1=v0[:].rearrange("p (s r) -> p s r", s=NSUB),
                              op=OP.mult)
            vec.tensor_tensor(out=wc4[:, :, :, 1],
                              in0=fr3,
                              in1=v1[:].rearrange("p (s r) -> p s r", s=NSUB),
                              op=OP.mult)

            # z window: rzq = floor((gz-64)/4) in [0,11]; W-slot hat weights
            gc6 = gcorn[:].rearrange("p (s hp a d) -> p s hp a d",
                                     s=NSUB, hp=10, a=3)
            gz = gc6[:, :, :, 2, 0]
            pz = co[:, :, 0:30].rearrange("p s (hp a) -> p s hp a", a=3)[:, :, :, 2]
            rzq = qpool.tile([128, NSUB * 10], F32, tag="rzq", name="rzq")
            tq = qpool.tile([128, NSUB * 10], F32, tag="tq", name="tq")
            vec.tensor_scalar(out=tq[:].rearrange("p (s h) -> p s h", s=NSUB),
                              in0=gz, scalar1=0.25, scalar2=16.375,
                              op0=OP.mult, op1=OP.subtract)
            vec.tensor_scalar(out=rzq[:], in0=tq[:],
                              scalar1=8388624.0, scalar2=8388624.0,
                              op0=OP.add, op1=OP.subtract)
            # d0 = (4*rzq + 64) - pz ; dk = d0 + k
            zb4 = qpool.tile([128, NSUB * 10], F32, tag="zb4", name="zb4")
            vec.tensor_scalar(out=zb4[:], in0=rzq[:], scalar1=4.0, scalar2=64.0,
                              op0=OP.mult, op1=OP.add)
            d0 = qpool.tile([128, NSUB * 10], F32, tag="d0", name="d0")
            vec.tensor_tensor(out=d0[:].rearrange("p (s h) -> p s h", s=NSUB),
                              in0=zb4[:].rearrange("p (s h) -> p s h", s=NSUB),
                              in1=pz, op=OP.subtract)
            dk = qpool.tile([128, NSUB * 10 * W], F32, tag="dk", name="dk")
            vec.tensor_tensor(
                out=dk[:].rearrange("p (sh k) -> p sh k", k=W),
                in0=d0[:].unsqueeze(2).to_broadcast([128, NSUB * 10, W]),
                in1=zoff_sb[:].unsqueeze(1).to_broadcast([128, NSUB * 10, W]),
                op=OP.add)
            adk = qpool.tile([128, NSUB * 10 * W], F32, tag="adk", name="adk")
            act.activation(out=adk[:], in_=dk[:], func=ACT.Abs)
            hat = qpool.tile([128, NSUB * 10 * W], F32, tag="hat", name="hat")
            act.activation(out=hat[:], in_=adk[:], func=ACT.Relu,
                           scale=-1.0, bias=1.0)
            # upper bound: slot z 4*rzq+64+k <= 111  <=>  dk <= 111 - pz
            ub = qpool.tile([128, NSUB * 10], F32, tag="ub", name="ub")
            vec.tensor_scalar(out=ub[:].rearrange("p (s h) -> p s h", s=NSUB),
                              in0=pz, scalar1=-1.0, scalar2=111.0,
                              op0=OP.mult, op1=OP.add)
            vub = qpool.tile([128, NSUB * 10 * W], F32, tag="vub", name="vub")
            vec.tensor_tensor(
                out=vub[:].rearrange("p (sh k) -> p sh k", k=W),
                in0=dk[:].rearrange("p (sh k) -> p sh k", k=W),
                in1=ub[:].unsqueeze(2).to_broadcast([128, NSUB * 10, W]),
                op=OP.is_le)
            wz = qpool.tile([128, NSUB * 10 * W], F32, tag="wz", name="wz")
            vec.tensor_tensor(out=wz[:], in0=hat[:], in1=vub[:], op=OP.mult)

            # slot weights: block bx = min(gx0, 110); slot s holds x = bx+s.
            # When gx0 == 111 (x >= 47) the corner-0 weight moves to slot 1.
            wc6 = wcorn[:].rearrange("p (s hp a d) -> p s hp a d",
                                     s=NSUB, hp=10, a=3)
            ws = qpool.tile([128, NSUB * 40], F32, tag="ws", name="ws")
            ws4 = ws[:].rearrange("p (s hp a d) -> p s hp a d", s=NSUB, hp=10, a=2)
            hi = qpool.tile([128, NSUB * 20], F32, tag="hi", name="hi")
            hi3 = hi[:].rearrange("p (s hp a) -> p s hp a", s=NSUB, hp=10)
            hit = qpool.tile([128, NSUB * 20], F32, tag="hit", name="hit")
            hit3 = hit[:].rearrange("p (s hp a) -> p s hp a", s=NSUB, hp=10)
            # hi = (gc0 == 111): corner-0 weight moves to slot 1.
            # lo = (g1m == 63): corner-1 weight (position 64) moves to slot 0.
            vec.tensor_scalar(out=hi3, in0=gc6[:, :, :, 0:2, 0],
                              scalar1=111.0, scalar2=None, op0=OP.is_equal)
            vec.tensor_tensor(out=hit3, in0=wc6[:, :, :, 0:2, 0], in1=hi3,
                              op=OP.mult)
            lo = qpool.tile([128, NSUB * 20], F32, tag="lo", name="lo")
            lo3 = lo[:].rearrange("p (s hp a) -> p s hp a", s=NSUB, hp=10)
            lot = qpool.tile([128, NSUB * 20], F32, tag="lot", name="lot")
            lot3 = lot[:].rearrange("p (s hp a) -> p s hp a", s=NSUB, hp=10)
            g1m3 = g1m[:].rearrange("p (s hp a) -> p s hp a", s=NSUB, hp=10)
            vec.tensor_scalar(out=lo3, in0=g1m3[:, :, :, 0:2],
                              scalar1=63.0, scalar2=None, op0=OP.is_equal)
            vec.tensor_tensor(out=lot3, in0=wc6[:, :, :, 0:2, 1], in1=lo3,
                              op=OP.mult)
            vec.tensor_tensor(out=ws4[:, :, :, :, 0],
                              in0=wc6[:, :, :, 0:2, 0], in1=hit3,
                              op=OP.subtract)
            vec.tensor_tensor(out=ws4[:, :, :, :, 0],
                              in0=ws4[:, :, :, :, 0], in1=lot3, op=OP.add)
            vec.tensor_tensor(out=ws4[:, :, :, :, 1],
                              in0=wc6[:, :, :, 0:2, 1], in1=hit3, op=OP.add)
            vec.tensor_tensor(out=ws4[:, :, :, :, 1],
                              in0=ws4[:, :, :, :, 1], in1=lot3, op=OP.subtract)
            wxs = ws4[:, :, :, 0, :]                    # (s, hp, xslot)
            wys = ws4[:, :, :, 1, :]                    # (s, hp, yslot)

            # mask: m[(s hp), zb, xp, yp, z4] = aw*wxs*wys*wz
            m1 = qpool.tile([128, NSUB * 20], F32, tag="m1", name="m1")
            vec.tensor_tensor(
                out=m1[:].rearrange("p (sh xp) -> p sh xp", xp=2),
                in0=aw[:].unsqueeze(2).to_broadcast([128, NSUB * 10, 2]),
                in1=wxs.rearrange("p s hp xp -> p (s hp) xp"), op=OP.mult)
            m2 = qpool.tile([128, NSUB * 40], F32, tag="m2", name="m2")
            vec.tensor_tensor(
                out=m2[:].rearrange("p (sh xp yp) -> p sh xp yp", xp=2, yp=2),
                in0=m1[:].rearrange("p (sh xp) -> p sh xp", xp=2)
                    .unsqueeze(3).to_broadcast([128, NSUB * 10, 2, 2]),
                in1=wys.rearrange("p s hp yp -> p (s hp) yp")
                    .unsqueeze(2).to_broadcast([128, NSUB * 10, 2, 2]),
                op=OP.mult)
            # mtmp layout (sh, zb, z4, xy); all three APs are 3-free-dim
            mtmp = qpool.tile([128, NSUB * 10 * 32], F32, tag="mtmp",
                              name="mtmp")
            vec.tensor_tensor(
                out=mtmp[:].rearrange("p (sh zz xy) -> p sh zz xy",
                                      zz=8, xy=4),
                in0=m2[:].rearrange("p (sh xy) -> p sh xy", xy=4)
                    .unsqueeze(2).to_broadcast([128, NSUB * 10, 8, 4]),
                in1=wz[:].rearrange("p (sh zz) -> p sh zz", zz=8)
                    .unsqueeze(3).to_broadcast([128, NSUB * 10, 8, 4]),
                op=OP.mult)
            # permute (zb, z4, xy) -> (zb, xy, z4); (sh, zb) merges both sides
            mv = maskb_all[:, g * MCOL:(g + 1) * MCOL]
            vec.tensor_copy(
                out=mv.rearrange("p (szb xy z4) -> p szb xy z4", xy=4, z4=4),
                in_=mtmp[:].rearrange("p (szb z4 xy) -> p szb xy z4",
                                      z4=4, xy=4))

            # gather row indices: 576*(bx-64) + 12*(by-64) + rzq
            bx = qpool.tile([128, NSUB * 10], F32, tag="bx", name="bx")
            vec.tensor_scalar(out=bx[:].rearrange("p (s h) -> p s h", s=NSUB),
                              in0=gc6[:, :, :, 0, 0], scalar1=110.0,
                              scalar2=576.0, op0=OP.min, op1=OP.mult)
            by = qpool.tile([128, NSUB * 10], F32, tag="by", name="by")
            vec.tensor_scalar(out=by[:].rearrange("p (s h) -> p s h", s=NSUB),
                              in0=gc6[:, :, :, 1, 0], scalar1=110.0,
                              scalar2=12.0, op0=OP.min, op1=OP.mult)
            t4 = qpool.tile([128, NSUB * 10], F32, tag="t4", name="t4")
            vec.tensor_tensor(out=t4[:], in0=bx[:], in1=by[:], op=OP.add)
            idxf = qpool.tile([128, NSUB * 10], F32, tag="idxf", name="idxf")
            vec.tensor_scalar(out=idxf[:], in0=t4[:],
                              scalar1=37632.0, scalar2=None, op0=OP.subtract)
            vec.tensor_tensor(out=idxf[:], in0=idxf[:], in1=rzq[:], op=OP.add)
            idx16 = qpool.tile([128, NSUB * 10], I16, tag="idx16", name="idx16")
            vec.tensor_copy(out=idx16[:], in_=idxf[:])

            # idx fold into the gather's 16-partition-wrapped layout:
            # bounce through DRAM with contiguous 320B packets, then a DVE
            # column permute (s2-major -> s2-interleaved).
            nc.sync.dma_start(out=idxscr[g * 128:(g + 1) * 128, :], in_=idx16[:])
            idxr = gpool.tile([128, ICOL], I16, tag="idxr", name="idxr")
            scr = idxscr[g * 128:(g + 1) * 128, :]
            for gg in range(8):
                # idxr[16*gg + p16, s2*40 + f] = idx16[s2*16 + p16, f]
                src = bass.AP(scr.tensor, scr.offset,
                              [[40, 16], [16 * 40, 8], [1, 40]])
                nc.sync.dma_start(
                    out=idxr[gg * 16:(gg + 1) * 16, :]
                        .rearrange("p (s2 f) -> p s2 f", s2=8),
                    in_=src)
            # permute cols: (s2, subhl, r) -> (subhl, r, s2)
            vec.tensor_copy(
                out=idxw_all[:, g * ICOL:(g + 1) * ICOL]
                    .rearrange("p (sh r s2) -> p sh r s2", sh=8, r=5),
                in_=idxr[:].rearrange("p (s2 sh r) -> p sh r s2", s2=8, sh=8))

        # ---- stage V: value projection (bf16) ----
        # Zero the 12 units at block (46, 47) whose (xp=1, yp=1) slots no
        # write pass covers but the z-window overrun can read, plus the back
        # guard.
        zpad = const.tile([16, 512], VDT, tag="zpad", name="zpad")
        vec.memset(zpad[:], 0.0)
        for hl in range(2):
            zb46 = (46 * 576 + 47 * 12)
            nc.sync.dma_start(
                out=bass.AP(vexp[:].tensor,
                            hl * VHEAD + (G0 + zb46) * 512 + 256 + 128,
                            [[512, 12], [1, 128]]),
                in_=zpad[0:12, 0:128])
            nc.sync.dma_start(
                out=bass.AP(vexp[:].tensor, hl * VHEAD + (G0 + NUNIT) * 512,
                            [[512, 16], [1, 512]]),
                in_=zpad[:])
        for vg in range(NVSUP // NR):          # flush groups of NR supertiles
            vb = [vpool.tile([128, NR * 256], VDT, tag=f"vb{hl}", name=f"vb{hl}")
                  for hl in range(2)]
            for i in range(NR):
                vt = vg * NR + i
                vin = [vpool.tile([128, VSUP], VDT, tag=f"vin{k}", name=f"vin{k}")
                       for k in range(2)]
                for k in range(2):
                    nc.sync.dma_start(
                        out=vin[k][:],
                        in_=value_in[k * 128:(k + 1) * 128,
                                     vt * VSUP:(vt + 1) * VSUP])
                psv = ps_v.tile([128, 512], F32, tag="psv", name="psv")
                for s in range(8):
                    lhs0 = vin[0][:].rearrange("p (v e) -> p e v", e=8)[:, s, :]
                    lhs1 = vin[1][:].rearrange("p (v e) -> p e v", e=8)[:, s, :]
                    nc.tensor.matmul(psv[:, s * 64:(s + 1) * 64], lhs0,
                                     wval_sb[0][:], start=True, stop=False)
                    nc.tensor.matmul(psv[:, s * 64:(s + 1) * 64], lhs1,
                                     wval_sb[1][:], start=False, stop=True)
                # split heads, add bias, pack (blk, c, v4) per 4-voxel block
                for hl in range(2):
                    src = psv[:].rearrange("p (blk v4 hc) -> p blk hc v4",
                                           blk=2, v4=4)[:, :, hl * 32:(hl + 1) * 32, :]
                    bv = bval_sb[:, hl * 32:(hl + 1) * 32] \
                        .unsqueeze(1).unsqueeze(3).to_broadcast([128, 2, 32, 4])
                    vec.tensor_tensor(
                        out=vb[hl][:, i * 256:(i + 1) * 256]
                            .rearrange("p (blk c v4) -> p blk c v4", blk=2, c=32),
                        in0=src, in1=bv, op=OP.add)
            # flush NR supertiles (NR*1024 voxels) per head: 4 quadrant
            # passes; pass (xp, yp) lands vblock at unit vblock-xp*576-yp*12
            # slot (xp, yp). Runs are 256B ((c, z4) per vblock).
            for hl in range(2):
                for xp in range(2):
                    for yp in range(2):
                        base = (hl * VHEAD
                                + (G0 + vg * NR * 256 - xp * 576 - yp * 12)
                                * 512 + xp * 256 + yp * 128)
                        for blk in range(2):
                            dst = bass.AP(vexp[:].tensor, base + blk * 512,
                                          [[1024, 128], [256 * 512, NR],
                                           [1, 128]])
                            src = vb[hl][:] \
                                .rearrange("p (i blk x) -> p i blk x",
                                           i=NR, blk=2)[:, :, blk, :]
                            eng = nc.sync if (xp * 2 + yp) % 2 == 0 \
                                else nc.scalar
                            eng.dma_start(out=dst, in_=src)

        # ---- stage G: gather + weighted reduce per (supertile, subtile, head) ----
        for g in range(NSUP):
            q0 = g * TQ
            S = qpool.tile([128, NSUB * 64], F32, tag="S", name="S")
            for s in range(NSUB):
                for hl in range(2):
                    G = gpool.tile([128, 5 * 1024], VDT, tag="G", name="G")
                    in_g = bass.AP(vexp[:].tensor, hl * VHEAD + G0 * 512,
                                   [[512, NUNIT], [1, 1024]])
                    nc.gpsimd.dma_gather(
                        out_ap=G[:].rearrange("p (i e) -> p i e", i=5),
                        in_ap=in_g,
                        idxs_ap=idxw_all[:, (g * 8 + s * 2 + hl) * 40:
                                         (g * 8 + s * 2 + hl + 1) * 40],
                        num_idxs=NIDX, num_idxs_reg=NIDX,
                        elem_size=1024, elem_step=512,
                        single_packet=False,
                        queue_num=(s * 2 + hl) % 4)
                    # P = G * mask in place; G row = (zb, xp, yp, c, z4),
                    # mask cols (pt, zb, xy, z4) bcast over c
                    moff = g * MCOL + (s * 2 + hl) * 160
                    mg = maskb_all[:, moff:moff + 160] \
                        .rearrange("p (rb z4) -> p rb z4", z4=4) \
                        .unsqueeze(2).to_broadcast([128, 40, 32, 4])
                    gv = G[:].rearrange("p (rb c z4) -> p rb c z4",
                                        rb=40, c=32)
                    vec.tensor_tensor(out=gv, in0=gv, in1=mg, op=OP.mult)
                    # single fused reduce over (rb, z4), keeping c
                    vec.tensor_reduce(
                        out=S[:, s * 64 + hl * 32:s * 64 + hl * 32 + 32],
                        in_=G[:].rearrange("p (rb c z4) -> p c rb z4",
                                           rb=40, c=32),
                        axis=AX.XY, op=OP.add)

            # transpose S [128, 64] -> [64, 128] per subtile
            for s in range(NSUB):
                pst = ps_t.tile([64, 128], F32, tag="pst", name="pst")
                nc.tensor.transpose(pst[:], S[:, s * 64:(s + 1) * 64], ident[:])
                act.activation(out=st_sb[:, q0 + s * 128:q0 + (s + 1) * 128],
                               in_=pst[:], func=ACT.Copy)

        # ---- GEMM2: outT = wout^T @ ST (float32r moving, N=512) ----
        for mc in range(2):
            for ntile in range(NQ // 512):
                ps2 = ps_c.tile([128, 512], F32, tag="ps2", name="ps2")
                nc.tensor.matmul(ps2[:],
                                 wout_sb[:, mc * 128:(mc + 1) * 128],
                                 st_sb[:, ntile * 512:(ntile + 1) * 512],
                                 start=True, stop=True)
                ob = opool.tile([128, 512], F32, tag="ob", name="ob")
                vec.tensor_copy(out=ob[:], in_=ps2[:])
                nc.sync.dma_start(
                    out=outp[mc * 128:(mc + 1) * 128,
                             ntile * 512:(ntile + 1) * 512],
                    in_=ob[:])

    nc.compile()
    return nc


def _prep_core_inputs(inputs, b, j):
    import ml_dtypes
    q = np.ascontiguousarray(inputs["query"][b].T, np.float32)
    p = np.ascontiguousarray(inputs["pos"][b].T, np.float32)
    r = np.concatenate([inputs["reference_points"][b].T,
                        np.ones((1, NQ), np.float32)]).astype(np.float32)
    r = np.ascontiguousarray(r)
    value = np.ascontiguousarray(
        inputs["value"][b].reshape(C, NVOX)).astype(ml_dtypes.bfloat16)

    W_off, b_off = inputs["W_off"], inputs["b_off"]
    W_attn, b_attn = inputs["W_attn"], inputs["b_attn"]
    heads = [2 * j, 2 * j + 1]
    rows, biases, refr = [], [], []
    for h in heads:
        for pp in range(P):
            for ax in range(3):
                rows.append(W_off[(h * P + pp) * 3 + ax])
                biases.append(b_off[(h * P + pp) * 3 + ax] - 0.5 + 64.0)
                e = np.zeros(3, np.float32)
                e[ax] = GRID
                refr.append(e)
    for h in heads:
        for pp in range(P):
            rows.append(W_attn[h * P + pp])
            biases.append(b_attn[h * P + pp])
            refr.append(np.zeros(3, np.float32))
    wcat = np.ascontiguousarray(np.stack(rows).T, np.float32)       # (256, 40)
    ref_rhs = np.concatenate(
        [np.stack(refr).T, np.asarray(biases, np.float32)[None, :]])
    ref_rhs = np.ascontiguousarray(ref_rhs, np.float32)             # (4, 40)

    wval = np.ascontiguousarray(
        inputs["W_val"][64 * j:64 * j + 64].T).astype(ml_dtypes.bfloat16)
    bval = np.ascontiguousarray(
        np.repeat(inputs["b_val"][64 * j:64 * j + 64][None, :], 128, axis=0),
        np.float32)
    wout = np.ascontiguousarray(inputs["W_out"][:, 64 * j:64 * j + 64].T,
                                np.float32)
    zoffs = np.repeat(np.arange(W, dtype=np.float32)[None, :], 128, axis=0)
    return {
        "value_in": value, "qT": q, "pT": p, "refT": r,
        "wcat": wcat, "ref_rhs": ref_rhs,
        "wval": wval, "bval": bval, "wout": wout, "zoff": zoffs,
    }


def get_nc():
    global _NC_CACHE
    if _NC_CACHE is None:
        _NC_CACHE = build_nc()
    return _NC_CACHE


def kernel(**inputs):
    from concourse.bass_utils import run_bass_kernel_spmd

    inputs = {k: np.asarray(v) for k, v in inputs.items()}
    nc = get_nc()
    in_maps = [_prep_core_inputs(inputs, core // 4, core % 4) for core in range(8)]
    res = run_bass_kernel_spmd(nc, in_maps, list(range(8)))
    bs = inputs["query"].shape[0]
    out = np.zeros((bs, NQ, C), np.float32)
    for core in range(8):
        out[core // 4] += res.results[core]["outp"].T
    out += inputs["b_out"][None, None, :].astype(np.float32)
    return out


# revision 29
# speedup vs baseline: 3.4010x; 1.0124x over previous
"""Trainium2 Bass kernel for 3D deformable attention (8 NeuronCores).

Sharding: core i handles batch b = i // 4 and head-pair j = i % 4
(heads 2j, 2j+1, i.e. value/out channels [64j, 64j+64)).

Per-core device pipeline (emission order = coords first so DVE mask work
overlaps the value-projection DMA/PE stage):
  C. per query-supertile (512 q): coords = qs^T @ Wcat^T + [48*ref | b] (PE);
     softmax over 5 points, trilinear corner weights remapped to block
     slots (lo/hi edge clamps), z-window "hat" weights, combined mask
     m[pt, zb, xp, yp, z4] = aw*wxs*wys*wz (DVE, bf16); int16 gather unit
     indices; idx fold into dma_gather's 16-partition-wrapped layout via a
     contiguous DRAM bounce (80B packets) + DVE column permute.
  V. value projection v = W_val[64j:64j+64] @ value[b] in bf16 (PE,
     voxel-stationary); packed per 4-voxel block as (c, v4) and written
     4x quad-shifted into vexp: unit (x0, y0, zbu) holds the (xp, yp)
     quadrants of the 4-z slab, so ONE 2KB gather row = the full
     2x2x(8z) trilinear neighborhood of a sample. The 4 write passes
     stay 256B-contiguous because unit(vblock) = vblock - xp*576 - yp*12
     is linear in vblock.
  G. per (query-subtile, head): one dma_gather of 640 rows x 2KB;
     P = G * mask in place (DVE bf16); one fused AX.XY reduce over
     (pt,zb,xy,z4) keeping c -> S[q, 64].
  O. PE transpose of S, then out^T = Wout^T @ S^T, DMA out.
Host combines: out[b] = sum_j outp_j^T + b_out.
"""
import numpy as np

import concourse.bass as bass
import concourse.mybir as mybir
from concourse import bacc, tile
from concourse.masks import make_identity
from contextlib import ExitStack

F32 = mybir.dt.float32
F32R = mybir.dt.float32r
I16 = mybir.dt.int16
AX = mybir.AxisListType
OP = mybir.AluOpType
ACT = mybir.ActivationFunctionType

H, P = 8, 5
NQ, C, GRID = 4096, 256, 48
NVOX = GRID ** 3            # 110592
NSUB = 4                    # query subtiles (of 128) per supertile
TQ = 128 * NSUB             # 512
NSUP = NQ // TQ             # 8
VSUP = 1024                 # voxels per value-proj supertile
NVSUP = NVOX // VSUP        # 108
NR = 4                      # value supertiles per DRAM flush

VDT = mybir.dt.bfloat16
W = 8                       # z-window voxels per gathered row
# vexp: quad-interleaved expanded volume. Per head, blocks (x0, y0) of
# 12 z-units; unit = (xp, yp, c, z4) = 512 els (4-z slab x 4 quadrants x
# 32ch). A gather row = 2 consecutive units = the full 2x2x(8z) trilinear
# neighborhood of one sample point. unit(vblock) = vblock - xp*576 - yp*12
# is linear in vblock, so the 4 write passes keep 256B-contiguous runs.
NUNIT = 48 * 48 * 12        # 27648 addressable units per head
G0 = 588                    # front guard units (absorbs xp/yp shifts)
G1 = 16                     # back guard units (zeroed; z-window overrun)
VHEAD = (G0 + NUNIT + G1) * 512
NIDX = 5 * 128              # rows per (subtile, head) gather
MCOL = NSUB * 40 * W        # mask columns per supertile (1280)
ICOL = NSUB * 2 * 40        # idx columns per supertile (320)

_NC_CACHE = None


def build_nc():
    nc = bacc.Bacc("TRN2", target_bir_lowering=False, debug=False, num_devices=8,
                   num_swdge_queues=4)

    value_in = nc.dram_tensor("value_in", [C, NVOX], VDT, kind="ExternalInput")
    qT = nc.dram_tensor("qT", [C, NQ], F32, kind="ExternalInput")
    pT = nc.dram_tensor("pT", [C, NQ], F32, kind="ExternalInput")
    refT = nc.dram_tensor("refT", [4, NQ], F32, kind="ExternalInput")
    wcat = nc.dram_tensor("wcat", [C, 40], F32, kind="ExternalInput")
    ref_rhs = nc.dram_tensor("ref_rhs", [4, 40], F32, kind="ExternalInput")
    wval = nc.dram_tensor("wval", [C, 64], VDT, kind="ExternalInput")
    bval = nc.dram_tensor("bval", [128, 64], F32, kind="ExternalInput")
    wout = nc.dram_tensor("wout", [64, C], F32, kind="ExternalInput")
    zoff = nc.dram_tensor("zoff", [128, W], F32, kind="ExternalInput")
    outp = nc.dram_tensor("outp", [C, NQ], F32, kind="ExternalOutput")
    vexp = nc.dram_tensor("vexp", [2 * VHEAD], VDT)
    idxscr = nc.dram_tensor("idxscr", [NSUP * 128, 40], I16)

    vec = nc.vector
    act = nc.scalar

    with tile.TileContext(nc) as tc, ExitStack() as ctx:
        const = ctx.enter_context(tc.tile_pool(name="const", bufs=1))
        vpool = ctx.enter_context(tc.tile_pool(name="vpool", bufs=2))
        qpool = ctx.enter_context(tc.tile_pool(name="qpool", bufs=2))
        gpool = ctx.enter_context(tc.tile_pool(name="gpool", bufs=3))
        opool = ctx.enter_context(tc.tile_pool(name="opool", bufs=2))
        ps_v = ctx.enter_context(tc.tile_pool(name="ps_v", bufs=2, space="PSUM"))
        ps_c = ctx.enter_context(tc.tile_pool(name="ps_c", bufs=2, space="PSUM"))
        ps_t = ctx.enter_context(tc.tile_pool(name="ps_t", bufs=2, space="PSUM"))

        # ---- constants into SBUF ----
        wcat_sb = [const.tile([128, 40], F32, tag=f"wcat{k}", name=f"wcat{k}")
                   for k in range(2)]
        for k in range(2):
            nc.sync.dma_start(out=wcat_sb[k][:], in_=wcat[k * 128:(k + 1) * 128, :])
        refrhs_sb = const.tile([4, 40], F32, tag="refrhs", name="refrhs")
        nc.sync.dma_start(out=refrhs_sb[:], in_=ref_rhs[:])
        wval_sb = [const.tile([128, 64], VDT, tag=f"wval{k}", name=f"wval{k}")
                   for k in range(2)]
        for k in range(2):
            nc.sync.dma_start(out=wval_sb[k][:], in_=wval[k * 128:(k + 1) * 128, :])
        bval_sb = const.tile([128, 64], F32, tag="bval", name="bval")
        nc.sync.dma_start(out=bval_sb[:], in_=bval[:])
        wout_sb = const.tile([64, C], F32, tag="wout", name="wout")
        nc.sync.dma_start(out=wout_sb[:], in_=wout[:])
        zoff_sb = const.tile([128, W], F32, tag="zoff", name="zoff")
        nc.sync.dma_start(out=zoff_sb[:], in_=zoff[:])
        ident = const.tile([128, 128], F32, tag="ident", name="ident")
        make_identity(nc, ident[:])

        # persistent big buffers
        qs_sb = [const.tile([128, NQ], F32, tag=f"qs{k}", name=f"qs{k}")
                 for k in range(2)]
        ref_sb = const.tile([4, NQ], F32, tag="refq", name="refq")
        st_sb = const.tile([64, NQ], F32, tag="st", name="st")
        maskb_all = const.tile([128, NSUP * MCOL], VDT, tag="maskb", name="maskb")
        idxw_all = const.tile([128, NSUP * ICOL], I16, tag="idxw", name="idxw")

        # ---- stage Q0: load q, pos, ref; qs = q + p ----
        for k in range(2):
            for half in range(4):
                sl = slice(half * (NQ // 4), (half + 1) * (NQ // 4))
                ptmp = qpool.tile([128, NQ // 4], F32, tag="ptmp", name="ptmp")
                nc.sync.dma_start(out=qs_sb[k][:, sl],
                                  in_=qT[k * 128:(k + 1) * 128, sl])
                nc.sync.dma_start(out=ptmp[:], in_=pT[k * 128:(k + 1) * 128, sl])
                vec.tensor_tensor(out=qs_sb[k][:, sl], in0=qs_sb[k][:, sl],
                                  in1=ptmp[:], op=OP.add)
        nc.sync.dma_start(out=ref_sb[:], in_=refT[:])

        # ---- stage C: coords / masks / gather indices, all supertiles ----
        for g in range(NSUP):
            q0 = g * TQ
            psc = ps_c.tile([128, 160], F32, tag="psc", name="psc")
            for s in range(NSUB):
                qsl = slice(q0 + s * 128, q0 + (s + 1) * 128)
                nc.tensor.matmul(psc[:, s * 40:(s + 1) * 40],
                                 qs_sb[0][:, qsl], wcat_sb[0][:],
                                 start=True, stop=False)
                nc.tensor.matmul(psc[:, s * 40:(s + 1) * 40],
                                 qs_sb[1][:, qsl], wcat_sb[1][:],
                                 start=False, stop=False)
                nc.tensor.matmul(psc[:, s * 40:(s + 1) * 40],
                                 ref_sb[:, qsl], refrhs_sb[:],
                                 start=False, stop=True)
            coords = qpool.tile([128, 160], F32, tag="coords", name="coords")
            act.activation(out=coords[:], in_=psc[:], func=ACT.Copy)

            co = coords[:].rearrange("p (s r) -> p s r", s=NSUB)
            pix = co[:, :, 0:30]                        # (s, hp*ax)
            logit = co[:, :, 30:40]                     # (s, hp)

            # softmax over P
            exlog = qpool.tile([128, NSUB * 10], F32, tag="exlog", name="exlog")
            act.activation(out=exlog[:], in_=logit, func=ACT.Exp)
            ex4 = exlog[:].rearrange("p (s h q) -> p s h q", s=NSUB, h=2)
            sums = qpool.tile([128, NSUB * 2], F32, tag="sums", name="sums")
            vec.tensor_reduce(out=sums[:].rearrange("p (s h) -> p s h", s=NSUB),
                              in_=ex4, axis=AX.X, op=OP.add)
            rsum = qpool.tile([128, NSUB * 2], F32, tag="rsum", name="rsum")
            vec.reciprocal(out=rsum[:], in_=sums[:])
            aw = qpool.tile([128, NSUB * 10], F32, tag="aw", name="aw")
            vec.tensor_tensor(
                out=aw[:].rearrange("p (sh q) -> p sh q", q=5),
                in0=exlog[:].rearrange("p (sh q) -> p sh q", q=5),
                in1=rsum[:].unsqueeze(2).to_broadcast([128, NSUB * 2, 5]),
                op=OP.mult)

            # corner math on the 30 pixel rows
            NPX = NSUB * 30
            # flo = round(pix - 0.5) via the 2^23 magic add (== floor except
            # exactly-integer pix, where the phantom corner gets zero weight)
            flo = qpool.tile([128, NPX], F32, tag="flo", name="flo")
            vec.tensor_scalar(out=flo[:].rearrange("p (s r) -> p s r", s=NSUB),
                              in0=pix, scalar1=8388607.5, scalar2=8388608.0,
                              op0=OP.add, op1=OP.subtract)
            fl3 = flo[:].rearrange("p (s r) -> p s r", s=NSUB)
            frac = qpool.tile([128, NPX], F32, tag="frac", name="frac")
            vec.tensor_tensor(out=frac[:].rearrange("p (s r) -> p s r", s=NSUB),
                              in0=pix, in1=fl3, op=OP.subtract)
            fr3 = frac[:].rearrange("p (s r) -> p s r", s=NSUB)
            # gcorn: (s, hp, ax, dx) — clamped corner coords (biased +64)
            gcorn = qpool.tile([128, NPX * 2], F32, tag="gcorn", name="gcorn")
            gc4 = gcorn[:].rearrange("p (s r d) -> p s r d", s=NSUB, d=2)
            vec.tensor_scalar(out=gc4[:, :, :, 0], in0=fl3,
                              scalar1=64.0, scalar2=111.0, op0=OP.max, op1=OP.min)
            g1m = qpool.tile([128, NPX], F32, tag="g1m", name="g1m")
            vec.tensor_scalar(out=g1m[:], in0=flo[:],
                              scalar1=63.0, scalar2=110.0, op0=OP.max, op1=OP.min)
            vec.tensor_scalar(out=gc4[:, :, :, 1], in0=g1m[:]
                              .rearrange("p (s r) -> p s r", s=NSUB),
                              scalar1=1.0, scalar2=None, op0=OP.add)
            # validity via clip-equality
            v0 = qpool.tile([128, NPX], F32, tag="v0", name="v0")
            vec.tensor_tensor(out=v0[:].rearrange("p (s r) -> p s r", s=NSUB),
                              in0=gc4[:, :, :, 0], in1=fl3, op=OP.is_equal)
            v1 = qpool.tile([128, NPX], F32, tag="v1", name="v1")
            vec.tensor_tensor(out=v1[:], in0=g1m[:], in1=flo[:], op=OP.is_equal)
            # corner weights (x/y rows used; z rows ignored later)
            om = qpool.tile([128, NPX], F32, tag="om", name="om")
            vec.tensor_scalar(out=om[:], in0=frac[:], scalar1=-1.0, scalar2=1.0,
                              op0=OP.mult, op1=OP.add)
            wcorn = qpool.tile([128, NPX * 2], F32, tag="wcorn", name="wcorn")
            wc4 = wcorn[:].rearrange("p (s r d) -> p s r d", s=NSUB, d=2)
            vec.tensor_tensor(out=wc4[:, :, :, 0],
                              in0=om[:].rearrange("p (s r) -> p s r", s=NSUB),
                              in1=v0[:].rearrange("p (s r) -> p s r", s=NSUB),
                              op=OP.mult)
            vec.tensor_tensor(out=wc4[:, :, :, 1],
                              in0=fr3,
                              in1=v1[:].rearrange("p (s r) -> p s r", s=NSUB),
                              op=OP.mult)

            # z window: rzq = floor((gz-64)/4) in [0,11]; W-slot hat weights
            gc6 = gcorn[:].rearrange("p (s hp a d) -> p s hp a d",
                                     s=NSUB, hp=10, a=3)
            gz = gc6[:, :, :, 2, 0]
            pz = co[:, :, 0:30].rearrange("p s (hp a) -> p s hp a", a=3)[:, :, :, 2]
            rzq = qpool.tile([128, NSUB * 10], F32, tag="rzq", name="rzq")
            tq = qpool.tile([128, NSUB * 10], F32, tag="tq", name="tq")
            vec.tensor_scalar(out=tq[:].rearrange("p (s h) -> p s h", s=NSUB),
                              in0=gz, scalar1=0.25, scalar2=16.375,
                              op0=OP.mult, op1=OP.subtract)
            vec.tensor_scalar(out=rzq[:], in0=tq[:],
                              scalar1=8388624.0, scalar2=8388624.0,
                              op0=OP.add, op1=OP.subtract)
            # d0 = (4*rzq + 64) - pz ; dk = d0 + k
            zb4 = qpool.tile([128, NSUB * 10], F32, tag="zb4", name="zb4")
            vec.tensor_scalar(out=zb4[:], in0=rzq[:], scalar1=4.0, scalar2=64.0,
                              op0=OP.mult, op1=OP.add)
            d0 = qpool.tile([128, NSUB * 10], F32, tag="d0", name="d0")
            vec.tensor_tensor(out=d0[:].rearrange("p (s h) -> p s h", s=NSUB),
                              in0=zb4[:].rearrange("p (s h) -> p s h", s=NSUB),
                              in1=pz, op=OP.subtract)
            dk = qpool.tile([128, NSUB * 10 * W], F32, tag="dk", name="dk")
            vec.tensor_tensor(
                out=dk[:].rearrange("p (sh k) -> p sh k", k=W),
                in0=d0[:].unsqueeze(2).to_broadcast([128, NSUB * 10, W]),
                in1=zoff_sb[:].unsqueeze(1).to_broadcast([128, NSUB * 10, W]),
                op=OP.add)
            adk = qpool.tile([128, NSUB * 10 * W], F32, tag="adk", name="adk")
            act.activation(out=adk[:], in_=dk[:], func=ACT.Abs)
            hat = qpool.tile([128, NSUB * 10 * W], F32, tag="hat", name="hat")
            act.activation(out=hat[:], in_=adk[:], func=ACT.Relu,
                           scale=-1.0, bias=1.0)
            # upper bound: slot z 4*rzq+64+k <= 111  <=>  dk <= 111 - pz
            ub = qpool.tile([128, NSUB * 10], F32, tag="ub", name="ub")
            vec.tensor_scalar(out=ub[:].rearrange("p (s h) -> p s h", s=NSUB),
                              in0=pz, scalar1=-1.0, scalar2=111.0,
                              op0=OP.mult, op1=OP.add)
            vub = qpool.tile([128, NSUB * 10 * W], F32, tag="vub", name="vub")
            vec.tensor_tensor(
                out=vub[:].rearrange("p (sh k) -> p sh k", k=W),
                in0=dk[:].rearrange("p (sh k) -> p sh k", k=W),
                in1=ub[:].unsqueeze(2).to_broadcast([128, NSUB * 10, W]),
                op=OP.is_le)
            wz = qpool.tile([128, NSUB * 10 * W], F32, tag="wz", name="wz")
            vec.tensor_tensor(out=wz[:], in0=hat[:], in1=vub[:], op=OP.mult)

            # slot weights: block bx = min(gx0, 110); slot s holds x = bx+s.
            # When gx0 == 111 (x >= 47) the corner-0 weight moves to slot 1.
            wc6 = wcorn[:].rearrange("p (s hp a d) -> p s hp a d",
                                     s=NSUB, hp=10, a=3)
            ws = qpool.tile([128, NSUB * 40], F32, tag="ws", name="ws")
            ws4 = ws[:].rearrange("p (s hp a d) -> p s hp a d", s=NSUB, hp=10, a=2)
            hi = qpool.tile([128, NSUB * 20], F32, tag="hi", name="hi")
            hi3 = hi[:].rearrange("p (s hp a) -> p s hp a", s=NSUB, hp=10)
            hit = qpool.tile([128, NSUB * 20], F32, tag="hit", name="hit")
            hit3 = hit[:].rearrange("p (s hp a) -> p s hp a", s=NSUB, hp=10)
            # hi = (gc0 == 111): corner-0 weight moves to slot 1.
            # lo = (g1m == 63): corner-1 weight (position 64) moves to slot 0.
            vec.tensor_scalar(out=hi3, in0=gc6[:, :, :, 0:2, 0],
                              scalar1=111.0, scalar2=None, op0=OP.is_equal)
            vec.tensor_tensor(out=hit3, in0=wc6[:, :, :, 0:2, 0], in1=hi3,
                              op=OP.mult)
            lo = qpool.tile([128, NSUB * 20], F32, tag="lo", name="lo")
            lo3 = lo[:].rearrange("p (s hp a) -> p s hp a", s=NSUB, hp=10)
            lot = qpool.tile([128, NSUB * 20], F32, tag="lot", name="lot")
            lot3 = lot[:].rearrange("p (s hp a) -> p s hp a", s=NSUB, hp=10)
            g1m3 = g1m[:].rearrange("p (s hp a) -> p s hp a", s=NSUB, hp=10)
            vec.tensor_scalar(out=lo3, in0=g1m3[:, :, :, 0:2],
                              scalar1=63.0, scalar2=None, op0=OP.is_equal)
            vec.tensor_tensor(out=lot3, in0=wc6[:, :, :, 0:2, 1], in1=lo3,
                              op=OP.mult)
            vec.tensor_tensor(out=ws4[:, :, :, :, 0],
                              in0=wc6[:, :, :, 0:2, 0], in1=hit3,
                              op=OP.subtract)
            vec.tensor_tensor(out=ws4[:, :, :, :, 0],
                              in0=ws4[:, :, :, :, 0], in1=lot3, op=OP.add)
            vec.tensor_tensor(out=ws4[:, :, :, :, 1],
                              in0=wc6[:, :, :, 0:2, 1], in1=hit3, op=OP.add)
            vec.tensor_tensor(out=ws4[:, :, :, :, 1],
                              in0=ws4[:, :, :, :, 1], in1=lot3, op=OP.subtract)
            wxs = ws4[:, :, :, 0, :]                    # (s, hp, xslot)
            wys = ws4[:, :, :, 1, :]                    # (s, hp, yslot)

            # mask: m[(s hp), zb, xp, yp, z4] = aw*wxs*wys*wz
            m1 = qpool.tile([128, NSUB * 20], F32, tag="m1", name="m1")
            vec.tensor_tensor(
                out=m1[:].rearrange("p (sh xp) -> p sh xp", xp=2),
                in0=aw[:].unsqueeze(2).to_broadcast([128, NSUB * 10, 2]),
                in1=wxs.rearrange("p s hp xp -> p (s hp) xp"), op=OP.mult)
            m2 = qpool.tile([128, NSUB * 40], F32, tag="m2", name="m2")
            vec.tensor_tensor(
                out=m2[:].rearrange("p (sh xp yp) -> p sh xp yp", xp=2, yp=2),
                in0=m1[:].rearrange("p (sh xp) -> p sh xp", xp=2)
                    .unsqueeze(3).to_broadcast([128, NSUB * 10, 2, 2]),
                in1=wys.rearrange("p s hp yp -> p (s hp) yp")
                    .unsqueeze(2).to_broadcast([128, NSUB * 10, 2, 2]),
                op=OP.mult)
            # mtmp layout (sh, zb, z4, xy); all three APs are 3-free-dim
            mtmp = qpool.tile([128, NSUB * 10 * 32], F32, tag="mtmp",
                              name="mtmp")
            vec.tensor_tensor(
                out=mtmp[:].rearrange("p (sh zz xy) -> p sh zz xy",
                                      zz=8, xy=4),
                in0=m2[:].rearrange("p (sh xy) -> p sh xy", xy=4)
                    .unsqueeze(2).to_broadcast([128, NSUB * 10, 8, 4]),
                in1=wz[:].rearrange("p (sh zz) -> p sh zz", zz=8)
                    .unsqueeze(3).to_broadcast([128, NSUB * 10, 8, 4]),
                op=OP.mult)
            # permute (zb, z4, xy) -> (zb, xy, z4); (sh, zb) merges both sides
            mv = maskb_all[:, g * MCOL:(g + 1) * MCOL]
            vec.tensor_copy(
                out=mv.rearrange("p (szb xy z4) -> p szb xy z4", xy=4, z4=4),
                in_=mtmp[:].rearrange("p (szb z4 xy) -> p szb xy z4",
                                      z4=4, xy=4))

            # gather row indices: 576*(bx-64) + 12*(by-64) + rzq
            bx = qpool.tile([128, NSUB * 10], F32, tag="bx", name="bx")
            vec.tensor_scalar(out=bx[:].rearrange("p (s h) -> p s h", s=NSUB),
                              in0=gc6[:, :, :, 0, 0], scalar1=110.0,
                              scalar2=576.0, op0=OP.min, op1=OP.mult)
            by = qpool.tile([128, NSUB * 10], F32, tag="by", name="by")
            vec.tensor_scalar(out=by[:].rearrange("p (s h) -> p s h", s=NSUB),
                              in0=gc6[:, :, :, 1, 0], scalar1=110.0,
                              scalar2=12.0, op0=OP.min, op1=OP.mult)
            t4 = qpool.tile([128, NSUB * 10], F32, tag="t4", name="t4")
            vec.tensor_tensor(out=t4[:], in0=bx[:], in1=by[:], op=OP.add)
            idxf = qpool.tile([128, NSUB * 10], F32, tag="idxf", name="idxf")
            vec.tensor_scalar(out=idxf[:], in0=t4[:],
                              scalar1=37632.0, scalar2=None, op0=OP.subtract)
            vec.tensor_tensor(out=idxf[:], in0=idxf[:], in1=rzq[:], op=OP.add)
            idx16 = qpool.tile([128, NSUB * 10], I16, tag="idx16", name="idx16")
            vec.tensor_copy(out=idx16[:], in_=idxf[:])

            # idx fold into the gather's 16-partition-wrapped layout:
            # bounce through DRAM with contiguous 320B packets, then a DVE
            # column permute (s2-major -> s2-interleaved).
            nc.sync.dma_start(out=idxscr[g * 128:(g + 1) * 128, :], in_=idx16[:])
            idxr = gpool.tile([128, ICOL], I16, tag="idxr", name="idxr")
            scr = idxscr[g * 128:(g + 1) * 128, :]
            for gg in range(8):
                # idxr[16*gg + p16, s2*40 + f] = idx16[s2*16 + p16, f]
                src = bass.AP(scr.tensor, scr.offset,
                              [[40, 16], [16 * 40, 8], [1, 40]])
                nc.sync.dma_start(
                    out=idxr[gg * 16:(gg + 1) * 16, :]
                        .rearrange("p (s2 f) -> p s2 f", s2=8),
                    in_=src)
            # permute cols: (s2, subhl, r) -> (subhl, r, s2)
            vec.tensor_copy(
                out=idxw_all[:, g * ICOL:(g + 1) * ICOL]
                    .rearrange("p (sh r s2) -> p sh r s2", sh=8, r=5),
                in_=idxr[:].rearrange("p (s2 sh r) -> p sh r s2", s2=8, sh=8))

        # ---- stage V: value projection (bf16) ----
        # Zero the 12 units at block (46, 47) whose (xp=1, yp=1) slots no
        # write pass covers but the z-window overrun can read, plus the back
        # guard.
        zpad = const.tile([16, 512], VDT, tag="zpad", name="zpad")
        vec.memset(zpad[:], 0.0)
        for hl in range(2):
            zb46 = (46 * 576 + 47 * 12)
            nc.sync.dma_start(
                out=bass.AP(vexp[:].tensor,
                            hl * VHEAD + (G0 + zb46) * 512 + 256 + 128,
                            [[512, 12], [1, 128]]),
                in_=zpad[0:12, 0:128])
            nc.sync.dma_start(
                out=bass.AP(vexp[:].tensor, hl * VHEAD + (G0 + NUNIT) * 512,
                            [[512, 16], [1, 512]]),
                in_=zpad[:])
        for vg in range(NVSUP // NR):          # flush groups of NR supertiles
            vb = [vpool.tile([128, NR * 256], VDT, tag=f"vb{hl}", name=f"vb{hl}")
                  for hl in range(2)]
            for i in range(NR):
                vt = vg * NR + i
                vin = [vpool.tile([128, VSUP], VDT, tag=f"vin{k}", name=f"vin{k}")
                       for k in range(2)]
                for k in range(2):
                    nc.sync.dma_start(
                        out=vin[k][:],
                        in_=value_in[k * 128:(k + 1) * 128,
                                     vt * VSUP:(vt + 1) * VSUP])
                psv = ps_v.tile([128, 512], F32, tag="psv", name="psv")
                for s in range(8):
                    lhs0 = vin[0][:].rearrange("p (v e) -> p e v", e=8)[:, s, :]
                    lhs1 = vin[1][:].rearrange("p (v e) -> p e v", e=8)[:, s, :]
                    nc.tensor.matmul(psv[:, s * 64:(s + 1) * 64], lhs0,
                                     wval_sb[0][:], start=True, stop=False)
                    nc.tensor.matmul(psv[:, s * 64:(s + 1) * 64], lhs1,
                                     wval_sb[1][:], start=False, stop=True)
                # split heads, add bias, pack (blk, c, v4) per 4-voxel block
                for hl in range(2):
                    src = psv[:].rearrange("p (blk v4 hc) -> p blk hc v4",
                                           blk=2, v4=4)[:, :, hl * 32:(hl + 1) * 32, :]
                    bv = bval_sb[:, hl * 32:(hl + 1) * 32] \
                        .unsqueeze(1).unsqueeze(3).to_broadcast([128, 2, 32, 4])
                    vec.tensor_tensor(
                        out=vb[hl][:, i * 256:(i + 1) * 256]
                            .rearrange("p (blk c v4) -> p blk c v4", blk=2, c=32),
                        in0=src, in1=bv, op=OP.add)
            # flush NR supertiles (NR*1024 voxels) per head: 4 quadrant
            # passes; pass (xp, yp) lands vblock at unit vblock-xp*576-yp*12
            # slot (xp, yp). Runs are 256B ((c, z4) per vblock).
            for hl in range(2):
                for xp in range(2):
                    for yp in range(2):
                        base = (hl * VHEAD
                                + (G0 + vg * NR * 256 - xp * 576 - yp * 12)
                                * 512 + xp * 256 + yp * 128)
                        for blk in range(2):
                            dst = bass.AP(vexp[:].tensor, base + blk * 512,
                                          [[1024, 128], [256 * 512, NR],
                                           [1, 128]])
                            src = vb[hl][:] \
                                .rearrange("p (i blk x) -> p i blk x",
                                           i=NR, blk=2)[:, :, blk, :]
                            eng = nc.sync if (xp * 2 + yp) % 2 == 0 \
                                else nc.scalar
                            eng.dma_start(out=dst, in_=src)

        # ---- stage G: gather + weighted reduce per (supertile, subtile, head) ----
        for g in range(NSUP):
            q0 = g * TQ
            S = qpool.tile([128, NSUB * 64], F32, tag="S", name="S")
            for s in range(NSUB):
                for hl in range(2):
                    G = gpool.tile([128, 5 * 1024], VDT, tag="G", name="G")
                    in_g = bass.AP(vexp[:].tensor, hl * VHEAD + G0 * 512,
                                   [[512, NUNIT], [1, 1024]])
                    nc.gpsimd.dma_gather(
                        out_ap=G[:].rearrange("p (i e) -> p i e", i=5),
                        in_ap=in_g,
                        idxs_ap=idxw_all[:, (g * 8 + s * 2 + hl) * 40:
                                         (g * 8 + s * 2 + hl + 1) * 40],
                        num_idxs=NIDX, num_idxs_reg=NIDX,
                        elem_size=1024, elem_step=512,
                        single_packet=False,
                        queue_num=(s * 2 + hl) % 4)
                    # P = G * mask in place; G row = (zb, xp, yp, c, z4),
                    # mask cols (pt, zb, xy, z4) bcast over c
                    moff = g * MCOL + (s * 2 + hl) * 160
                    mg = maskb_all[:, moff:moff + 160] \
                        .rearrange("p (rb z4) -> p rb z4", z4=4) \
                        .unsqueeze(2).to_broadcast([128, 40, 32, 4])
                    gv = G[:].rearrange("p (rb c z4) -> p rb c z4",
                                        rb=40, c=32)
                    vec.tensor_tensor(out=gv, in0=gv, in1=mg, op=OP.mult)
                    # single fused reduce over (rb, z4), keeping c
                    vec.tensor_reduce(
                        out=S[:, s * 64 + hl * 32:s * 64 + hl * 32 + 32],
                        in_=G[:].rearrange("p (rb c z4) -> p c rb z4",
                                           rb=40, c=32),
                        axis=AX.XY, op=OP.add)

            # transpose S [128, 64] -> [64, 128] per subtile
            for s in range(NSUB):
                pst = ps_t.tile([64, 128], F32, tag="pst", name="pst")
                nc.tensor.transpose(pst[:], S[:, s * 64:(s + 1) * 64], ident[:])
                act.activation(out=st_sb[:, q0 + s * 128:q0 + (s + 1) * 128],
                               in_=pst[:], func=ACT.Copy)

        # ---- GEMM2: outT = wout^T @ ST (float32r moving, N=512) ----
        for mc in range(2):
            for ntile in range(NQ // 512):
                ps2 = ps_c.tile([128, 512], F32, tag="ps2", name="ps2")
                nc.tensor.matmul(ps2[:],
                                 wout_sb[:, mc * 128:(mc + 1) * 128],
                                 st_sb[:, ntile * 512:(ntile + 1) * 512],
                                 start=True, stop=True)
                ob = opool.tile([128, 512], F32, tag="ob", name="ob")
                vec.tensor_copy(out=ob[:], in_=ps2[:])
                nc.sync.dma_start(
                    out=outp[mc * 128:(mc + 1) * 128,
                             ntile * 512:(ntile + 1) * 512],
                    in_=ob[:])

    nc.compile()
    return nc


def _prep_core_inputs(inputs, b, j):
    import ml_dtypes
    q = np.ascontiguousarray(inputs["query"][b].T, np.float32)
    p = np.ascontiguousarray(inputs["pos"][b].T, np.float32)
    r = np.concatenate([inputs["reference_points"][b].T,
                        np.ones((1, NQ), np.float32)]).astype(np.float32)
    r = np.ascontiguousarray(r)
    value = np.ascontiguousarray(
        inputs["value"][b].reshape(C, NVOX)).astype(ml_dtypes.bfloat16)

    W_off, b_off = inputs["W_off"], inputs["b_off"]
    W_attn, b_attn = inputs["W_attn"], inputs["b_attn"]
    heads = [2 * j, 2 * j + 1]
    rows, biases, refr = [], [], []
    for h in heads:
        for pp in range(P):
            for ax in range(3):
                rows.append(W_off[(h * P + pp) * 3 + ax])
                biases.append(b_off[(h * P + pp) * 3 + ax] - 0.5 + 64.0)
                e = np.zeros(3, np.float32)
                e[ax] = GRID
                refr.append(e)
    for h in heads:
        for pp in range(P):
            rows.append(W_attn[h * P + pp])
            biases.append(b_attn[h * P + pp])
            refr.append(np.zeros(3, np.float32))
    wcat = np.ascontiguousarray(np.stack(rows).T, np.float32)       # (256, 40)
    ref_rhs = np.concatenate(
        [np.stack(refr).T, np.asarray(biases, np.float32)[None, :]])
    ref_rhs = np.ascontiguousarray(ref_rhs, np.float32)             # (4, 40)

    wval = np.ascontiguousarray(
        inputs["W_val"][64 * j:64 * j + 64].T).astype(ml_dtypes.bfloat16)
    bval = np.ascontiguousarray(
        np.repeat(inputs["b_val"][64 * j:64 * j + 64][None, :], 128, axis=0),
        np.float32)
    wout = np.ascontiguousarray(inputs["W_out"][:, 64 * j:64 * j + 64].T,
                                np.float32)
    zoffs = np.repeat(np.arange(W, dtype=np.float32)[None, :], 128, axis=0)
    return {
        "value_in": value, "qT": q, "pT": p, "refT": r,
        "wcat": wcat, "ref_rhs": ref_rhs,
        "wval": wval, "bval": bval, "wout": wout, "zoff": zoffs,
    }


def get_nc():
    global _NC_CACHE
    if _NC_CACHE is None:
        _NC_CACHE = build_nc()
    return _NC_CACHE


def kernel(**inputs):
    from concourse.bass_utils import run_bass_kernel_spmd

    inputs = {k: np.asarray(v) for k, v in inputs.items()}
    nc = get_nc()
    in_maps = [_prep_core_inputs(inputs, core // 4, core % 4) for core in range(8)]
    res = run_bass_kernel_spmd(nc, in_maps, list(range(8)))
    bs = inputs["query"].shape[0]
    out = np.zeros((bs, NQ, C), np.float32)
    for core in range(8):
        out[core // 4] += res.results[core]["outp"].T
    out += inputs["b_out"][None, None, :].astype(np.float32)
    return out


# revision 31
# speedup vs baseline: 3.5305x; 1.0381x over previous
"""Trainium2 Bass kernel for 3D deformable attention (8 NeuronCores).

Sharding: core i handles batch b = i // 4 and head-pair j = i % 4
(heads 2j, 2j+1, i.e. value/out channels [64j, 64j+64)).

Per-core device pipeline (emission order = coords first so DVE mask work
overlaps the value-projection DMA/PE stage):
  C. per query-supertile (512 q): coords = qs^T @ Wcat^T + [48*ref | b] (PE);
     softmax over 5 points, trilinear corner weights remapped to block
     slots (lo/hi edge clamps), z-window "hat" weights, combined mask
     m[pt, zb, xp, yp, z4] = aw*wxs*wys*wz (DVE, bf16); int16 gather unit
     indices; idx fold into dma_gather's 16-partition-wrapped layout via a
     contiguous DRAM bounce (80B packets) + DVE column permute.
  V. value projection v = W_val[64j:64j+64] @ value[b] in bf16 (PE,
     voxel-stationary); packed per 4-voxel block as (c, v4) and written
     4x quad-shifted into vexp: unit (x0, y0, zbu) holds the (xp, yp)
     quadrants of the 4-z slab, so ONE 2KB gather row = the full
     2x2x(8z) trilinear neighborhood of a sample. The 4 write passes
     stay 256B-contiguous because unit(vblock) = vblock - xp*576 - yp*12
     is linear in vblock.
  G. per (query-subtile, head): one dma_gather of 640 rows x 2KB;
     P = G * mask in place (DVE bf16); one fused AX.XY reduce over
     (pt,zb,xy,z4) keeping c -> S[q, 64].
  O. PE transpose of S, then out^T = Wout^T @ S^T, DMA out.
Host combines: out[b] = sum_j outp_j^T + b_out.
"""
import numpy as np

import concourse.bass as bass
import concourse.mybir as mybir
from concourse import bacc, tile
from concourse.masks import make_identity
from contextlib import ExitStack

F32 = mybir.dt.float32
F32R = mybir.dt.float32r
I16 = mybir.dt.int16
AX = mybir.AxisListType
OP = mybir.AluOpType
ACT = mybir.ActivationFunctionType

H, P = 8, 5
NQ, C, GRID = 4096, 256, 48
NVOX = GRID ** 3            # 110592
NSUB = 4                    # query subtiles (of 128) per supertile
TQ = 128 * NSUB             # 512
NSUP = NQ // TQ             # 8
VSUP = 1024                 # voxels per value-proj supertile
NVSUP = NVOX // VSUP        # 108
NR = 4                      # value supertiles per DRAM flush

VDT = mybir.dt.bfloat16
W = 8                       # z-window voxels per gathered row
# vexp: quad-interleaved expanded volume. Per head, blocks (x0, y0) of
# 12 z-units; unit = (xp, yp, c, z4) = 512 els (4-z slab x 4 quadrants x
# 32ch). A gather row = 2 consecutive units = the full 2x2x(8z) trilinear
# neighborhood of one sample point. unit(vblock) = vblock - xp*576 - yp*12
# is linear in vblock, so the 4 write passes keep 256B-contiguous runs.
NUNIT = 48 * 48 * 12        # 27648 addressable units per head
G0 = 588                    # front guard units (absorbs xp/yp shifts)
G1 = 16                     # back guard units (zeroed; z-window overrun)
VHEAD = (G0 + NUNIT + G1) * 512
NIDX = 5 * 128              # rows per (subtile, head) gather
MCOL = NSUB * 40 * W        # mask columns per supertile (1280)
ICOL = NSUB * 2 * 40        # idx columns per supertile (320)

_NC_CACHE = None


def build_nc():
    nc = bacc.Bacc("TRN2", target_bir_lowering=False, debug=False, num_devices=8,
                   num_swdge_queues=4)

    value_in = nc.dram_tensor("value_in", [C, NVOX], VDT, kind="ExternalInput")
    qT = nc.dram_tensor("qT", [C, NQ], F32, kind="ExternalInput")
    pT = nc.dram_tensor("pT", [C, NQ], F32, kind="ExternalInput")
    refT = nc.dram_tensor("refT", [4, NQ], F32, kind="ExternalInput")
    wcat = nc.dram_tensor("wcat", [C, 40], F32, kind="ExternalInput")
    ref_rhs = nc.dram_tensor("ref_rhs", [4, 40], F32, kind="ExternalInput")
    wval = nc.dram_tensor("wval", [C, 64], VDT, kind="ExternalInput")
    bval = nc.dram_tensor("bval", [128, 64], F32, kind="ExternalInput")
    wout = nc.dram_tensor("wout", [64, C], F32, kind="ExternalInput")
    zoff = nc.dram_tensor("zoff", [128, W], F32, kind="ExternalInput")
    outp = nc.dram_tensor("outp", [C, NQ], F32, kind="ExternalOutput")
    vexp = nc.dram_tensor("vexp", [2 * VHEAD], VDT)
    idxscr = nc.dram_tensor("idxscr", [NSUP * 128, 40], I16)

    vec = nc.vector
    act = nc.scalar

    with tile.TileContext(nc) as tc, ExitStack() as ctx:
        const = ctx.enter_context(tc.tile_pool(name="const", bufs=1))
        vpool = ctx.enter_context(tc.tile_pool(name="vpool", bufs=3))
        qpool = ctx.enter_context(tc.tile_pool(name="qpool", bufs=2))
        gpool = ctx.enter_context(tc.tile_pool(name="gpool", bufs=3))
        opool = ctx.enter_context(tc.tile_pool(name="opool", bufs=2))
        ps_v = ctx.enter_context(tc.tile_pool(name="ps_v", bufs=2, space="PSUM"))
        ps_c = ctx.enter_context(tc.tile_pool(name="ps_c", bufs=2, space="PSUM"))
        ps_t = ctx.enter_context(tc.tile_pool(name="ps_t", bufs=2, space="PSUM"))

        # ---- constants into SBUF ----
        wcat_sb = [const.tile([128, 40], F32, tag=f"wcat{k}", name=f"wcat{k}")
                   for k in range(2)]
        for k in range(2):
            nc.sync.dma_start(out=wcat_sb[k][:], in_=wcat[k * 128:(k + 1) * 128, :])
        refrhs_sb = const.tile([4, 40], F32, tag="refrhs", name="refrhs")
        nc.sync.dma_start(out=refrhs_sb[:], in_=ref_rhs[:])
        wval_sb = [const.tile([128, 64], VDT, tag=f"wval{k}", name=f"wval{k}")
                   for k in range(2)]
        for k in range(2):
            nc.sync.dma_start(out=wval_sb[k][:], in_=wval[k * 128:(k + 1) * 128, :])
        bval_sb = const.tile([128, 64], F32, tag="bval", name="bval")
        nc.sync.dma_start(out=bval_sb[:], in_=bval[:])
        wout_sb = const.tile([64, C], F32, tag="wout", name="wout")
        nc.sync.dma_start(out=wout_sb[:], in_=wout[:])
        zoff_sb = const.tile([128, W], F32, tag="zoff", name="zoff")
        nc.sync.dma_start(out=zoff_sb[:], in_=zoff[:])
        ident = const.tile([128, 128], F32, tag="ident", name="ident")
        make_identity(nc, ident[:])

        # persistent big buffers
        qs_sb = [const.tile([128, NQ], F32, tag=f"qs{k}", name=f"qs{k}")
                 for k in range(2)]
        ref_sb = const.tile([4, NQ], F32, tag="refq", name="refq")
        st_sb = const.tile([64, NQ], F32, tag="st", name="st")
        maskb_all = const.tile([128, NSUP * MCOL], VDT, tag="maskb", name="maskb")
        idxw_all = const.tile([128, NSUP * ICOL], I16, tag="idxw", name="idxw")

        # ---- stage Q0: load q, pos, ref; qs = q + p ----
        for k in range(2):
            for half in range(4):
                sl = slice(half * (NQ // 4), (half + 1) * (NQ // 4))
                ptmp = qpool.tile([128, NQ // 4], F32, tag="ptmp", name="ptmp")
                nc.sync.dma_start(out=qs_sb[k][:, sl],
                                  in_=qT[k * 128:(k + 1) * 128, sl])
                nc.sync.dma_start(out=ptmp[:], in_=pT[k * 128:(k + 1) * 128, sl])
                vec.tensor_tensor(out=qs_sb[k][:, sl], in0=qs_sb[k][:, sl],
                                  in1=ptmp[:], op=OP.add)
        nc.sync.dma_start(out=ref_sb[:], in_=refT[:])

        # ---- stage C: coords / masks / gather indices, all supertiles ----
        for g in range(NSUP):
            q0 = g * TQ
            psc = ps_c.tile([128, 160], F32, tag="psc", name="psc")
            for s in range(NSUB):
                qsl = slice(q0 + s * 128, q0 + (s + 1) * 128)
                nc.tensor.matmul(psc[:, s * 40:(s + 1) * 40],
                                 qs_sb[0][:, qsl], wcat_sb[0][:],
                                 start=True, stop=False)
                nc.tensor.matmul(psc[:, s * 40:(s + 1) * 40],
                                 qs_sb[1][:, qsl], wcat_sb[1][:],
                                 start=False, stop=False)
                nc.tensor.matmul(psc[:, s * 40:(s + 1) * 40],
                                 ref_sb[:, qsl], refrhs_sb[:],
                                 start=False, stop=True)
            coords = qpool.tile([128, 160], F32, tag="coords", name="coords")
            act.activation(out=coords[:], in_=psc[:], func=ACT.Copy)

            co = coords[:].rearrange("p (s r) -> p s r", s=NSUB)
            pix = co[:, :, 0:30]                        # (s, hp*ax)
            logit = co[:, :, 30:40]                     # (s, hp)

            # softmax over P
            exlog = qpool.tile([128, NSUB * 10], F32, tag="exlog", name="exlog")
            act.activation(out=exlog[:], in_=logit, func=ACT.Exp)
            ex4 = exlog[:].rearrange("p (s h q) -> p s h q", s=NSUB, h=2)
            sums = qpool.tile([128, NSUB * 2], F32, tag="sums", name="sums")
            vec.tensor_reduce(out=sums[:].rearrange("p (s h) -> p s h", s=NSUB),
                              in_=ex4, axis=AX.X, op=OP.add)
            rsum = qpool.tile([128, NSUB * 2], F32, tag="rsum", name="rsum")
            vec.reciprocal(out=rsum[:], in_=sums[:])
            aw = qpool.tile([128, NSUB * 10], F32, tag="aw", name="aw")
            vec.tensor_tensor(
                out=aw[:].rearrange("p (sh q) -> p sh q", q=5),
                in0=exlog[:].rearrange("p (sh q) -> p sh q", q=5),
                in1=rsum[:].unsqueeze(2).to_broadcast([128, NSUB * 2, 5]),
                op=OP.mult)

            # corner math on the 30 pixel rows
            NPX = NSUB * 30
            # flo = round(pix - 0.5) via the 2^23 magic add (== floor except
            # exactly-integer pix, where the phantom corner gets zero weight)
            flo = qpool.tile([128, NPX], F32, tag="flo", name="flo")
            vec.tensor_scalar(out=flo[:].rearrange("p (s r) -> p s r", s=NSUB),
                              in0=pix, scalar1=8388607.5, scalar2=8388608.0,
                              op0=OP.add, op1=OP.subtract)
            fl3 = flo[:].rearrange("p (s r) -> p s r", s=NSUB)
            frac = qpool.tile([128, NPX], F32, tag="frac", name="frac")
            vec.tensor_tensor(out=frac[:].rearrange("p (s r) -> p s r", s=NSUB),
                              in0=pix, in1=fl3, op=OP.subtract)
            fr3 = frac[:].rearrange("p (s r) -> p s r", s=NSUB)
            # gcorn: (s, hp, ax, dx) — clamped corner coords (biased +64)
            gcorn = qpool.tile([128, NPX * 2], F32, tag="gcorn", name="gcorn")
            gc4 = gcorn[:].rearrange("p (s r d) -> p s r d", s=NSUB, d=2)
            vec.tensor_scalar(out=gc4[:, :, :, 0], in0=fl3,
                              scalar1=64.0, scalar2=111.0, op0=OP.max, op1=OP.min)
            g1m = qpool.tile([128, NPX], F32, tag="g1m", name="g1m")
            vec.tensor_scalar(out=g1m[:], in0=flo[:],
                              scalar1=63.0, scalar2=110.0, op0=OP.max, op1=OP.min)
            vec.tensor_scalar(out=gc4[:, :, :, 1], in0=g1m[:]
                              .rearrange("p (s r) -> p s r", s=NSUB),
                              scalar1=1.0, scalar2=None, op0=OP.add)
            # validity via clip-equality
            v0 = qpool.tile([128, NPX], F32, tag="v0", name="v0")
            vec.tensor_tensor(out=v0[:].rearrange("p (s r) -> p s r", s=NSUB),
                              in0=gc4[:, :, :, 0], in1=fl3, op=OP.is_equal)
            v1 = qpool.tile([128, NPX], F32, tag="v1", name="v1")
            vec.tensor_tensor(out=v1[:], in0=g1m[:], in1=flo[:], op=OP.is_equal)
            # corner weights (x/y rows used; z rows ignored later)
            om = qpool.tile([128, NPX], F32, tag="om", name="om")
            vec.tensor_scalar(out=om[:], in0=frac[:], scalar1=-1.0, scalar2=1.0,
                              op0=OP.mult, op1=OP.add)
            wcorn = qpool.tile([128, NPX * 2], F32, tag="wcorn", name="wcorn")
            wc4 = wcorn[:].rearrange("p (s r d) -> p s r d", s=NSUB, d=2)
            vec.tensor_tensor(out=wc4[:, :, :, 0],
                              in0=om[:].rearrange("p (s r) -> p s r", s=NSUB),
                              in1=v0[:].rearrange("p (s r) -> p s r", s=NSUB),
                              op=OP.mult)
            vec.tensor_tensor(out=wc4[:, :, :, 1],
                              in0=fr3,
                              in1=v1[:].rearrange("p (s r) -> p s r", s=NSUB),
                              op=OP.mult)

            # z window: rzq = floor((gz-64)/4) in [0,11]; W-slot hat weights
            gc6 = gcorn[:].rearrange("p (s hp a d) -> p s hp a d",
                                     s=NSUB, hp=10, a=3)
            gz = gc6[:, :, :, 2, 0]
            pz = co[:, :, 0:30].rearrange("p s (hp a) -> p s hp a", a=3)[:, :, :, 2]
            rzq = qpool.tile([128, NSUB * 10], F32, tag="rzq", name="rzq")
            tq = qpool.tile([128, NSUB * 10], F32, tag="tq", name="tq")
            vec.tensor_scalar(out=tq[:].rearrange("p (s h) -> p s h", s=NSUB),
                              in0=gz, scalar1=0.25, scalar2=16.375,
                              op0=OP.mult, op1=OP.subtract)
            vec.tensor_scalar(out=rzq[:], in0=tq[:],
                              scalar1=8388624.0, scalar2=8388624.0,
                              op0=OP.add, op1=OP.subtract)
            # d0 = (4*rzq + 64) - pz ; dk = d0 + k
            zb4 = qpool.tile([128, NSUB * 10], F32, tag="zb4", name="zb4")
            vec.tensor_scalar(out=zb4[:], in0=rzq[:], scalar1=4.0, scalar2=64.0,
                              op0=OP.mult, op1=OP.add)
            d0 = qpool.tile([128, NSUB * 10], F32, tag="d0", name="d0")
            vec.tensor_tensor(out=d0[:].rearrange("p (s h) -> p s h", s=NSUB),
                              in0=zb4[:].rearrange("p (s h) -> p s h", s=NSUB),
                              in1=pz, op=OP.subtract)
            dk = qpool.tile([128, NSUB * 10 * W], F32, tag="dk", name="dk")
            vec.tensor_tensor(
                out=dk[:].rearrange("p (sh k) -> p sh k", k=W),
                in0=d0[:].unsqueeze(2).to_broadcast([128, NSUB * 10, W]),
                in1=zoff_sb[:].unsqueeze(1).to_broadcast([128, NSUB * 10, W]),
                op=OP.add)
            adk = qpool.tile([128, NSUB * 10 * W], F32, tag="adk", name="adk")
            act.activation(out=adk[:], in_=dk[:], func=ACT.Abs)
            hat = qpool.tile([128, NSUB * 10 * W], F32, tag="hat", name="hat")
            act.activation(out=hat[:], in_=adk[:], func=ACT.Relu,
                           scale=-1.0, bias=1.0)
            # upper bound: slot z 4*rzq+64+k <= 111  <=>  dk <= 111 - pz
            ub = qpool.tile([128, NSUB * 10], F32, tag="ub", name="ub")
            vec.tensor_scalar(out=ub[:].rearrange("p (s h) -> p s h", s=NSUB),
                              in0=pz, scalar1=-1.0, scalar2=111.0,
                              op0=OP.mult, op1=OP.add)
            vub = qpool.tile([128, NSUB * 10 * W], F32, tag="vub", name="vub")
            vec.tensor_tensor(
                out=vub[:].rearrange("p (sh k) -> p sh k", k=W),
                in0=dk[:].rearrange("p (sh k) -> p sh k", k=W),
                in1=ub[:].unsqueeze(2).to_broadcast([128, NSUB * 10, W]),
                op=OP.is_le)
            wz = qpool.tile([128, NSUB * 10 * W], F32, tag="wz", name="wz")
            vec.tensor_tensor(out=wz[:], in0=hat[:], in1=vub[:], op=OP.mult)

            # slot weights: block bx = min(gx0, 110); slot s holds x = bx+s.
            # When gx0 == 111 (x >= 47) the corner-0 weight moves to slot 1.
            wc6 = wcorn[:].rearrange("p (s hp a d) -> p s hp a d",
                                     s=NSUB, hp=10, a=3)
            ws = qpool.tile([128, NSUB * 40], F32, tag="ws", name="ws")
            ws4 = ws[:].rearrange("p (s hp a d) -> p s hp a d", s=NSUB, hp=10, a=2)
            hi = qpool.tile([128, NSUB * 20], F32, tag="hi", name="hi")
            hi3 = hi[:].rearrange("p (s hp a) -> p s hp a", s=NSUB, hp=10)
            hit = qpool.tile([128, NSUB * 20], F32, tag="hit", name="hit")
            hit3 = hit[:].rearrange("p (s hp a) -> p s hp a", s=NSUB, hp=10)
            # hi = (gc0 == 111): corner-0 weight moves to slot 1.
            # lo = (g1m == 63): corner-1 weight (position 64) moves to slot 0.
            vec.tensor_scalar(out=hi3, in0=gc6[:, :, :, 0:2, 0],
                              scalar1=111.0, scalar2=None, op0=OP.is_equal)
            vec.tensor_tensor(out=hit3, in0=wc6[:, :, :, 0:2, 0], in1=hi3,
                              op=OP.mult)
            lo = qpool.tile([128, NSUB * 20], F32, tag="lo", name="lo")
            lo3 = lo[:].rearrange("p (s hp a) -> p s hp a", s=NSUB, hp=10)
            lot = qpool.tile([128, NSUB * 20], F32, tag="lot", name="lot")
            lot3 = lot[:].rearrange("p (s hp a) -> p s hp a", s=NSUB, hp=10)
            g1m3 = g1m[:].rearrange("p (s hp a) -> p s hp a", s=NSUB, hp=10)
            vec.tensor_scalar(out=lo3, in0=g1m3[:, :, :, 0:2],
                              scalar1=63.0, scalar2=None, op0=OP.is_equal)
            vec.tensor_tensor(out=lot3, in0=wc6[:, :, :, 0:2, 1], in1=lo3,
                              op=OP.mult)
            vec.tensor_tensor(out=ws4[:, :, :, :, 0],
                              in0=wc6[:, :, :, 0:2, 0], in1=hit3,
                              op=OP.subtract)
            vec.tensor_tensor(out=ws4[:, :, :, :, 0],
                              in0=ws4[:, :, :, :, 0], in1=lot3, op=OP.add)
            vec.tensor_tensor(out=ws4[:, :, :, :, 1],
                              in0=wc6[:, :, :, 0:2, 1], in1=hit3, op=OP.add)
            vec.tensor_tensor(out=ws4[:, :, :, :, 1],
                              in0=ws4[:, :, :, :, 1], in1=lot3, op=OP.subtract)
            wxs = ws4[:, :, :, 0, :]                    # (s, hp, xslot)
            wys = ws4[:, :, :, 1, :]                    # (s, hp, yslot)

            # mask: m[(s hp), zb, xp, yp, z4] = aw*wxs*wys*wz
            m1 = qpool.tile([128, NSUB * 20], F32, tag="m1", name="m1")
            vec.tensor_tensor(
                out=m1[:].rearrange("p (sh xp) -> p sh xp", xp=2),
                in0=aw[:].unsqueeze(2).to_broadcast([128, NSUB * 10, 2]),
                in1=wxs.rearrange("p s hp xp -> p (s hp) xp"), op=OP.mult)
            m2 = qpool.tile([128, NSUB * 40], F32, tag="m2", name="m2")
            vec.tensor_tensor(
                out=m2[:].rearrange("p (sh xp yp) -> p sh xp yp", xp=2, yp=2),
                in0=m1[:].rearrange("p (sh xp) -> p sh xp", xp=2)
                    .unsqueeze(3).to_broadcast([128, NSUB * 10, 2, 2]),
                in1=wys.rearrange("p s hp yp -> p (s hp) yp")
                    .unsqueeze(2).to_broadcast([128, NSUB * 10, 2, 2]),
                op=OP.mult)
            # mtmp layout (sh, zb, z4, xy); all three APs are 3-free-dim
            mtmp = qpool.tile([128, NSUB * 10 * 32], F32, tag="mtmp",
                              name="mtmp")
            vec.tensor_tensor(
                out=mtmp[:].rearrange("p (sh zz xy) -> p sh zz xy",
                                      zz=8, xy=4),
                in0=m2[:].rearrange("p (sh xy) -> p sh xy", xy=4)
                    .unsqueeze(2).to_broadcast([128, NSUB * 10, 8, 4]),
                in1=wz[:].rearrange("p (sh zz) -> p sh zz", zz=8)
                    .unsqueeze(3).to_broadcast([128, NSUB * 10, 8, 4]),
                op=OP.mult)
            # permute (zb, z4, xy) -> (zb, xy, z4); (sh, zb) merges both sides
            mv = maskb_all[:, g * MCOL:(g + 1) * MCOL]
            vec.tensor_copy(
                out=mv.rearrange("p (szb xy z4) -> p szb xy z4", xy=4, z4=4),
                in_=mtmp[:].rearrange("p (szb z4 xy) -> p szb xy z4",
                                      z4=4, xy=4))

            # gather row indices: 576*(bx-64) + 12*(by-64) + rzq
            bx = qpool.tile([128, NSUB * 10], F32, tag="bx", name="bx")
            vec.tensor_scalar(out=bx[:].rearrange("p (s h) -> p s h", s=NSUB),
                              in0=gc6[:, :, :, 0, 0], scalar1=110.0,
                              scalar2=576.0, op0=OP.min, op1=OP.mult)
            by = qpool.tile([128, NSUB * 10], F32, tag="by", name="by")
            vec.tensor_scalar(out=by[:].rearrange("p (s h) -> p s h", s=NSUB),
                              in0=gc6[:, :, :, 1, 0], scalar1=110.0,
                              scalar2=12.0, op0=OP.min, op1=OP.mult)
            t4 = qpool.tile([128, NSUB * 10], F32, tag="t4", name="t4")
            vec.tensor_tensor(out=t4[:], in0=bx[:], in1=by[:], op=OP.add)
            idxf = qpool.tile([128, NSUB * 10], F32, tag="idxf", name="idxf")
            vec.tensor_scalar(out=idxf[:], in0=t4[:],
                              scalar1=37632.0, scalar2=None, op0=OP.subtract)
            vec.tensor_tensor(out=idxf[:], in0=idxf[:], in1=rzq[:], op=OP.add)
            idx16 = qpool.tile([128, NSUB * 10], I16, tag="idx16", name="idx16")
            vec.tensor_copy(out=idx16[:], in_=idxf[:])

            # idx fold into the gather's 16-partition-wrapped layout:
            # bounce through DRAM with contiguous 320B packets, then a DVE
            # column permute (s2-major -> s2-interleaved).
            nc.sync.dma_start(out=idxscr[g * 128:(g + 1) * 128, :], in_=idx16[:])
            idxr = gpool.tile([128, ICOL], I16, tag="idxr", name="idxr")
            scr = idxscr[g * 128:(g + 1) * 128, :]
            for gg in range(8):
                # idxr[16*gg + p16, s2*40 + f] = idx16[s2*16 + p16, f]
                src = bass.AP(scr.tensor, scr.offset,
                              [[40, 16], [16 * 40, 8], [1, 40]])
                nc.sync.dma_start(
                    out=idxr[gg * 16:(gg + 1) * 16, :]
                        .rearrange("p (s2 f) -> p s2 f", s2=8),
                    in_=src)
            # permute cols: (s2, subhl, r) -> (subhl, r, s2)
            vec.tensor_copy(
                out=idxw_all[:, g * ICOL:(g + 1) * ICOL]
                    .rearrange("p (sh r s2) -> p sh r s2", sh=8, r=5),
                in_=idxr[:].rearrange("p (s2 sh r) -> p sh r s2", s2=8, sh=8))

        # ---- stage V: value projection (bf16) ----
        # Zero the 12 units at block (46, 47) whose (xp=1, yp=1) slots no
        # write pass covers but the z-window overrun can read, plus the back
        # guard.
        zpad = const.tile([16, 512], VDT, tag="zpad", name="zpad")
        vec.memset(zpad[:], 0.0)
        for hl in range(2):
            zb46 = (46 * 576 + 47 * 12)
            nc.sync.dma_start(
                out=bass.AP(vexp[:].tensor,
                            hl * VHEAD + (G0 + zb46) * 512 + 256 + 128,
                            [[512, 12], [1, 128]]),
                in_=zpad[0:12, 0:128])
            nc.sync.dma_start(
                out=bass.AP(vexp[:].tensor, hl * VHEAD + (G0 + NUNIT) * 512,
                            [[512, 16], [1, 512]]),
                in_=zpad[:])
        for vg in range(NVSUP // NR):          # flush groups of NR supertiles
            vb = [vpool.tile([128, NR * 256], VDT, tag=f"vb{hl}", name=f"vb{hl}")
                  for hl in range(2)]
            for i in range(NR):
                vt = vg * NR + i
                vin = [vpool.tile([128, VSUP], VDT, tag=f"vin{k}", name=f"vin{k}")
                       for k in range(2)]
                for k in range(2):
                    nc.sync.dma_start(
                        out=vin[k][:],
                        in_=value_in[k * 128:(k + 1) * 128,
                                     vt * VSUP:(vt + 1) * VSUP])
                psv = ps_v.tile([128, 512], F32, tag="psv", name="psv")
                for s in range(8):
                    lhs0 = vin[0][:].rearrange("p (v e) -> p e v", e=8)[:, s, :]
                    lhs1 = vin[1][:].rearrange("p (v e) -> p e v", e=8)[:, s, :]
                    nc.tensor.matmul(psv[:, s * 64:(s + 1) * 64], lhs0,
                                     wval_sb[0][:], start=True, stop=False)
                    nc.tensor.matmul(psv[:, s * 64:(s + 1) * 64], lhs1,
                                     wval_sb[1][:], start=False, stop=True)
                # split heads, add bias, pack (blk, c, v4) per 4-voxel block
                for hl in range(2):
                    src = psv[:].rearrange("p (blk v4 hc) -> p blk hc v4",
                                           blk=2, v4=4)[:, :, hl * 32:(hl + 1) * 32, :]
                    bv = bval_sb[:, hl * 32:(hl + 1) * 32] \
                        .unsqueeze(1).unsqueeze(3).to_broadcast([128, 2, 32, 4])
                    vec.tensor_tensor(
                        out=vb[hl][:, i * 256:(i + 1) * 256]
                            .rearrange("p (blk c v4) -> p blk c v4", blk=2, c=32),
                        in0=src, in1=bv, op=OP.add)
            # flush NR supertiles (NR*1024 voxels) per head: 4 quadrant
            # passes; pass (xp, yp) lands vblock at unit vblock-xp*576-yp*12
            # slot (xp, yp). Runs are 256B ((c, z4) per vblock).
            for hl in range(2):
                for xp in range(2):
                    for yp in range(2):
                        base = (hl * VHEAD
                                + (G0 + vg * NR * 256 - xp * 576 - yp * 12)
                                * 512 + xp * 256 + yp * 128)
                        for blk in range(2):
                            dst = bass.AP(vexp[:].tensor, base + blk * 512,
                                          [[1024, 128], [256 * 512, NR],
                                           [1, 128]])
                            src = vb[hl][:] \
                                .rearrange("p (i blk x) -> p i blk x",
                                           i=NR, blk=2)[:, :, blk, :]
                            eng = nc.sync if (xp * 2 + yp) % 2 == 0 \
                                else nc.scalar
                            eng.dma_start(out=dst, in_=src)

        # ---- stage G: gather + weighted reduce per (supertile, subtile, head) ----
        for g in range(NSUP):
            q0 = g * TQ
            S = qpool.tile([128, NSUB * 64], F32, tag="S", name="S")
            for s in range(NSUB):
                for hl in range(2):
                    G = gpool.tile([128, 5 * 1024], VDT, tag="G", name="G")
                    in_g = bass.AP(vexp[:].tensor, hl * VHEAD + G0 * 512,
                                   [[512, NUNIT], [1, 1024]])
                    nc.gpsimd.dma_gather(
                        out_ap=G[:].rearrange("p (i e) -> p i e", i=5),
                        in_ap=in_g,
                        idxs_ap=idxw_all[:, (g * 8 + s * 2 + hl) * 40:
                                         (g * 8 + s * 2 + hl + 1) * 40],
                        num_idxs=NIDX, num_idxs_reg=NIDX,
                        elem_size=1024, elem_step=512,
                        single_packet=False,
                        queue_num=(s * 2 + hl) % 4)
                    # P = G * mask in place; G row = (zb, xp, yp, c, z4),
                    # mask cols (pt, zb, xy, z4) bcast over c
                    moff = g * MCOL + (s * 2 + hl) * 160
                    mg = maskb_all[:, moff:moff + 160] \
                        .rearrange("p (rb z4) -> p rb z4", z4=4) \
                        .unsqueeze(2).to_broadcast([128, 40, 32, 4])
                    gv = G[:].rearrange("p (rb c z4) -> p rb c z4",
                                        rb=40, c=32)
                    vec.tensor_tensor(out=gv, in0=gv, in1=mg, op=OP.mult)
                    # single fused reduce over (rb, z4), keeping c
                    vec.tensor_reduce(
                        out=S[:, s * 64 + hl * 32:s * 64 + hl * 32 + 32],
                        in_=G[:].rearrange("p (rb c z4) -> p c rb z4",
                                           rb=40, c=32),
                        axis=AX.XY, op=OP.add)

            # transpose S [128, 64] -> [64, 128] per subtile
            for s in range(NSUB):
                pst = ps_t.tile([64, 128], F32, tag="pst", name="pst")
                nc.tensor.transpose(pst[:], S[:, s * 64:(s + 1) * 64], ident[:])
                act.activation(out=st_sb[:, q0 + s * 128:q0 + (s + 1) * 128],
                               in_=pst[:], func=ACT.Copy)

            # GEMM2 for this supertile: outT = wout^T @ ST (overlaps the
            # next supertile's gathers)
            for mc in range(2):
                ps2 = ps_c.tile([128, 512], F32, tag="ps2", name="ps2")
                nc.tensor.matmul(ps2[:],
                                 wout_sb[:, mc * 128:(mc + 1) * 128],
                                 st_sb[:, q0:q0 + TQ],
                                 start=True, stop=True)
                ob = opool.tile([128, 512], F32, tag="ob", name="ob")
                vec.tensor_copy(out=ob[:], in_=ps2[:])
                nc.sync.dma_start(
                    out=outp[mc * 128:(mc + 1) * 128, q0:q0 + TQ],
                    in_=ob[:])

    nc.compile()
    return nc


def _prep_core_inputs(inputs, b, j):
    import ml_dtypes
    q = np.ascontiguousarray(inputs["query"][b].T, np.float32)
    p = np.ascontiguousarray(inputs["pos"][b].T, np.float32)
    r = np.concatenate([inputs["reference_points"][b].T,
                        np.ones((1, NQ), np.float32)]).astype(np.float32)
    r = np.ascontiguousarray(r)
    value = np.ascontiguousarray(
        inputs["value"][b].reshape(C, NVOX)).astype(ml_dtypes.bfloat16)

    W_off, b_off = inputs["W_off"], inputs["b_off"]
    W_attn, b_attn = inputs["W_attn"], inputs["b_attn"]
    heads = [2 * j, 2 * j + 1]
    rows, biases, refr = [], [], []
    for h in heads:
        for pp in range(P):
            for ax in range(3):
                rows.append(W_off[(h * P + pp) * 3 + ax])
                biases.append(b_off[(h * P + pp) * 3 + ax] - 0.5 + 64.0)
                e = np.zeros(3, np.float32)
                e[ax] = GRID
                refr.append(e)
    for h in heads:
        for pp in range(P):
            rows.append(W_attn[h * P + pp])
            biases.append(b_attn[h * P + pp])
            refr.append(np.zeros(3, np.float32))
    wcat = np.ascontiguousarray(np.stack(rows).T, np.float32)       # (256, 40)
    ref_rhs = np.concatenate(
        [np.stack(refr).T, np.asarray(biases, np.float32)[None, :]])
    ref_rhs = np.ascontiguousarray(ref_rhs, np.float32)             # (4, 40)

    wval = np.ascontiguousarray(
        inputs["W_val"][64 * j:64 * j + 64].T).astype(ml_dtypes.bfloat16)
    bval = np.ascontiguousarray(
        np.repeat(inputs["b_val"][64 * j:64 * j + 64][None, :], 128, axis=0),
        np.float32)
    wout = np.ascontiguousarray(inputs["W_out"][:, 64 * j:64 * j + 64].T,
                                np.float32)
    zoffs = np.repeat(np.arange(W, dtype=np.float32)[None, :], 128, axis=0)
    return {
        "value_in": value, "qT": q, "pT": p, "refT": r,
        "wcat": wcat, "ref_rhs": ref_rhs,
        "wval": wval, "bval": bval, "wout": wout, "zoff": zoffs,
    }


def get_nc():
    global _NC_CACHE
    if _NC_CACHE is None:
        _NC_CACHE = build_nc()
    return _NC_CACHE


def kernel(**inputs):
    from concourse.bass_utils import run_bass_kernel_spmd

    inputs = {k: np.asarray(v) for k, v in inputs.items()}
    nc = get_nc()
    in_maps = [_prep_core_inputs(inputs, core // 4, core % 4) for core in range(8)]
    res = run_bass_kernel_spmd(nc, in_maps, list(range(8)))
    bs = inputs["query"].shape[0]
    out = np.zeros((bs, NQ, C), np.float32)
    for core in range(8):
        out[core // 4] += res.results[core]["outp"].T
    out += inputs["b_out"][None, None, :].astype(np.float32)
    return out


# revision 34
# speedup vs baseline: 3.7903x; 1.0736x over previous
"""Trainium2 Bass kernel for 3D deformable attention (8 NeuronCores).

Sharding: core i handles batch b = i // 4 and head-pair j = i % 4
(heads 2j, 2j+1, i.e. value/out channels [64j, 64j+64)).

Per-core device pipeline (emission order = coords first so DVE mask work
overlaps the value-projection DMA/PE stage):
  C. per query-supertile (512 q): coords = qs^T @ Wcat^T + [48*ref | b] (PE);
     softmax over 5 points, trilinear corner weights remapped to block
     slots (lo/hi edge clamps), z-window "hat" weights, combined mask
     m[pt, zb, xp, yp, z4] = aw*wxs*wys*wz (DVE, bf16); int16 gather unit
     indices; idx fold into dma_gather's 16-partition-wrapped layout via a
     contiguous DRAM bounce (80B packets) + DVE column permute.
  V. value projection v = W_val[64j:64j+64] @ value[b] in bf16 (PE,
     voxel-stationary); packed per 4-voxel block as (c, v4) and written
     4x quad-shifted into vexp: unit (x0, y0, zbu) holds the (xp, yp)
     quadrants of the 4-z slab, so ONE 2KB gather row = the full
     2x2x(8z) trilinear neighborhood of a sample. The 4 write passes
     stay 256B-contiguous because unit(vblock) = vblock - xp*576 - yp*12
     is linear in vblock.
  G. per (query-subtile, head): one dma_gather of 640 rows x 2KB;
     P = G * mask in place (DVE bf16); one fused AX.XY reduce over
     (pt,zb,xy,z4) keeping c -> S[q, 64].
  O. PE transpose of S, then out^T = Wout^T @ S^T, DMA out.
Host combines: out[b] = sum_j outp_j^T + b_out.
"""
import numpy as np

import concourse.bass as bass
import concourse.mybir as mybir
from concourse import bacc, tile
from concourse.masks import make_identity
from contextlib import ExitStack

F32 = mybir.dt.float32
F32R = mybir.dt.float32r
I16 = mybir.dt.int16
AX = mybir.AxisListType
OP = mybir.AluOpType
ACT = mybir.ActivationFunctionType

H, P = 8, 5
NQ, C, GRID = 4096, 256, 48
NVOX = GRID ** 3            # 110592
NSUB = 4                    # query subtiles (of 128) per supertile
TQ = 128 * NSUB             # 512
NSUP = NQ // TQ             # 8
VSUP = 1024                 # voxels per value-proj supertile
NVSUP = NVOX // VSUP        # 108
NR = 4                      # value supertiles per DRAM flush

VDT = mybir.dt.bfloat16
W = 8                       # z-window voxels per gathered row
# vexp: quad-interleaved expanded volume. Per head, blocks (x0, y0) of
# 12 z-units; unit = (xp, yp, c, z4) = 512 els (4-z slab x 4 quadrants x
# 32ch). A gather row = 2 consecutive units = the full 2x2x(8z) trilinear
# neighborhood of one sample point. unit(vblock) = vblock - xp*576 - yp*12
# is linear in vblock, so the 4 write passes keep 256B-contiguous runs.
NUNIT = 48 * 48 * 12        # 27648 addressable units per head
G0 = 588                    # front guard units (absorbs xp/yp shifts)
G1 = 16                     # back guard units (zeroed; z-window overrun)
VHEAD = (G0 + NUNIT + G1) * 512
NIDX = 5 * 128              # rows per (subtile, head) gather
MCOL = NSUB * 40 * W        # mask columns per supertile (1280)
ICOL = NSUB * 2 * 40        # idx columns per supertile (320)

_NC_CACHE = None


def build_nc():
    nc = bacc.Bacc("TRN2", target_bir_lowering=False, debug=False, num_devices=8,
                   num_swdge_queues=4)

    value_in = nc.dram_tensor("value_in", [C, NVOX], VDT, kind="ExternalInput")
    qT = nc.dram_tensor("qT", [C, NQ], F32, kind="ExternalInput")
    pT = nc.dram_tensor("pT", [C, NQ], F32, kind="ExternalInput")
    refT = nc.dram_tensor("refT", [4, NQ], F32, kind="ExternalInput")
    wcat = nc.dram_tensor("wcat", [C, 40], F32, kind="ExternalInput")
    ref_rhs = nc.dram_tensor("ref_rhs", [4, 40], F32, kind="ExternalInput")
    wval = nc.dram_tensor("wval", [C, 64], VDT, kind="ExternalInput")
    bval = nc.dram_tensor("bval", [128, 64], F32, kind="ExternalInput")
    wout = nc.dram_tensor("wout", [64, C], F32, kind="ExternalInput")
    zoff = nc.dram_tensor("zoff", [128, W], F32, kind="ExternalInput")
    outp = nc.dram_tensor("outp", [C, NQ], F32, kind="ExternalOutput")
    vexp = nc.dram_tensor("vexp", [2 * VHEAD], VDT)
    idxscr = nc.dram_tensor("idxscr", [NSUP * 128, 40], I16)

    vec = nc.vector
    act = nc.scalar

    with tile.TileContext(nc) as tc, ExitStack() as ctx:
        const = ctx.enter_context(tc.tile_pool(name="const", bufs=1))
        vpool = ctx.enter_context(tc.tile_pool(name="vpool", bufs=3))
        qpool = ctx.enter_context(tc.tile_pool(name="qpool", bufs=2))
        gpool = ctx.enter_context(tc.tile_pool(name="gpool", bufs=4))
        ipool = ctx.enter_context(tc.tile_pool(name="ipool", bufs=2))
        opool = ctx.enter_context(tc.tile_pool(name="opool", bufs=1))
        ps_v = ctx.enter_context(tc.tile_pool(name="ps_v", bufs=2, space="PSUM"))
        ps_c = ctx.enter_context(tc.tile_pool(name="ps_c", bufs=2, space="PSUM"))
        ps_t = ctx.enter_context(tc.tile_pool(name="ps_t", bufs=2, space="PSUM"))

        # ---- constants into SBUF ----
        wcat_sb = [const.tile([128, 40], F32, tag=f"wcat{k}", name=f"wcat{k}")
                   for k in range(2)]
        for k in range(2):
            nc.sync.dma_start(out=wcat_sb[k][:], in_=wcat[k * 128:(k + 1) * 128, :])
        refrhs_sb = const.tile([4, 40], F32, tag="refrhs", name="refrhs")
        nc.sync.dma_start(out=refrhs_sb[:], in_=ref_rhs[:])
        wval_sb = [const.tile([128, 64], VDT, tag=f"wval{k}", name=f"wval{k}")
                   for k in range(2)]
        for k in range(2):
            nc.sync.dma_start(out=wval_sb[k][:], in_=wval[k * 128:(k + 1) * 128, :])
        bval_sb = const.tile([128, 64], F32, tag="bval", name="bval")
        nc.sync.dma_start(out=bval_sb[:], in_=bval[:])
        wout_sb = const.tile([64, C], F32, tag="wout", name="wout")
        nc.sync.dma_start(out=wout_sb[:], in_=wout[:])
        zoff_sb = const.tile([128, W], F32, tag="zoff", name="zoff")
        nc.sync.dma_start(out=zoff_sb[:], in_=zoff[:])
        ident = const.tile([128, 128], F32, tag="ident", name="ident")
        make_identity(nc, ident[:])

        # persistent big buffers
        qs_sb = [const.tile([128, NQ], F32, tag=f"qs{k}", name=f"qs{k}")
                 for k in range(2)]
        ref_sb = const.tile([4, NQ], F32, tag="refq", name="refq")
        st_sb = const.tile([64, NQ], F32, tag="st", name="st")
        maskb_all = const.tile([128, NSUP * MCOL], VDT, tag="maskb", name="maskb")
        idxw_all = const.tile([128, NSUP * ICOL], I16, tag="idxw", name="idxw")

        # ---- stage Q0: load q, pos, ref; qs = q + p ----
        for k in range(2):
            for half in range(4):
                sl = slice(half * (NQ // 4), (half + 1) * (NQ // 4))
                ptmp = qpool.tile([128, NQ // 4], F32, tag="ptmp", name="ptmp")
                nc.sync.dma_start(out=qs_sb[k][:, sl],
                                  in_=qT[k * 128:(k + 1) * 128, sl])
                nc.sync.dma_start(out=ptmp[:], in_=pT[k * 128:(k + 1) * 128, sl])
                vec.tensor_tensor(out=qs_sb[k][:, sl], in0=qs_sb[k][:, sl],
                                  in1=ptmp[:], op=OP.add)
        nc.sync.dma_start(out=ref_sb[:], in_=refT[:])

        # ---- stage C: coords / masks / gather indices, all supertiles ----
        for g in range(NSUP):
            q0 = g * TQ
            psc = ps_c.tile([128, 160], F32, tag="psc", name="psc")
            for s in range(NSUB):
                qsl = slice(q0 + s * 128, q0 + (s + 1) * 128)
                nc.tensor.matmul(psc[:, s * 40:(s + 1) * 40],
                                 qs_sb[0][:, qsl], wcat_sb[0][:],
                                 start=True, stop=False)
                nc.tensor.matmul(psc[:, s * 40:(s + 1) * 40],
                                 qs_sb[1][:, qsl], wcat_sb[1][:],
                                 start=False, stop=False)
                nc.tensor.matmul(psc[:, s * 40:(s + 1) * 40],
                                 ref_sb[:, qsl], refrhs_sb[:],
                                 start=False, stop=True)
            coords = qpool.tile([128, 160], F32, tag="coords", name="coords")
            act.activation(out=coords[:], in_=psc[:], func=ACT.Copy)

            co = coords[:].rearrange("p (s r) -> p s r", s=NSUB)
            pix = co[:, :, 0:30]                        # (s, hp*ax)
            logit = co[:, :, 30:40]                     # (s, hp)

            # softmax over P
            exlog = qpool.tile([128, NSUB * 10], F32, tag="exlog", name="exlog")
            act.activation(out=exlog[:], in_=logit, func=ACT.Exp)
            ex4 = exlog[:].rearrange("p (s h q) -> p s h q", s=NSUB, h=2)
            sums = qpool.tile([128, NSUB * 2], F32, tag="sums", name="sums")
            vec.tensor_reduce(out=sums[:].rearrange("p (s h) -> p s h", s=NSUB),
                              in_=ex4, axis=AX.X, op=OP.add)
            rsum = qpool.tile([128, NSUB * 2], F32, tag="rsum", name="rsum")
            vec.reciprocal(out=rsum[:], in_=sums[:])
            aw = qpool.tile([128, NSUB * 10], F32, tag="aw", name="aw")
            vec.tensor_tensor(
                out=aw[:].rearrange("p (sh q) -> p sh q", q=5),
                in0=exlog[:].rearrange("p (sh q) -> p sh q", q=5),
                in1=rsum[:].unsqueeze(2).to_broadcast([128, NSUB * 2, 5]),
                op=OP.mult)

            # corner math on the 30 pixel rows
            NPX = NSUB * 30
            # flo = round(pix - 0.5) via the 2^23 magic add (== floor except
            # exactly-integer pix, where the phantom corner gets zero weight)
            flo = qpool.tile([128, NPX], F32, tag="flo", name="flo")
            vec.tensor_scalar(out=flo[:].rearrange("p (s r) -> p s r", s=NSUB),
                              in0=pix, scalar1=8388607.5, scalar2=8388608.0,
                              op0=OP.add, op1=OP.subtract)
            fl3 = flo[:].rearrange("p (s r) -> p s r", s=NSUB)
            frac = qpool.tile([128, NPX], F32, tag="frac", name="frac")
            vec.tensor_tensor(out=frac[:].rearrange("p (s r) -> p s r", s=NSUB),
                              in0=pix, in1=fl3, op=OP.subtract)
            fr3 = frac[:].rearrange("p (s r) -> p s r", s=NSUB)
            # gcorn: (s, hp, ax, dx) — clamped corner coords (biased +64)
            gcorn = qpool.tile([128, NPX * 2], F32, tag="gcorn", name="gcorn")
            gc4 = gcorn[:].rearrange("p (s r d) -> p s r d", s=NSUB, d=2)
            vec.tensor_scalar(out=gc4[:, :, :, 0], in0=fl3,
                              scalar1=64.0, scalar2=111.0, op0=OP.max, op1=OP.min)
            g1m = qpool.tile([128, NPX], F32, tag="g1m", name="g1m")
            vec.tensor_scalar(out=g1m[:], in0=flo[:],
                              scalar1=63.0, scalar2=110.0, op0=OP.max, op1=OP.min)
            vec.tensor_scalar(out=gc4[:, :, :, 1], in0=g1m[:]
                              .rearrange("p (s r) -> p s r", s=NSUB),
                              scalar1=1.0, scalar2=None, op0=OP.add)
            # validity via clip-equality
            v0 = qpool.tile([128, NPX], F32, tag="v0", name="v0")
            vec.tensor_tensor(out=v0[:].rearrange("p (s r) -> p s r", s=NSUB),
                              in0=gc4[:, :, :, 0], in1=fl3, op=OP.is_equal)
            v1 = qpool.tile([128, NPX], F32, tag="v1", name="v1")
            vec.tensor_tensor(out=v1[:], in0=g1m[:], in1=flo[:], op=OP.is_equal)
            # corner weights (x/y rows used; z rows ignored later)
            om = qpool.tile([128, NPX], F32, tag="om", name="om")
            vec.tensor_scalar(out=om[:], in0=frac[:], scalar1=-1.0, scalar2=1.0,
                              op0=OP.mult, op1=OP.add)
            wcorn = qpool.tile([128, NPX * 2], F32, tag="wcorn", name="wcorn")
            wc4 = wcorn[:].rearrange("p (s r d) -> p s r d", s=NSUB, d=2)
            vec.tensor_tensor(out=wc4[:, :, :, 0],
                              in0=om[:].rearrange("p (s r) -> p s r", s=NSUB),
                              in1=v0[:].rearrange("p (s r) -> p s r", s=NSUB),
                              op=OP.mult)
            vec.tensor_tensor(out=wc4[:, :, :, 1],
                              in0=fr3,
                              in1=v1[:].rearrange("p (s r) -> p s r", s=NSUB),
                              op=OP.mult)

            # z window: rzq = floor((gz-64)/4) in [0,11]; W-slot hat weights
            gc6 = gcorn[:].rearrange("p (s hp a d) -> p s hp a d",
                                     s=NSUB, hp=10, a=3)
            gz = gc6[:, :, :, 2, 0]
            pz = co[:, :, 0:30].rearrange("p s (hp a) -> p s hp a", a=3)[:, :, :, 2]
            rzq = qpool.tile([128, NSUB * 10], F32, tag="rzq", name="rzq")
            tq = qpool.tile([128, NSUB * 10], F32, tag="tq", name="tq")
            vec.tensor_scalar(out=tq[:].rearrange("p (s h) -> p s h", s=NSUB),
                              in0=gz, scalar1=0.25, scalar2=16.375,
                              op0=OP.mult, op1=OP.subtract)
            vec.tensor_scalar(out=rzq[:], in0=tq[:],
                              scalar1=8388624.0, scalar2=8388624.0,
                              op0=OP.add, op1=OP.subtract)
            # d0 = (4*rzq + 64) - pz ; dk = d0 + k
            zb4 = qpool.tile([128, NSUB * 10], F32, tag="zb4", name="zb4")
            vec.tensor_scalar(out=zb4[:], in0=rzq[:], scalar1=4.0, scalar2=64.0,
                              op0=OP.mult, op1=OP.add)
            d0 = qpool.tile([128, NSUB * 10], F32, tag="d0", name="d0")
            vec.tensor_tensor(out=d0[:].rearrange("p (s h) -> p s h", s=NSUB),
                              in0=zb4[:].rearrange("p (s h) -> p s h", s=NSUB),
                              in1=pz, op=OP.subtract)
            dk = qpool.tile([128, NSUB * 10 * W], F32, tag="dk", name="dk")
            vec.tensor_tensor(
                out=dk[:].rearrange("p (sh k) -> p sh k", k=W),
                in0=d0[:].unsqueeze(2).to_broadcast([128, NSUB * 10, W]),
                in1=zoff_sb[:].unsqueeze(1).to_broadcast([128, NSUB * 10, W]),
                op=OP.add)
            adk = qpool.tile([128, NSUB * 10 * W], F32, tag="adk", name="adk")
            act.activation(out=adk[:], in_=dk[:], func=ACT.Abs)
            hat = qpool.tile([128, NSUB * 10 * W], F32, tag="hat", name="hat")
            act.activation(out=hat[:], in_=adk[:], func=ACT.Relu,
                           scale=-1.0, bias=1.0)
            # upper bound: slot z 4*rzq+64+k <= 111  <=>  dk <= 111 - pz
            ub = qpool.tile([128, NSUB * 10], F32, tag="ub", name="ub")
            vec.tensor_scalar(out=ub[:].rearrange("p (s h) -> p s h", s=NSUB),
                              in0=pz, scalar1=-1.0, scalar2=111.0,
                              op0=OP.mult, op1=OP.add)
            vub = qpool.tile([128, NSUB * 10 * W], F32, tag="vub", name="vub")
            vec.tensor_tensor(
                out=vub[:].rearrange("p (sh k) -> p sh k", k=W),
                in0=dk[:].rearrange("p (sh k) -> p sh k", k=W),
                in1=ub[:].unsqueeze(2).to_broadcast([128, NSUB * 10, W]),
                op=OP.is_le)
            wz = qpool.tile([128, NSUB * 10 * W], F32, tag="wz", name="wz")
            vec.tensor_tensor(out=wz[:], in0=hat[:], in1=vub[:], op=OP.mult)

            # slot weights: block bx = min(gx0, 110); slot s holds x = bx+s.
            # When gx0 == 111 (x >= 47) the corner-0 weight moves to slot 1.
            wc6 = wcorn[:].rearrange("p (s hp a d) -> p s hp a d",
                                     s=NSUB, hp=10, a=3)
            ws = qpool.tile([128, NSUB * 40], F32, tag="ws", name="ws")
            ws4 = ws[:].rearrange("p (s hp a d) -> p s hp a d", s=NSUB, hp=10, a=2)
            hi = qpool.tile([128, NSUB * 20], F32, tag="hi", name="hi")
            hi3 = hi[:].rearrange("p (s hp a) -> p s hp a", s=NSUB, hp=10)
            hit = qpool.tile([128, NSUB * 20], F32, tag="hit", name="hit")
            hit3 = hit[:].rearrange("p (s hp a) -> p s hp a", s=NSUB, hp=10)
            # hi = (gc0 == 111): corner-0 weight moves to slot 1.
            # lo = (g1m == 63): corner-1 weight (position 64) moves to slot 0.
            vec.tensor_scalar(out=hi3, in0=gc6[:, :, :, 0:2, 0],
                              scalar1=111.0, scalar2=None, op0=OP.is_equal)
            vec.tensor_tensor(out=hit3, in0=wc6[:, :, :, 0:2, 0], in1=hi3,
                              op=OP.mult)
            lo = qpool.tile([128, NSUB * 20], F32, tag="lo", name="lo")
            lo3 = lo[:].rearrange("p (s hp a) -> p s hp a", s=NSUB, hp=10)
            lot = qpool.tile([128, NSUB * 20], F32, tag="lot", name="lot")
            lot3 = lot[:].rearrange("p (s hp a) -> p s hp a", s=NSUB, hp=10)
            g1m3 = g1m[:].rearrange("p (s hp a) -> p s hp a", s=NSUB, hp=10)
            vec.tensor_scalar(out=lo3, in0=g1m3[:, :, :, 0:2],
                              scalar1=63.0, scalar2=None, op0=OP.is_equal)
            vec.tensor_tensor(out=lot3, in0=wc6[:, :, :, 0:2, 1], in1=lo3,
                              op=OP.mult)
            vec.tensor_tensor(out=ws4[:, :, :, :, 0],
                              in0=wc6[:, :, :, 0:2, 0], in1=hit3,
                              op=OP.subtract)
            vec.tensor_tensor(out=ws4[:, :, :, :, 0],
                              in0=ws4[:, :, :, :, 0], in1=lot3, op=OP.add)
            vec.tensor_tensor(out=ws4[:, :, :, :, 1],
                              in0=wc6[:, :, :, 0:2, 1], in1=hit3, op=OP.add)
            vec.tensor_tensor(out=ws4[:, :, :, :, 1],
                              in0=ws4[:, :, :, :, 1], in1=lot3, op=OP.subtract)
            wxs = ws4[:, :, :, 0, :]                    # (s, hp, xslot)
            wys = ws4[:, :, :, 1, :]                    # (s, hp, yslot)

            # mask: m[(s hp), zb, xp, yp, z4] = aw*wxs*wys*wz
            m1 = qpool.tile([128, NSUB * 20], F32, tag="m1", name="m1")
            vec.tensor_tensor(
                out=m1[:].rearrange("p (sh xp) -> p sh xp", xp=2),
                in0=aw[:].unsqueeze(2).to_broadcast([128, NSUB * 10, 2]),
                in1=wxs.rearrange("p s hp xp -> p (s hp) xp"), op=OP.mult)
            m2 = qpool.tile([128, NSUB * 40], F32, tag="m2", name="m2")
            vec.tensor_tensor(
                out=m2[:].rearrange("p (sh xp yp) -> p sh xp yp", xp=2, yp=2),
                in0=m1[:].rearrange("p (sh xp) -> p sh xp", xp=2)
                    .unsqueeze(3).to_broadcast([128, NSUB * 10, 2, 2]),
                in1=wys.rearrange("p s hp yp -> p (s hp) yp")
                    .unsqueeze(2).to_broadcast([128, NSUB * 10, 2, 2]),
                op=OP.mult)
            # mtmp layout (sh, zb, z4, xy); all three APs are 3-free-dim
            mtmp = qpool.tile([128, NSUB * 10 * 32], VDT, tag="mtmp",
                              name="mtmp")
            vec.tensor_tensor(
                out=mtmp[:].rearrange("p (sh zz xy) -> p sh zz xy",
                                      zz=8, xy=4),
                in0=m2[:].rearrange("p (sh xy) -> p sh xy", xy=4)
                    .unsqueeze(2).to_broadcast([128, NSUB * 10, 8, 4]),
                in1=wz[:].rearrange("p (sh zz) -> p sh zz", zz=8)
                    .unsqueeze(3).to_broadcast([128, NSUB * 10, 8, 4]),
                op=OP.mult)
            # permute (zb, z4, xy) -> (zb, xy, z4); (sh, zb) merges both sides
            mv = maskb_all[:, g * MCOL:(g + 1) * MCOL]
            vec.tensor_copy(
                out=mv.rearrange("p (szb xy z4) -> p szb xy z4", xy=4, z4=4),
                in_=mtmp[:].rearrange("p (szb z4 xy) -> p szb xy z4",
                                      z4=4, xy=4))

            # gather row indices: 576*(bx-64) + 12*(by-64) + rzq
            bx = qpool.tile([128, NSUB * 10], F32, tag="bx", name="bx")
            vec.tensor_scalar(out=bx[:].rearrange("p (s h) -> p s h", s=NSUB),
                              in0=gc6[:, :, :, 0, 0], scalar1=110.0,
                              scalar2=576.0, op0=OP.min, op1=OP.mult)
            by = qpool.tile([128, NSUB * 10], F32, tag="by", name="by")
            vec.tensor_scalar(out=by[:].rearrange("p (s h) -> p s h", s=NSUB),
                              in0=gc6[:, :, :, 1, 0], scalar1=110.0,
                              scalar2=12.0, op0=OP.min, op1=OP.mult)
            t4 = qpool.tile([128, NSUB * 10], F32, tag="t4", name="t4")
            vec.tensor_tensor(out=t4[:], in0=bx[:], in1=by[:], op=OP.add)
            idxf = qpool.tile([128, NSUB * 10], F32, tag="idxf", name="idxf")
            vec.tensor_scalar(out=idxf[:], in0=t4[:],
                              scalar1=37632.0, scalar2=None, op0=OP.subtract)
            vec.tensor_tensor(out=idxf[:], in0=idxf[:], in1=rzq[:], op=OP.add)
            idx16 = qpool.tile([128, NSUB * 10], I16, tag="idx16", name="idx16")
            vec.tensor_copy(out=idx16[:], in_=idxf[:])

            # idx fold into the gather's 16-partition-wrapped layout:
            # bounce through DRAM with contiguous 320B packets, then a DVE
            # column permute (s2-major -> s2-interleaved).
            nc.sync.dma_start(out=idxscr[g * 128:(g + 1) * 128, :], in_=idx16[:])
            idxr = ipool.tile([128, ICOL], I16, tag="idxr", name="idxr")
            scr = idxscr[g * 128:(g + 1) * 128, :]
            for gg in range(8):
                # idxr[16*gg + p16, s2*40 + f] = idx16[s2*16 + p16, f]
                src = bass.AP(scr.tensor, scr.offset,
                              [[40, 16], [16 * 40, 8], [1, 40]])
                nc.sync.dma_start(
                    out=idxr[gg * 16:(gg + 1) * 16, :]
                        .rearrange("p (s2 f) -> p s2 f", s2=8),
                    in_=src)
            # permute cols: (s2, subhl, r) -> (subhl, r, s2)
            vec.tensor_copy(
                out=idxw_all[:, g * ICOL:(g + 1) * ICOL]
                    .rearrange("p (sh r s2) -> p sh r s2", sh=8, r=5),
                in_=idxr[:].rearrange("p (s2 sh r) -> p sh r s2", s2=8, sh=8))

        # ---- stage V: value projection (bf16) ----
        # Zero the 12 units at block (46, 47) whose (xp=1, yp=1) slots no
        # write pass covers but the z-window overrun can read, plus the back
        # guard.
        zpad = const.tile([16, 512], VDT, tag="zpad", name="zpad")
        vec.memset(zpad[:], 0.0)
        for hl in range(2):
            zb46 = (46 * 576 + 47 * 12)
            nc.sync.dma_start(
                out=bass.AP(vexp[:].tensor,
                            hl * VHEAD + (G0 + zb46) * 512 + 256 + 128,
                            [[512, 12], [1, 128]]),
                in_=zpad[0:12, 0:128])
            nc.sync.dma_start(
                out=bass.AP(vexp[:].tensor, hl * VHEAD + (G0 + NUNIT) * 512,
                            [[512, 16], [1, 512]]),
                in_=zpad[:])
        for vg in range(NVSUP // NR):          # flush groups of NR supertiles
            vb = [vpool.tile([128, NR * 256], VDT, tag=f"vb{hl}", name=f"vb{hl}")
                  for hl in range(2)]
            for i in range(NR):
                vt = vg * NR + i
                vin = [vpool.tile([128, VSUP], VDT, tag=f"vin{k}", name=f"vin{k}")
                       for k in range(2)]
                for k in range(2):
                    nc.sync.dma_start(
                        out=vin[k][:],
                        in_=value_in[k * 128:(k + 1) * 128,
                                     vt * VSUP:(vt + 1) * VSUP])
                psv = ps_v.tile([128, 512], F32, tag="psv", name="psv")
                for s in range(8):
                    lhs0 = vin[0][:].rearrange("p (v e) -> p e v", e=8)[:, s, :]
                    lhs1 = vin[1][:].rearrange("p (v e) -> p e v", e=8)[:, s, :]
                    nc.tensor.matmul(psv[:, s * 64:(s + 1) * 64], lhs0,
                                     wval_sb[0][:], start=True, stop=False)
                    nc.tensor.matmul(psv[:, s * 64:(s + 1) * 64], lhs1,
                                     wval_sb[1][:], start=False, stop=True)
                # split heads, add bias, pack (blk, c, v4) per 4-voxel block
                for hl in range(2):
                    src = psv[:].rearrange("p (blk v4 hc) -> p blk hc v4",
                                           blk=2, v4=4)[:, :, hl * 32:(hl + 1) * 32, :]
                    bv = bval_sb[:, hl * 32:(hl + 1) * 32] \
                        .unsqueeze(1).unsqueeze(3).to_broadcast([128, 2, 32, 4])
                    vec.tensor_tensor(
                        out=vb[hl][:, i * 256:(i + 1) * 256]
                            .rearrange("p (blk c v4) -> p blk c v4", blk=2, c=32),
                        in0=src, in1=bv, op=OP.add)
            # flush NR supertiles (NR*1024 voxels) per head: 4 quadrant
            # passes; pass (xp, yp) lands vblock at unit vblock-xp*576-yp*12
            # slot (xp, yp). Runs are 256B ((c, z4) per vblock).
            for hl in range(2):
                for xp in range(2):
                    for yp in range(2):
                        base = (hl * VHEAD
                                + (G0 + vg * NR * 256 - xp * 576 - yp * 12)
                                * 512 + xp * 256 + yp * 128)
                        for blk in range(2):
                            dst = bass.AP(vexp[:].tensor, base + blk * 512,
                                          [[1024, 128], [256 * 512, NR],
                                           [1, 128]])
                            src = vb[hl][:] \
                                .rearrange("p (i blk x) -> p i blk x",
                                           i=NR, blk=2)[:, :, blk, :]
                            eng = nc.sync if (xp * 2 + yp) % 2 == 0 \
                                else nc.scalar
                            eng.dma_start(out=dst, in_=src)

        # ---- stage G: gather + weighted reduce per (supertile, subtile, head) ----
        for g in range(NSUP):
            q0 = g * TQ
            S = qpool.tile([128, NSUB * 64], F32, tag="S", name="S")
            for s in range(NSUB):
                for hl in range(2):
                    G = gpool.tile([128, 5 * 1024], VDT, tag="G", name="G")
                    in_g = bass.AP(vexp[:].tensor, hl * VHEAD + G0 * 512,
                                   [[512, NUNIT], [1, 1024]])
                    nc.gpsimd.dma_gather(
                        out_ap=G[:].rearrange("p (i e) -> p i e", i=5),
                        in_ap=in_g,
                        idxs_ap=idxw_all[:, (g * 8 + s * 2 + hl) * 40:
                                         (g * 8 + s * 2 + hl + 1) * 40],
                        num_idxs=NIDX, num_idxs_reg=NIDX,
                        elem_size=1024, elem_step=512,
                        single_packet=False,
                        queue_num=(s * 2 + hl) % 4)
                    # P = G * mask in place; G row = (zb, xp, yp, c, z4),
                    # mask cols (pt, zb, xy, z4) bcast over c
                    moff = g * MCOL + (s * 2 + hl) * 160
                    mg = maskb_all[:, moff:moff + 160] \
                        .rearrange("p (rb z4) -> p rb z4", z4=4) \
                        .unsqueeze(2).to_broadcast([128, 40, 32, 4])
                    gv = G[:].rearrange("p (rb c z4) -> p rb c z4",
                                        rb=40, c=32)
                    vec.tensor_tensor(out=gv, in0=gv, in1=mg, op=OP.mult)
                    # single fused reduce over (rb, z4), keeping c
                    vec.tensor_reduce(
                        out=S[:, s * 64 + hl * 32:s * 64 + hl * 32 + 32],
                        in_=G[:].rearrange("p (rb c z4) -> p c rb z4",
                                           rb=40, c=32),
                        axis=AX.XY, op=OP.add)

            # transpose S [128, 64] -> [64, 128] per subtile
            for s in range(NSUB):
                pst = ps_t.tile([64, 128], F32, tag="pst", name="pst")
                nc.tensor.transpose(pst[:], S[:, s * 64:(s + 1) * 64], ident[:])
                act.activation(out=st_sb[:, q0 + s * 128:q0 + (s + 1) * 128],
                               in_=pst[:], func=ACT.Copy)

            # GEMM2 for this supertile: outT = wout^T @ ST (overlaps the
            # next supertile's gathers)
            for mc in range(2):
                ps2 = ps_c.tile([128, 512], F32, tag="ps2", name="ps2")
                nc.tensor.matmul(ps2[:],
                                 wout_sb[:, mc * 128:(mc + 1) * 128],
                                 st_sb[:, q0:q0 + TQ],
                                 start=True, stop=True)
                ob = opool.tile([128, 512], F32, tag="ob", name="ob")
                vec.tensor_copy(out=ob[:], in_=ps2[:])
                nc.sync.dma_start(
                    out=outp[mc * 128:(mc + 1) * 128, q0:q0 + TQ],
                    in_=ob[:])

    nc.compile()
    return nc


def _prep_core_inputs(inputs, b, j):
    import ml_dtypes
    q = np.ascontiguousarray(inputs["query"][b].T, np.float32)
    p = np.ascontiguousarray(inputs["pos"][b].T, np.float32)
    r = np.concatenate([inputs["reference_points"][b].T,
                        np.ones((1, NQ), np.float32)]).astype(np.float32)
    r = np.ascontiguousarray(r)
    value = np.ascontiguousarray(
        inputs["value"][b].reshape(C, NVOX)).astype(ml_dtypes.bfloat16)

    W_off, b_off = inputs["W_off"], inputs["b_off"]
    W_attn, b_attn = inputs["W_attn"], inputs["b_attn"]
    heads = [2 * j, 2 * j + 1]
    rows, biases, refr = [], [], []
    for h in heads:
        for pp in range(P):
            for ax in range(3):
                rows.append(W_off[(h * P + pp) * 3 + ax])
                biases.append(b_off[(h * P + pp) * 3 + ax] - 0.5 + 64.0)
                e = np.zeros(3, np.float32)
                e[ax] = GRID
                refr.append(e)
    for h in heads:
        for pp in range(P):
            rows.append(W_attn[h * P + pp])
            biases.append(b_attn[h * P + pp])
            refr.append(np.zeros(3, np.float32))
    wcat = np.ascontiguousarray(np.stack(rows).T, np.float32)       # (256, 40)
    ref_rhs = np.concatenate(
        [np.stack(refr).T, np.asarray(biases, np.float32)[None, :]])
    ref_rhs = np.ascontiguousarray(ref_rhs, np.float32)             # (4, 40)

    wval = np.ascontiguousarray(
        inputs["W_val"][64 * j:64 * j + 64].T).astype(ml_dtypes.bfloat16)
    bval = np.ascontiguousarray(
        np.repeat(inputs["b_val"][64 * j:64 * j + 64][None, :], 128, axis=0),
        np.float32)
    wout = np.ascontiguousarray(inputs["W_out"][:, 64 * j:64 * j + 64].T,
                                np.float32)
    zoffs = np.repeat(np.arange(W, dtype=np.float32)[None, :], 128, axis=0)
    return {
        "value_in": value, "qT": q, "pT": p, "refT": r,
        "wcat": wcat, "ref_rhs": ref_rhs,
        "wval": wval, "bval": bval, "wout": wout, "zoff": zoffs,
    }


def get_nc():
    global _NC_CACHE
    if _NC_CACHE is None:
        _NC_CACHE = build_nc()
    return _NC_CACHE


def kernel(**inputs):
    from concourse.bass_utils import run_bass_kernel_spmd

    inputs = {k: np.asarray(v) for k, v in inputs.items()}
    nc = get_nc()
    in_maps = [_prep_core_inputs(inputs, core // 4, core % 4) for core in range(8)]
    res = run_bass_kernel_spmd(nc, in_maps, list(range(8)))
    bs = inputs["query"].shape[0]
    out = np.zeros((bs, NQ, C), np.float32)
    for core in range(8):
        out[core // 4] += res.results[core]["outp"].T
    out += inputs["b_out"][None, None, :].astype(np.float32)
    return out


# revision 35
# speedup vs baseline: 3.8268x; 1.0096x over previous
"""Trainium2 Bass kernel for 3D deformable attention (8 NeuronCores).

Sharding: core i handles batch b = i // 4 and head-pair j = i % 4
(heads 2j, 2j+1, i.e. value/out channels [64j, 64j+64)).

Per-core device pipeline (emission order = coords first so DVE mask work
overlaps the value-projection DMA/PE stage):
  C. per query-supertile (512 q): coords = qs^T @ Wcat^T + [48*ref | b] (PE);
     softmax over 5 points, trilinear corner weights remapped to block
     slots (lo/hi edge clamps), z-window "hat" weights, combined mask
     m[pt, zb, xp, yp, z4] = aw*wxs*wys*wz (DVE, bf16); int16 gather unit
     indices; idx fold into dma_gather's 16-partition-wrapped layout via a
     contiguous DRAM bounce (80B packets) + DVE column permute.
  V. value projection v = W_val[64j:64j+64] @ value[b] in bf16 (PE,
     voxel-stationary); packed per 4-voxel block as (c, v4) and written
     4x quad-shifted into vexp: unit (x0, y0, zbu) holds the (xp, yp)
     quadrants of the 4-z slab, so ONE 2KB gather row = the full
     2x2x(8z) trilinear neighborhood of a sample. The 4 write passes
     stay 256B-contiguous because unit(vblock) = vblock - xp*576 - yp*12
     is linear in vblock.
  G. per (query-subtile, head): one dma_gather of 640 rows x 2KB;
     P = G * mask in place (DVE bf16); one fused AX.XY reduce over
     (pt,zb,xy,z4) keeping c -> S[q, 64].
  O. PE transpose of S, then out^T = Wout^T @ S^T, DMA out.
Host combines: out[b] = sum_j outp_j^T + b_out.
"""
import numpy as np

import concourse.bass as bass
import concourse.mybir as mybir
from concourse import bacc, tile
from concourse.masks import make_identity
from contextlib import ExitStack

F32 = mybir.dt.float32
F32R = mybir.dt.float32r
I16 = mybir.dt.int16
AX = mybir.AxisListType
OP = mybir.AluOpType
ACT = mybir.ActivationFunctionType

H, P = 8, 5
NQ, C, GRID = 4096, 256, 48
NVOX = GRID ** 3            # 110592
NSUB = 4                    # query subtiles (of 128) per supertile
TQ = 128 * NSUB             # 512
NSUP = NQ // TQ             # 8
VSUP = 1024                 # voxels per value-proj supertile
NVSUP = NVOX // VSUP        # 108
NR = 4                      # value supertiles per DRAM flush

VDT = mybir.dt.bfloat16
W = 8                       # z-window voxels per gathered row
# vexp: quad-interleaved expanded volume. Per head, blocks (x0, y0) of
# 12 z-units; unit = (xp, yp, c, z4) = 512 els (4-z slab x 4 quadrants x
# 32ch). A gather row = 2 consecutive units = the full 2x2x(8z) trilinear
# neighborhood of one sample point. unit(vblock) = vblock - xp*576 - yp*12
# is linear in vblock, so the 4 write passes keep 256B-contiguous runs.
NUNIT = 48 * 48 * 12        # 27648 addressable units per head
G0 = 588                    # front guard units (absorbs xp/yp shifts)
G1 = 16                     # back guard units (zeroed; z-window overrun)
VHEAD = (G0 + NUNIT + G1) * 512
NIDX = 5 * 128              # rows per (subtile, head) gather
MCOL = NSUB * 40 * W        # mask columns per supertile (1280)
ICOL = NSUB * 2 * 40        # idx columns per supertile (320)

_NC_CACHE = None


def build_nc():
    nc = bacc.Bacc("TRN2", target_bir_lowering=False, debug=False, num_devices=8,
                   num_swdge_queues=4)

    value_in = nc.dram_tensor("value_in", [C, NVOX], VDT, kind="ExternalInput")
    qT = nc.dram_tensor("qT", [C, NQ], F32, kind="ExternalInput")
    pT = nc.dram_tensor("pT", [C, NQ], F32, kind="ExternalInput")
    refT = nc.dram_tensor("refT", [4, NQ], F32, kind="ExternalInput")
    wcat = nc.dram_tensor("wcat", [C, 40], F32, kind="ExternalInput")
    ref_rhs = nc.dram_tensor("ref_rhs", [4, 40], F32, kind="ExternalInput")
    wval = nc.dram_tensor("wval", [C, 64], VDT, kind="ExternalInput")
    bval = nc.dram_tensor("bval", [128, 64], F32, kind="ExternalInput")
    wout = nc.dram_tensor("wout", [64, C], F32, kind="ExternalInput")
    zoff = nc.dram_tensor("zoff", [128, W], F32, kind="ExternalInput")
    outp = nc.dram_tensor("outp", [C, NQ], F32, kind="ExternalOutput")
    vexp = nc.dram_tensor("vexp", [2 * VHEAD], VDT)
    idxscr = nc.dram_tensor("idxscr", [NSUP * 128, 40], I16)

    vec = nc.vector
    act = nc.scalar

    with tile.TileContext(nc) as tc, ExitStack() as ctx:
        const = ctx.enter_context(tc.tile_pool(name="const", bufs=1))
        vpool = ctx.enter_context(tc.tile_pool(name="vpool", bufs=3))
        qpool = ctx.enter_context(tc.tile_pool(name="qpool", bufs=2))
        gpool = ctx.enter_context(tc.tile_pool(name="gpool", bufs=4))
        ipool = ctx.enter_context(tc.tile_pool(name="ipool", bufs=2))
        opool = ctx.enter_context(tc.tile_pool(name="opool", bufs=1))
        ps_v = ctx.enter_context(tc.tile_pool(name="ps_v", bufs=4, space="PSUM"))
        ps_c = ctx.enter_context(tc.tile_pool(name="ps_c", bufs=1, space="PSUM"))
        ps_t = ctx.enter_context(tc.tile_pool(name="ps_t", bufs=1, space="PSUM"))

        # ---- constants into SBUF ----
        wcat_sb = [const.tile([128, 40], F32, tag=f"wcat{k}", name=f"wcat{k}")
                   for k in range(2)]
        for k in range(2):
            nc.sync.dma_start(out=wcat_sb[k][:], in_=wcat[k * 128:(k + 1) * 128, :])
        refrhs_sb = const.tile([4, 40], F32, tag="refrhs", name="refrhs")
        nc.sync.dma_start(out=refrhs_sb[:], in_=ref_rhs[:])
        wval_sb = [const.tile([128, 64], VDT, tag=f"wval{k}", name=f"wval{k}")
                   for k in range(2)]
        for k in range(2):
            nc.sync.dma_start(out=wval_sb[k][:], in_=wval[k * 128:(k + 1) * 128, :])
        bval_sb = const.tile([128, 64], F32, tag="bval", name="bval")
        nc.sync.dma_start(out=bval_sb[:], in_=bval[:])
        wout_sb = const.tile([64, C], F32, tag="wout", name="wout")
        nc.sync.dma_start(out=wout_sb[:], in_=wout[:])
        zoff_sb = const.tile([128, W], F32, tag="zoff", name="zoff")
        nc.sync.dma_start(out=zoff_sb[:], in_=zoff[:])
        ident = const.tile([128, 128], F32, tag="ident", name="ident")
        make_identity(nc, ident[:])

        # persistent big buffers
        qs_sb = [const.tile([128, NQ], F32, tag=f"qs{k}", name=f"qs{k}")
                 for k in range(2)]
        ref_sb = const.tile([4, NQ], F32, tag="refq", name="refq")
        st_sb = const.tile([64, NQ], F32, tag="st", name="st")
        maskb_all = const.tile([128, NSUP * MCOL], VDT, tag="maskb", name="maskb")
        idxw_all = const.tile([128, NSUP * ICOL], I16, tag="idxw", name="idxw")

        # ---- stage Q0: load q, pos, ref; qs = q + p ----
        for k in range(2):
            for half in range(4):
                sl = slice(half * (NQ // 4), (half + 1) * (NQ // 4))
                ptmp = qpool.tile([128, NQ // 4], F32, tag="ptmp", name="ptmp")
                nc.sync.dma_start(out=qs_sb[k][:, sl],
                                  in_=qT[k * 128:(k + 1) * 128, sl])
                nc.sync.dma_start(out=ptmp[:], in_=pT[k * 128:(k + 1) * 128, sl])
                vec.tensor_tensor(out=qs_sb[k][:, sl], in0=qs_sb[k][:, sl],
                                  in1=ptmp[:], op=OP.add)
        nc.sync.dma_start(out=ref_sb[:], in_=refT[:])

        # ---- stage C: coords / masks / gather indices, all supertiles ----
        for g in range(NSUP):
            q0 = g * TQ
            psc = ps_c.tile([128, 160], F32, tag="psc", name="psc")
            for s in range(NSUB):
                qsl = slice(q0 + s * 128, q0 + (s + 1) * 128)
                nc.tensor.matmul(psc[:, s * 40:(s + 1) * 40],
                                 qs_sb[0][:, qsl], wcat_sb[0][:],
                                 start=True, stop=False)
                nc.tensor.matmul(psc[:, s * 40:(s + 1) * 40],
                                 qs_sb[1][:, qsl], wcat_sb[1][:],
                                 start=False, stop=False)
                nc.tensor.matmul(psc[:, s * 40:(s + 1) * 40],
                                 ref_sb[:, qsl], refrhs_sb[:],
                                 start=False, stop=True)
            coords = qpool.tile([128, 160], F32, tag="coords", name="coords")
            act.activation(out=coords[:], in_=psc[:], func=ACT.Copy)

            co = coords[:].rearrange("p (s r) -> p s r", s=NSUB)
            pix = co[:, :, 0:30]                        # (s, hp*ax)
            logit = co[:, :, 30:40]                     # (s, hp)

            # softmax over P
            exlog = qpool.tile([128, NSUB * 10], F32, tag="exlog", name="exlog")
            act.activation(out=exlog[:], in_=logit, func=ACT.Exp)
            ex4 = exlog[:].rearrange("p (s h q) -> p s h q", s=NSUB, h=2)
            sums = qpool.tile([128, NSUB * 2], F32, tag="sums", name="sums")
            vec.tensor_reduce(out=sums[:].rearrange("p (s h) -> p s h", s=NSUB),
                              in_=ex4, axis=AX.X, op=OP.add)
            rsum = qpool.tile([128, NSUB * 2], F32, tag="rsum", name="rsum")
            vec.reciprocal(out=rsum[:], in_=sums[:])
            aw = qpool.tile([128, NSUB * 10], F32, tag="aw", name="aw")
            vec.tensor_tensor(
                out=aw[:].rearrange("p (sh q) -> p sh q", q=5),
                in0=exlog[:].rearrange("p (sh q) -> p sh q", q=5),
                in1=rsum[:].unsqueeze(2).to_broadcast([128, NSUB * 2, 5]),
                op=OP.mult)

            # corner math on the 30 pixel rows
            NPX = NSUB * 30
            # flo = round(pix - 0.5) via the 2^23 magic add (== floor except
            # exactly-integer pix, where the phantom corner gets zero weight)
            flo = qpool.tile([128, NPX], F32, tag="flo", name="flo")
            vec.tensor_scalar(out=flo[:].rearrange("p (s r) -> p s r", s=NSUB),
                              in0=pix, scalar1=8388607.5, scalar2=8388608.0,
                              op0=OP.add, op1=OP.subtract)
            fl3 = flo[:].rearrange("p (s r) -> p s r", s=NSUB)
            frac = qpool.tile([128, NPX], F32, tag="frac", name="frac")
            vec.tensor_tensor(out=frac[:].rearrange("p (s r) -> p s r", s=NSUB),
                              in0=pix, in1=fl3, op=OP.subtract)
            fr3 = frac[:].rearrange("p (s r) -> p s r", s=NSUB)
            # gcorn: (s, hp, ax, dx) — clamped corner coords (biased +64)
            gcorn = qpool.tile([128, NPX * 2], F32, tag="gcorn", name="gcorn")
            gc4 = gcorn[:].rearrange("p (s r d) -> p s r d", s=NSUB, d=2)
            vec.tensor_scalar(out=gc4[:, :, :, 0], in0=fl3,
                              scalar1=64.0, scalar2=111.0, op0=OP.max, op1=OP.min)
            g1m = qpool.tile([128, NPX], F32, tag="g1m", name="g1m")
            vec.tensor_scalar(out=g1m[:], in0=flo[:],
                              scalar1=63.0, scalar2=110.0, op0=OP.max, op1=OP.min)
            vec.tensor_scalar(out=gc4[:, :, :, 1], in0=g1m[:]
                              .rearrange("p (s r) -> p s r", s=NSUB),
                              scalar1=1.0, scalar2=None, op0=OP.add)
            # validity via clip-equality
            v0 = qpool.tile([128, NPX], F32, tag="v0", name="v0")
            vec.tensor_tensor(out=v0[:].rearrange("p (s r) -> p s r", s=NSUB),
                              in0=gc4[:, :, :, 0], in1=fl3, op=OP.is_equal)
            v1 = qpool.tile([128, NPX], F32, tag="v1", name="v1")
            vec.tensor_tensor(out=v1[:], in0=g1m[:], in1=flo[:], op=OP.is_equal)
            # corner weights (x/y rows used; z rows ignored later)
            om = qpool.tile([128, NPX], F32, tag="om", name="om")
            vec.tensor_scalar(out=om[:], in0=frac[:], scalar1=-1.0, scalar2=1.0,
                              op0=OP.mult, op1=OP.add)
            wcorn = qpool.tile([128, NPX * 2], F32, tag="wcorn", name="wcorn")
            wc4 = wcorn[:].rearrange("p (s r d) -> p s r d", s=NSUB, d=2)
            vec.tensor_tensor(out=wc4[:, :, :, 0],
                              in0=om[:].rearrange("p (s r) -> p s r", s=NSUB),
                              in1=v0[:].rearrange("p (s r) -> p s r", s=NSUB),
                              op=OP.mult)
            vec.tensor_tensor(out=wc4[:, :, :, 1],
                              in0=fr3,
                              in1=v1[:].rearrange("p (s r) -> p s r", s=NSUB),
                              op=OP.mult)

            # z window: rzq = floor((gz-64)/4) in [0,11]; W-slot hat weights
            gc6 = gcorn[:].rearrange("p (s hp a d) -> p s hp a d",
                                     s=NSUB, hp=10, a=3)
            gz = gc6[:, :, :, 2, 0]
            pz = co[:, :, 0:30].rearrange("p s (hp a) -> p s hp a", a=3)[:, :, :, 2]
            rzq = qpool.tile([128, NSUB * 10], F32, tag="rzq", name="rzq")
            tq = qpool.tile([128, NSUB * 10], F32, tag="tq", name="tq")
            vec.tensor_scalar(out=tq[:].rearrange("p (s h) -> p s h", s=NSUB),
                              in0=gz, scalar1=0.25, scalar2=16.375,
                              op0=OP.mult, op1=OP.subtract)
            vec.tensor_scalar(out=rzq[:], in0=tq[:],
                              scalar1=8388624.0, scalar2=8388624.0,
                              op0=OP.add, op1=OP.subtract)
            # d0 = (4*rzq + 64) - pz ; dk = d0 + k
            zb4 = qpool.tile([128, NSUB * 10], F32, tag="zb4", name="zb4")
            vec.tensor_scalar(out=zb4[:], in0=rzq[:], scalar1=4.0, scalar2=64.0,
                              op0=OP.mult, op1=OP.add)
            d0 = qpool.tile([128, NSUB * 10], F32, tag="d0", name="d0")
            vec.tensor_tensor(out=d0[:].rearrange("p (s h) -> p s h", s=NSUB),
                              in0=zb4[:].rearrange("p (s h) -> p s h", s=NSUB),
                              in1=pz, op=OP.subtract)
            dk = qpool.tile([128, NSUB * 10 * W], F32, tag="dk", name="dk")
            vec.tensor_tensor(
                out=dk[:].rearrange("p (sh k) -> p sh k", k=W),
                in0=d0[:].unsqueeze(2).to_broadcast([128, NSUB * 10, W]),
                in1=zoff_sb[:].unsqueeze(1).to_broadcast([128, NSUB * 10, W]),
                op=OP.add)
            adk = qpool.tile([128, NSUB * 10 * W], F32, tag="adk", name="adk")
            act.activation(out=adk[:], in_=dk[:], func=ACT.Abs)
            hat = qpool.tile([128, NSUB * 10 * W], F32, tag="hat", name="hat")
            act.activation(out=hat[:], in_=adk[:], func=ACT.Relu,
                           scale=-1.0, bias=1.0)
            # upper bound: slot z 4*rzq+64+k <= 111  <=>  dk <= 111 - pz
            ub = qpool.tile([128, NSUB * 10], F32, tag="ub", name="ub")
            vec.tensor_scalar(out=ub[:].rearrange("p (s h) -> p s h", s=NSUB),
                              in0=pz, scalar1=-1.0, scalar2=111.0,
                              op0=OP.mult, op1=OP.add)
            vub = qpool.tile([128, NSUB * 10 * W], F32, tag="vub", name="vub")
            vec.tensor_tensor(
                out=vub[:].rearrange("p (sh k) -> p sh k", k=W),
                in0=dk[:].rearrange("p (sh k) -> p sh k", k=W),
                in1=ub[:].unsqueeze(2).to_broadcast([128, NSUB * 10, W]),
                op=OP.is_le)
            wz = qpool.tile([128, NSUB * 10 * W], F32, tag="wz", name="wz")
            vec.tensor_tensor(out=wz[:], in0=hat[:], in1=vub[:], op=OP.mult)

            # slot weights: block bx = min(gx0, 110); slot s holds x = bx+s.
            # When gx0 == 111 (x >= 47) the corner-0 weight moves to slot 1.
            wc6 = wcorn[:].rearrange("p (s hp a d) -> p s hp a d",
                                     s=NSUB, hp=10, a=3)
            ws = qpool.tile([128, NSUB * 40], F32, tag="ws", name="ws")
            ws4 = ws[:].rearrange("p (s hp a d) -> p s hp a d", s=NSUB, hp=10, a=2)
            hi = qpool.tile([128, NSUB * 20], F32, tag="hi", name="hi")
            hi3 = hi[:].rearrange("p (s hp a) -> p s hp a", s=NSUB, hp=10)
            hit = qpool.tile([128, NSUB * 20], F32, tag="hit", name="hit")
            hit3 = hit[:].rearrange("p (s hp a) -> p s hp a", s=NSUB, hp=10)
            # hi = (gc0 == 111): corner-0 weight moves to slot 1.
            # lo = (g1m == 63): corner-1 weight (position 64) moves to slot 0.
            vec.tensor_scalar(out=hi3, in0=gc6[:, :, :, 0:2, 0],
                              scalar1=111.0, scalar2=None, op0=OP.is_equal)
            vec.tensor_tensor(out=hit3, in0=wc6[:, :, :, 0:2, 0], in1=hi3,
                              op=OP.mult)
            lo = qpool.tile([128, NSUB * 20], F32, tag="lo", name="lo")
            lo3 = lo[:].rearrange("p (s hp a) -> p s hp a", s=NSUB, hp=10)
            lot = qpool.tile([128, NSUB * 20], F32, tag="lot", name="lot")
            lot3 = lot[:].rearrange("p (s hp a) -> p s hp a", s=NSUB, hp=10)
            g1m3 = g1m[:].rearrange("p (s hp a) -> p s hp a", s=NSUB, hp=10)
            vec.tensor_scalar(out=lo3, in0=g1m3[:, :, :, 0:2],
                              scalar1=63.0, scalar2=None, op0=OP.is_equal)
            vec.tensor_tensor(out=lot3, in0=wc6[:, :, :, 0:2, 1], in1=lo3,
                              op=OP.mult)
            vec.tensor_tensor(out=ws4[:, :, :, :, 0],
                              in0=wc6[:, :, :, 0:2, 0], in1=hit3,
                              op=OP.subtract)
            vec.tensor_tensor(out=ws4[:, :, :, :, 0],
                              in0=ws4[:, :, :, :, 0], in1=lot3, op=OP.add)
            vec.tensor_tensor(out=ws4[:, :, :, :, 1],
                              in0=wc6[:, :, :, 0:2, 1], in1=hit3, op=OP.add)
            vec.tensor_tensor(out=ws4[:, :, :, :, 1],
                              in0=ws4[:, :, :, :, 1], in1=lot3, op=OP.subtract)
            wxs = ws4[:, :, :, 0, :]                    # (s, hp, xslot)
            wys = ws4[:, :, :, 1, :]                    # (s, hp, yslot)

            # mask: m[(s hp), zb, xp, yp, z4] = aw*wxs*wys*wz
            m1 = qpool.tile([128, NSUB * 20], F32, tag="m1", name="m1")
            vec.tensor_tensor(
                out=m1[:].rearrange("p (sh xp) -> p sh xp", xp=2),
                in0=aw[:].unsqueeze(2).to_broadcast([128, NSUB * 10, 2]),
                in1=wxs.rearrange("p s hp xp -> p (s hp) xp"), op=OP.mult)
            m2 = qpool.tile([128, NSUB * 40], F32, tag="m2", name="m2")
            vec.tensor_tensor(
                out=m2[:].rearrange("p (sh xp yp) -> p sh xp yp", xp=2, yp=2),
                in0=m1[:].rearrange("p (sh xp) -> p sh xp", xp=2)
                    .unsqueeze(3).to_broadcast([128, NSUB * 10, 2, 2]),
                in1=wys.rearrange("p s hp yp -> p (s hp) yp")
                    .unsqueeze(2).to_broadcast([128, NSUB * 10, 2, 2]),
                op=OP.mult)
            # mtmp layout (sh, zb, z4, xy); all three APs are 3-free-dim
            mtmp = qpool.tile([128, NSUB * 10 * 32], VDT, tag="mtmp",
                              name="mtmp")
            vec.tensor_tensor(
                out=mtmp[:].rearrange("p (sh zz xy) -> p sh zz xy",
                                      zz=8, xy=4),
                in0=m2[:].rearrange("p (sh xy) -> p sh xy", xy=4)
                    .unsqueeze(2).to_broadcast([128, NSUB * 10, 8, 4]),
                in1=wz[:].rearrange("p (sh zz) -> p sh zz", zz=8)
                    .unsqueeze(3).to_broadcast([128, NSUB * 10, 8, 4]),
                op=OP.mult)
            # permute (zb, z4, xy) -> (zb, xy, z4); (sh, zb) merges both sides
            mv = maskb_all[:, g * MCOL:(g + 1) * MCOL]
            vec.tensor_copy(
                out=mv.rearrange("p (szb xy z4) -> p szb xy z4", xy=4, z4=4),
                in_=mtmp[:].rearrange("p (szb z4 xy) -> p szb xy z4",
                                      z4=4, xy=4))

            # gather row indices: 576*(bx-64) + 12*(by-64) + rzq
            bx = qpool.tile([128, NSUB * 10], F32, tag="bx", name="bx")
            vec.tensor_scalar(out=bx[:].rearrange("p (s h) -> p s h", s=NSUB),
                              in0=gc6[:, :, :, 0, 0], scalar1=110.0,
                              scalar2=576.0, op0=OP.min, op1=OP.mult)
            by = qpool.tile([128, NSUB * 10], F32, tag="by", name="by")
            vec.tensor_scalar(out=by[:].rearrange("p (s h) -> p s h", s=NSUB),
                              in0=gc6[:, :, :, 1, 0], scalar1=110.0,
                              scalar2=12.0, op0=OP.min, op1=OP.mult)
            t4 = qpool.tile([128, NSUB * 10], F32, tag="t4", name="t4")
            vec.tensor_tensor(out=t4[:], in0=bx[:], in1=by[:], op=OP.add)
            idxf = qpool.tile([128, NSUB * 10], F32, tag="idxf", name="idxf")
            vec.tensor_scalar(out=idxf[:], in0=t4[:],
                              scalar1=37632.0, scalar2=None, op0=OP.subtract)
            vec.tensor_tensor(out=idxf[:], in0=idxf[:], in1=rzq[:], op=OP.add)
            idx16 = qpool.tile([128, NSUB * 10], I16, tag="idx16", name="idx16")
            vec.tensor_copy(out=idx16[:], in_=idxf[:])

            # idx fold into the gather's 16-partition-wrapped layout:
            # bounce through DRAM with contiguous 320B packets, then a DVE
            # column permute (s2-major -> s2-interleaved).
            nc.sync.dma_start(out=idxscr[g * 128:(g + 1) * 128, :], in_=idx16[:])
            idxr = ipool.tile([128, ICOL], I16, tag="idxr", name="idxr")
            scr = idxscr[g * 128:(g + 1) * 128, :]
            for gg in range(8):
                # idxr[16*gg + p16, s2*40 + f] = idx16[s2*16 + p16, f]
                src = bass.AP(scr.tensor, scr.offset,
                              [[40, 16], [16 * 40, 8], [1, 40]])
                nc.sync.dma_start(
                    out=idxr[gg * 16:(gg + 1) * 16, :]
                        .rearrange("p (s2 f) -> p s2 f", s2=8),
                    in_=src)
            # permute cols: (s2, subhl, r) -> (subhl, r, s2)
            vec.tensor_copy(
                out=idxw_all[:, g * ICOL:(g + 1) * ICOL]
                    .rearrange("p (sh r s2) -> p sh r s2", sh=8, r=5),
                in_=idxr[:].rearrange("p (s2 sh r) -> p sh r s2", s2=8, sh=8))

        # ---- stage V: value projection (bf16) ----
        # Zero the 12 units at block (46, 47) whose (xp=1, yp=1) slots no
        # write pass covers but the z-window overrun can read, plus the back
        # guard.
        zpad = const.tile([16, 512], VDT, tag="zpad", name="zpad")
        vec.memset(zpad[:], 0.0)
        for hl in range(2):
            zb46 = (46 * 576 + 47 * 12)
            nc.sync.dma_start(
                out=bass.AP(vexp[:].tensor,
                            hl * VHEAD + (G0 + zb46) * 512 + 256 + 128,
                            [[512, 12], [1, 128]]),
                in_=zpad[0:12, 0:128])
            nc.sync.dma_start(
                out=bass.AP(vexp[:].tensor, hl * VHEAD + (G0 + NUNIT) * 512,
                            [[512, 16], [1, 512]]),
                in_=zpad[:])
        for vg in range(NVSUP // NR):          # flush groups of NR supertiles
            vb = [vpool.tile([128, NR * 256], VDT, tag=f"vb{hl}", name=f"vb{hl}")
                  for hl in range(2)]
            for i in range(NR):
                vt = vg * NR + i
                vin = [vpool.tile([128, VSUP], VDT, tag=f"vin{k}", name=f"vin{k}")
                       for k in range(2)]
                for k in range(2):
                    nc.sync.dma_start(
                        out=vin[k][:],
                        in_=value_in[k * 128:(k + 1) * 128,
                                     vt * VSUP:(vt + 1) * VSUP])
                psv = ps_v.tile([128, 512], F32, tag="psv", name="psv")
                for s in range(8):
                    lhs0 = vin[0][:].rearrange("p (v e) -> p e v", e=8)[:, s, :]
                    lhs1 = vin[1][:].rearrange("p (v e) -> p e v", e=8)[:, s, :]
                    nc.tensor.matmul(psv[:, s * 64:(s + 1) * 64], lhs0,
                                     wval_sb[0][:], start=True, stop=False)
                    nc.tensor.matmul(psv[:, s * 64:(s + 1) * 64], lhs1,
                                     wval_sb[1][:], start=False, stop=True)
                # split heads, add bias, pack (blk, c, v4) per 4-voxel block
                for hl in range(2):
                    src = psv[:].rearrange("p (blk v4 hc) -> p blk hc v4",
                                           blk=2, v4=4)[:, :, hl * 32:(hl + 1) * 32, :]
                    bv = bval_sb[:, hl * 32:(hl + 1) * 32] \
                        .unsqueeze(1).unsqueeze(3).to_broadcast([128, 2, 32, 4])
                    vec.tensor_tensor(
                        out=vb[hl][:, i * 256:(i + 1) * 256]
                            .rearrange("p (blk c v4) -> p blk c v4", blk=2, c=32),
                        in0=src, in1=bv, op=OP.add)
            # flush NR supertiles (NR*1024 voxels) per head: 4 quadrant
            # passes; pass (xp, yp) lands vblock at unit vblock-xp*576-yp*12
            # slot (xp, yp). Runs are 256B ((c, z4) per vblock).
            for hl in range(2):
                for xp in range(2):
                    for yp in range(2):
                        base = (hl * VHEAD
                                + (G0 + vg * NR * 256 - xp * 576 - yp * 12)
                                * 512 + xp * 256 + yp * 128)
                        for blk in range(2):
                            dst = bass.AP(vexp[:].tensor, base + blk * 512,
                                          [[1024, 128], [256 * 512, NR],
                                           [1, 128]])
                            src = vb[hl][:] \
                                .rearrange("p (i blk x) -> p i blk x",
                                           i=NR, blk=2)[:, :, blk, :]
                            eng = nc.sync if (xp * 2 + yp) % 2 == 0 \
                                else nc.scalar
                            eng.dma_start(out=dst, in_=src)

        # ---- stage G: gather + weighted reduce per (supertile, subtile, head) ----
        for g in range(NSUP):
            q0 = g * TQ
            S = qpool.tile([128, NSUB * 64], F32, tag="S", name="S")
            for s in range(NSUB):
                for hl in range(2):
                    G = gpool.tile([128, 5 * 1024], VDT, tag="G", name="G")
                    in_g = bass.AP(vexp[:].tensor, hl * VHEAD + G0 * 512,
                                   [[512, NUNIT], [1, 1024]])
                    nc.gpsimd.dma_gather(
                        out_ap=G[:].rearrange("p (i e) -> p i e", i=5),
                        in_ap=in_g,
                        idxs_ap=idxw_all[:, (g * 8 + s * 2 + hl) * 40:
                                         (g * 8 + s * 2 + hl + 1) * 40],
                        num_idxs=NIDX, num_idxs_reg=NIDX,
                        elem_size=1024, elem_step=512,
                        single_packet=False,
                        queue_num=(s * 2 + hl) % 4)
                    # P = G * mask in place; G row = (zb, xp, yp, c, z4),
                    # mask cols (pt, zb, xy, z4) bcast over c
                    moff = g * MCOL + (s * 2 + hl) * 160
                    mg = maskb_all[:, moff:moff + 160] \
                        .rearrange("p (rb z4) -> p rb z4", z4=4) \
                        .unsqueeze(2).to_broadcast([128, 40, 32, 4])
                    gv = G[:].rearrange("p (rb c z4) -> p rb c z4",
                                        rb=40, c=32)
                    vec.tensor_tensor(out=gv, in0=gv, in1=mg, op=OP.mult)
                    # single fused reduce over (rb, z4), keeping c
                    vec.tensor_reduce(
                        out=S[:, s * 64 + hl * 32:s * 64 + hl * 32 + 32],
                        in_=G[:].rearrange("p (rb c z4) -> p c rb z4",
                                           rb=40, c=32),
                        axis=AX.XY, op=OP.add)

            # transpose S [128, 64] -> [64, 128] per subtile
            for s in range(NSUB):
                pst = ps_t.tile([64, 128], F32, tag="pst", name="pst")
                nc.tensor.transpose(pst[:], S[:, s * 64:(s + 1) * 64], ident[:])
                act.activation(out=st_sb[:, q0 + s * 128:q0 + (s + 1) * 128],
                               in_=pst[:], func=ACT.Copy)

            # GEMM2 for this supertile: outT = wout^T @ ST (overlaps the
            # next supertile's gathers)
            for mc in range(2):
                ps2 = ps_c.tile([128, 512], F32, tag="ps2", name="ps2")
                nc.tensor.matmul(ps2[:],
                                 wout_sb[:, mc * 128:(mc + 1) * 128],
                                 st_sb[:, q0:q0 + TQ],
                                 start=True, stop=True)
                ob = opool.tile([128, 512], F32, tag="ob", name="ob")
                vec.tensor_copy(out=ob[:], in_=ps2[:])
                nc.sync.dma_start(
                    out=outp[mc * 128:(mc + 1) * 128, q0:q0 + TQ],
                    in_=ob[:])

    nc.compile()
    return nc


def _prep_core_inputs(inputs, b, j):
    import ml_dtypes
    q = np.ascontiguousarray(inputs["query"][b].T, np.float32)
    p = np.ascontiguousarray(inputs["pos"][b].T, np.float32)
    r = np.concatenate([inputs["reference_points"][b].T,
                        np.ones((1, NQ), np.float32)]).astype(np.float32)
    r = np.ascontiguousarray(r)
    value = np.ascontiguousarray(
        inputs["value"][b].reshape(C, NVOX)).astype(ml_dtypes.bfloat16)

    W_off, b_off = inputs["W_off"], inputs["b_off"]
    W_attn, b_attn = inputs["W_attn"], inputs["b_attn"]
    heads = [2 * j, 2 * j + 1]
    rows, biases, refr = [], [], []
    for h in heads:
        for pp in range(P):
            for ax in range(3):
                rows.append(W_off[(h * P + pp) * 3 + ax])
                biases.append(b_off[(h * P + pp) * 3 + ax] - 0.5 + 64.0)
                e = np.zeros(3, np.float32)
                e[ax] = GRID
                refr.append(e)
    for h in heads:
        for pp in range(P):
            rows.append(W_attn[h * P + pp])
            biases.append(b_attn[h * P + pp])
            refr.append(np.zeros(3, np.float32))
    wcat = np.ascontiguousarray(np.stack(rows).T, np.float32)       # (256, 40)
    ref_rhs = np.concatenate(
        [np.stack(refr).T, np.asarray(biases, np.float32)[None, :]])
    ref_rhs = np.ascontiguousarray(ref_rhs, np.float32)             # (4, 40)

    wval = np.ascontiguousarray(
        inputs["W_val"][64 * j:64 * j + 64].T).astype(ml_dtypes.bfloat16)
    bval = np.ascontiguousarray(
        np.repeat(inputs["b_val"][64 * j:64 * j + 64][None, :], 128, axis=0),
        np.float32)
    wout = np.ascontiguousarray(inputs["W_out"][:, 64 * j:64 * j + 64].T,
                                np.float32)
    zoffs = np.repeat(np.arange(W, dtype=np.float32)[None, :], 128, axis=0)
    return {
        "value_in": value, "qT": q, "pT": p, "refT": r,
        "wcat": wcat, "ref_rhs": ref_rhs,
        "wval": wval, "bval": bval, "wout": wout, "zoff": zoffs,
    }


def get_nc():
    global _NC_CACHE
    if _NC_CACHE is None:
        _NC_CACHE = build_nc()
    return _NC_CACHE


def kernel(**inputs):
    from concourse.bass_utils import run_bass_kernel_spmd

    inputs = {k: np.asarray(v) for k, v in inputs.items()}
    nc = get_nc()
    in_maps = [_prep_core_inputs(inputs, core // 4, core % 4) for core in range(8)]
    res = run_bass_kernel_spmd(nc, in_maps, list(range(8)))
    bs = inputs["query"].shape[0]
    out = np.zeros((bs, NQ, C), np.float32)
    for core in range(8):
        out[core // 4] += res.results[core]["outp"].T
    out += inputs["b_out"][None, None, :].astype(np.float32)
    return out
